# revision 23
# baseline (speedup 1.0000x reference)
"""GAttNHP model as a Bass/Tile kernel on 8 Trainium2 NeuronCores.

Strategy: pure data-parallel over batch (B=16 -> 2 batches/core, no
collectives).  bf16 matmuls accumulating in fp32 PSUM; the dominant
intensity head runs fp8e4 with DoubleRow (2 weights/cell).

Key structural move vs the straightforward lowering: the merge Linear is
folded into the intensity head ON THE HOST.  With
  enhanced = [enc | s_emb | r_emb | gathered] @ mg_w + mg_b
  out      = softplus(enhanced @ int_w + int_b)
we precompute W1 = mg_w[:512] @ int_w (enc part), W2 = mg_w[1024:] @ int_w
(gathered part) and a per-batch constant row (s/r embeddings are constant
per batch).  The device-side intensity matmul then has K = 512 (enc, fp8
DoubleRow, 2 instructions) + 64+1 (gather one-hot + const row, packed into
one more DoubleRow instruction) instead of K = 1024 + a separate [*,1088]
merge matmul.  The gather chunk's stationary is padded to the full 128
partitions (zero rows) so every matmul in a PSUM accumulation group covers
the same PE row-group (disjoint row-groups race on the PSUM accumulate and
fault the exec unit).

The intensity epilogue softplus = ln(1+exp(x)) is two full ACT passes over
every output element and is the hard floor of the kernel (~110us/core); the
schedule is arranged so the Scalar engine never idles during the intensity
phases: Exp per 1024-col strip from a 2-bank PSUM tile, one Ln per 2048-col
strip-pair, attention softmax normalization moved off ACT onto the DVE
(reciprocal_approx_fast on the appended-ones column sum), and ACT keeps a
single activation table set (natural_log_exp_and_others) throughout -- see
_pin_act_tables.

Device pipeline, emitted as a complete chain PER BATCH so batch 1's
latency-bound front half overlaps batch 0's ACT-bound intensity epilogue:
  1. AttNHP encoder, 2 layers, activations kept in transposed [d, t]
     layout (plus a natural [t, d] copy for the group scatter).  Causal
     softmax in s^T layout: exp (no max-subtract, scores are tiny),
     triangular mask on the diagonal block, column sums via an appended
     ones-column on v, normalization via DVE reciprocal + rank-1
     broadcast matmul.  Encoder output additionally cast to fp8 (enc8)
     as the intensity stationary.
  2. Group scatter-mean as a matmul against a host-built one-hot
     matrix (64 batch-local segments on partitions 0..63).
  3. Tiny group transformer block (attn + ffn + 2 layernorms; rstd via
     ln/exp so the whole kernel uses ONE ACT table set).
  4. Gather back gathc8[p,t] = gout[gid(t),p]*fm(t) via one matmul
     against the one-hot gather matrix, cast fp8, plus a preset
     all-ones row (pairs with the per-batch const row of W2).
  5. Intensity head: per (strip-pair, t-tile): 3 fp8 DoubleRow matmuls
     per 512-col half into 2-bank [128,1024] PSUM strips; Exp per
     strip, Ln per strip-pair, DMA out per strip-pair.
"""

import os

import numpy as np
import ml_dtypes

bf16 = ml_dtypes.bfloat16
f8 = ml_dtypes.float8_e4m3
INTW_SCALE = 128.0

N_ENTITY = 8000
N_REL = 100
N_GROUPS = 64
HIDDEN = 256
D_MODEL = 256
N_LAYERS = 2
N_HEADS = 4
GP = 64
GH = 2
D_TOTAL = D_MODEL * N_LAYERS          # 512
D_FEAT = D_TOTAL + 2 * HIDDEN         # 1024
B, L = 16, 512
Lh = L - 1                            # 511
NCORES = 8
BPC = B // NCORES                     # 2 batches per core
T = 512                               # padded seq length
NT = T // 128                         # 4 t-tiles per batch
R = BPC * T                           # 1024 rows per core
NSEG = N_GROUPS                       # 64 batch-local segments
NE_PAD = 8192
NSTRIP = 8                            # 1024-col strips (last covers 832)
NPAIR = NSTRIP // 2                   # 2048-col strip-pairs

LAST_EXEC_NS = None
LAST_RESULTS = None
_CACHED = {}


def _time_enc(t, d=D_MODEL):
    i = np.arange(d // 2)
    freqs = np.exp(-np.log(10000.0) * (2.0 * i / d)).astype(np.float32)
    ang = t[..., None].astype(np.float32) * freqs
    return np.concatenate([np.sin(ang), np.cos(ang)], axis=-1).astype(np.float32)


def _pack_T(a):
    # [512, 256] natural -> [128, 2, 512] transposed tiles (d = c*128+p)
    return np.ascontiguousarray(a.T.reshape(2, 128, T).transpose(1, 0, 2))


def _pack_N(a):
    # [512, 256] natural -> [128, 4, 256] natural tiles (t = m*128+p)
    return np.ascontiguousarray(a.reshape(NT, 128, D_MODEL).transpose(1, 0, 2))


def _wpack(w):
    # [256, 256] -> [128, 2, 256]  (rows d = c*128+p)
    return np.ascontiguousarray(w.reshape(2, 128, D_MODEL).transpose(1, 0, 2))


def prep_inputs(inputs):
    """Returns in_maps per core."""
    f32 = np.float32
    subs = np.asarray(inputs["subs"])
    marks = np.asarray(inputs["marks"])
    objs = np.asarray(inputs["objs"])
    times = np.asarray(inputs["times"], f32)
    dt = np.asarray(inputs["dt"], f32)
    mask = np.asarray(inputs["mask"])
    group_map = np.asarray(inputs["group_map"])
    g = lambda k: np.asarray(inputs[k], f32)
    obj_embed = g("obj_embed")
    core_Wq, core_Wk, core_Wv, core_Wo = (
        g("core_Wq"), g("core_Wk"), g("core_Wv"), g("core_Wo"))
    sub_embed, rel_embed = g("sub_embed"), g("rel_embed")
    gp_w, gp_b = g("gp_w"), g("gp_b")
    ga_in_w, ga_in_b = g("ga_in_w"), g("ga_in_b")
    ga_out_w, ga_out_b = g("ga_out_w"), g("ga_out_b")
    ffn_w1, ffn_b1, ffn_w2, ffn_b2 = g("ffn_w1"), g("ffn_b1"), g("ffn_w2"), g("ffn_b2")
    n1_w, n1_b, n2_w, n2_b = g("n1_w"), g("n1_b"), g("n2_w"), g("n2_b")
    mg_w, mg_b = g("mg_w"), g("mg_b")
    int_w, int_b = g("int_w"), g("int_b")

    # ---- host-fused intensity weights ----
    # enhanced = [enc | sr | gathered] @ mg_w + mg_b; out = sp(enh @ int_w + int_b)
    w1f = mg_w[:D_TOTAL] @ int_w                     # [512, 8000]
    w2f = mg_w[D_FEAT:D_FEAT + GP] @ int_w           # [64, 8000]
    mgb_row = mg_b @ int_w + int_b                   # [8000]

    shared = {}
    shared["wq"] = np.stack([_wpack(core_Wq[l] / np.sqrt(64.0))
                             for l in range(N_LAYERS)]).astype(bf16)
    shared["wk"] = np.stack([_wpack(core_Wk[l]) for l in range(N_LAYERS)]).astype(bf16)
    shared["wv"] = np.stack([_wpack(core_Wv[l]) for l in range(N_LAYERS)]).astype(bf16)
    shared["wo"] = np.stack([_wpack(core_Wo[l]) for l in range(N_LAYERS)]).astype(bf16)
    shared["gpw"] = np.ascontiguousarray(
        gp_w[:D_TOTAL].reshape(4, 128, GP).transpose(1, 0, 2)).astype(bf16)
    gain = ga_in_w.copy()
    gainb = ga_in_b.copy().reshape(3, GP).T.copy()   # [64, 3] columns q/k/v
    gain[:, :GP] /= np.sqrt(32.0)
    gainb[:, 0] /= np.sqrt(32.0)
    shared["gain"] = gain.astype(bf16)
    shared["gainb"] = gainb.astype(f32)
    shared["gaout"] = ga_out_w.astype(bf16)
    shared["gaoutb"] = ga_out_b.reshape(GP, 1).astype(f32)
    shared["fw1"] = ffn_w1.astype(bf16)
    shared["fw2"] = ffn_w2.astype(bf16)
    shared["fb1"] = ffn_b1.reshape(1, GP).astype(bf16)
    shared["fb2"] = ffn_b2.reshape(1, GP).astype(bf16)
    shared["lnw1"] = np.tile(n1_w, (NSEG, 1)).astype(f32)
    shared["lnb1"] = np.tile(n1_b, (NSEG, 1)).astype(f32)
    shared["lnw2"] = np.tile(n2_w, (NSEG, 1)).astype(f32)
    shared["lnb2"] = np.tile(n2_b, (NSEG, 1)).astype(f32)
    w1pad = np.zeros((D_TOTAL, NE_PAD), np.float32)
    w1pad[:, :N_ENTITY] = w1f * INTW_SCALE
    # device tile [128, 4, NE_PAD]: w1[p, c, n] = W1[c*128+p, n]
    shared["w1"] = np.ascontiguousarray(
        w1pad.reshape(4, 128, NE_PAD).transpose(1, 0, 2)).astype(f8)
    tri = (np.arange(128)[None, :] >= np.arange(128)[:, None])
    shared["tri"] = tri.astype(bf16)

    in_maps = []
    for core in range(NCORES):
        m = dict(shared)
        xT = np.zeros((BPC, 128, 2, T), np.float32)
        c0T = np.zeros((BPC, 128, 2, T), np.float32)
        c0n = np.zeros((BPC, 128, NT, D_MODEL), np.float32)
        mscT = np.zeros((128, BPC * NT, NSEG), np.float32)  # [p, b*m, seg]
        mga = np.zeros((NSEG, BPC, T), np.float32)          # [seg, b, t]
        gpbias = np.zeros((NSEG, BPC, GP), np.float32)
        w2c = np.zeros((BPC, 128, 2, NE_PAD), np.float32)
        for b in range(BPC):
            gb = core * BPC + b
            hist = objs[gb, :Lh]
            x_nat = np.zeros((T, D_MODEL), np.float32)
            x_nat[:Lh] = (obj_embed[hist] + _time_enc(times[gb, :Lh])
                          + _time_enc(dt[gb, :Lh]))
            cur0 = np.zeros((T, D_MODEL), np.float32)
            cur0[:Lh] = _time_enc(times[gb, 1:])
            xT[b] = _pack_T(x_nat)
            c0T[b] = _pack_T(cur0)
            c0n[b] = _pack_N(cur0)

            gids = group_map[subs[gb] * N_REL + marks[gb]][:Lh]
            fm = mask[gb, :Lh].astype(np.float32)
            cnt = np.bincount(gids, weights=fm, minlength=NSEG)
            ts = np.arange(Lh)
            mga[gids, b, ts] = fm
            msc = np.zeros((T, NSEG), np.float32)      # [t, seg]
            msc[ts, gids] = fm / np.maximum(cnt, 1.0)[gids]
            mscT[:, NT * b:NT * b + NT, :] = msc.reshape(
                NT, 128, NSEG).transpose(1, 0, 2)
            sr = np.concatenate([sub_embed[subs[gb, 0]], rel_embed[marks[gb, 0]]])
            nz = (cnt > 0).astype(np.float32)
            gpbias[:, b, :] = (nz[:, None] * (sr @ gp_w[D_TOTAL:D_FEAT])[None, :]
                               + gp_b[None, :])
            const_row = (sr @ mg_w[D_TOTAL:D_FEAT]) @ int_w + mgb_row  # [8000]
            # gather/const chunk rhs: rows (p, j): (0..63, 0) = W2, (64, 0) =
            # const row, rest zero; pairs with gathc8 on the device.
            w2c[b, 0:GP, 0, :N_ENTITY] = w2f * INTW_SCALE
            w2c[b, GP, 0, :N_ENTITY] = const_row * INTW_SCALE
        m["xT"] = xT.astype(bf16)
        m["c0T"] = c0T.astype(bf16)
        m["c0n"] = c0n.astype(bf16)
        m["mscT"] = mscT.astype(bf16)
        m["mgath"] = mga.astype(bf16)
        m["gpbias"] = gpbias
        m["w2c"] = w2c.astype(f8)
        in_maps.append(m)
    return in_maps


def _chain_gens(*gens):
    for g in gens:
        yield from g


def _pin_act_tables():
    # bacc assigns each InstActivation a table set greedily, which makes a
    # mixed Exp/Ln instruction stream alternate between exp_and_others and
    # natural_log -> one ~1.3us ACT_TABLE_LOAD per switch.  Empty every set
    # except natural_log_exp_and_others (which contains Exp/Ln/Copy/Identity/
    # Square -- everything we use) so the chooser is forced onto one set;
    # positional set ids are preserved.
    import concourse.bacc as bacc
    from concourse import hw_specs
    if getattr(bacc.get_activation_tables, "_pinned", False):
        return
    orig = hw_specs.get_activation_tables
    KEEP = "natural_log_exp_and_others"

    def pinned(arch):
        t = dict(orig(arch))
        return {k: (v if k == KEEP else set()) for k, v in t.items()}

    pinned._pinned = True
    bacc.get_activation_tables = pinned


def build_nc(debug_stop=99):
    import concourse.bacc as bacc
    import concourse.mybir as mybir
    import concourse.tile as tile
    from concourse import masks as cmasks
    _pin_act_tables()
    NORM_MODE = os.environ.get("BASS_NORM", "recip")

    dtb = mybir.dt.bfloat16
    dtf = mybir.dt.float32
    dt8 = mybir.dt.float8e4
    AF = mybir.ActivationFunctionType
    ALU = mybir.AluOpType
    AX = mybir.AxisListType
    DR = mybir.MatmulPerfMode.DoubleRow

    nc = bacc.Bacc()

    def din(name, shape, dt=dtb):
        return nc.dram_tensor(name, shape, dt, kind="ExternalInput")

    xT_d = din("xT", [BPC, 128, 2, T])
    c0T_d = din("c0T", [BPC, 128, 2, T])
    c0n_d = din("c0n", [BPC, 128, NT, D_MODEL])
    wq_d = din("wq", [N_LAYERS, 128, 2, D_MODEL])
    wk_d = din("wk", [N_LAYERS, 128, 2, D_MODEL])
    wv_d = din("wv", [N_LAYERS, 128, 2, D_MODEL])
    wo_d = din("wo", [N_LAYERS, 128, 2, D_MODEL])
    mscT_d = din("mscT", [128, BPC * NT, NSEG])
    mgath_d = din("mgath", [NSEG, BPC, T])
    gpw_d = din("gpw", [128, 4, GP])
    gpbias_d = din("gpbias", [NSEG, BPC, GP], mybir.dt.float32)
    gain_d = din("gain", [GP, 3 * GP])
    gainb_d = din("gainb", [GP, 3], mybir.dt.float32)
    gaout_d = din("gaout", [GP, GP])
    gaoutb_d = din("gaoutb", [GP, 1], mybir.dt.float32)
    fw1_d = din("fw1", [GP, GP])
    fw2_d = din("fw2", [GP, GP])
    fb1_d = din("fb1", [1, GP])
    fb2_d = din("fb2", [1, GP])
    lnw1_d = din("lnw1", [NSEG, GP], mybir.dt.float32)
    lnb1_d = din("lnb1", [NSEG, GP], mybir.dt.float32)
    lnw2_d = din("lnw2", [NSEG, GP], mybir.dt.float32)
    lnb2_d = din("lnb2", [NSEG, GP], mybir.dt.float32)
    w1_d = din("w1", [128, 4, NE_PAD], dt8)
    w2c_d = din("w2c", [BPC, 128, 2, NE_PAD], dt8)
    tri_d = din("tri", [128, 128])
    out_d = nc.dram_tensor("out", [R, N_ENTITY], mybir.dt.bfloat16,
                           kind="ExternalOutput")
    debug_dump = bool(os.environ.get("BASS_DEBUG_DUMP"))
    if debug_dump:
        encdbg_d = nc.dram_tensor("encdbg", [N_LAYERS, 128, 2, T],
                                  mybir.dt.bfloat16, kind="ExternalOutput")
        odbg_d = nc.dram_tensor("odbg", [N_LAYERS, 128, 2, T],
                                mybir.dt.bfloat16, kind="ExternalOutput")

    with tile.TileContext(nc) as tc:
        with (
            tc.tile_pool(name="persist", bufs=1) as pp,
            tc.tile_pool(name="work", bufs=2) as wp,
            tc.tile_pool(name="acts", bufs=5) as ap,
            tc.tile_pool(name="outp", bufs=3) as op,
            tc.tile_pool(name="psum", bufs=1, space="PSUM") as ps,
        ):
            def pt(shape, tag, dt=dtb):
                return pp.tile(shape, dt, tag=tag, name=tag)

            def dma(dst, src):
                nc.sync.dma_start(dst, src)

            # PSUM budget (16KB/partition = 8 banks), by tag:
            #   pi 2x[128,1024]f32 = 8KB; f 2x[128,512]f32 = 4KB;
            #   po 2x[65,512]f32 = 4KB.  (pb lives in the f tag)
            def ps_pi():
                return ps.tile([128, 1024], dtf, tag="pi", bufs=2, name="pi")

            def ps_f(shape, dt=dtf):
                return ps.tile(shape, dt, tag="f", bufs=2, name="psf",
                               padded_shape=[128, 512])

            def ps_po():
                return ps.tile([65, T], dtf, tag="po", bufs=2, name="po")

            # ---- constants in ----
            # Input DMAs are split into small per-queue chunks, ordered so
            # batch 0 layer 0's operands land first: the front can start
            # ~5us in instead of waiting ~20us for monolithic transfers.
            xT = [pt([128, 2, T], f"xT{b}") for b in range(BPC)]
            c0T = [pt([128, 2, T], f"c0T{b}") for b in range(BPC)]
            c0n = [pt([128, NT, D_MODEL], f"c0n{b}") for b in range(BPC)]
            wq = [pt([128, 2, D_MODEL], f"wq{l}") for l in range(N_LAYERS)]
            wk = [pt([128, 2, D_MODEL], f"wk{l}") for l in range(N_LAYERS)]
            wo = [pt([128, 2, D_MODEL], f"wo{l}") for l in range(N_LAYERS)]
            wv = [pt([128, 2, D_MODEL], f"wv{l}") for l in range(N_LAYERS)]
            tri_s = pt([128, 128], "tri")

            def dma_qkv_w(l):
                dma(wq[l][:], wq_d[l])
                dma(wk[l][:], wk_d[l])
                dma(wv[l][:], wv_d[l])
                dma(wo[l][:], wo_d[l])

            def dma_batch_in(b):
                dma(c0T[b][:], c0T_d[b])
                dma(xT[b][:], xT_d[b])
                dma(c0n[b][:], c0n_d[b])

            dma_qkv_w(0)
            dma_batch_in(0)
            dma(tri_s[:], tri_d[:])
            dma_qkv_w(1)
            dma_batch_in(1)
            mscT_s = pt([128, BPC * NT, NSEG], "mscT")
            dma(mscT_s[:], mscT_d[:])
            mgath_s = pt([NSEG, BPC, T], "mgath")
            dma(mgath_s[:], mgath_d[:])
            gpw_s = pt([128, 4, GP], "gpw")
            dma(gpw_s[:], gpw_d[:])
            gpbias_s = pt([NSEG, BPC, GP], "gpbias", dtf)
            dma(gpbias_s[:], gpbias_d[:])
            gain_s = pt([GP, 3 * GP], "gain")
            dma(gain_s[:], gain_d[:])
            gainb_s = pt([GP, 3], "gainb", dtf)
            dma(gainb_s[:], gainb_d[:])
            gaout_s = pt([GP, GP], "gaout")
            dma(gaout_s[:], gaout_d[:])
            gaoutb_s = pt([GP, 1], "gaoutb", dtf)
            dma(gaoutb_s[:], gaoutb_d[:])
            fw1_s = pt([GP, GP], "fw1")
            dma(fw1_s[:], fw1_d[:])
            fw2_s = pt([GP, GP], "fw2")
            dma(fw2_s[:], fw2_d[:])
            fb1_s = pt([1, GP], "fb1")
            dma(fb1_s[:], fb1_d[:])
            fb2_s = pt([1, GP], "fb2")
            dma(fb2_s[:], fb2_d[:])
            ln_s = {}
            for nm, d in [("lnw1", lnw1_d), ("lnb1", lnb1_d),
                          ("lnw2", lnw2_d), ("lnb2", lnb2_d)]:
                ln_s[nm] = pt([NSEG, GP], nm, dtf)
                dma(ln_s[nm][:], d[:])

            # intensity weights, split into per-queue chunks so all DMA
            # rings pull in parallel and early strips land first
            w2c_s = [pt([128, 2, NE_PAD], f"w2c{b}", dt8) for b in range(BPC)]
            w1_s = pt([128, 4, NE_PAD], "w1", dt8)
            for c in range(2):
                for blk in range(4):
                    cs = slice(2048 * blk, 2048 * blk + 2048)
                    dma(w2c_s[0][:, c, cs], w2c_d[0, :, c, cs])
            for blk in range(4):
                for c in range(4):
                    cs = slice(2048 * blk, 2048 * blk + 2048)
                    dma(w1_s[:, c, cs], w1_d[:, c, cs])
            for c in range(2):
                for blk in range(4):
                    cs = slice(2048 * blk, 2048 * blk + 2048)
                    dma(w2c_s[1][:, c, cs], w2c_d[1, :, c, cs])

            eps_s = pt([NSEG, 1], "eps", dtf)
            nc.gpsimd.memset(eps_s[:], 1e-5)
            ident = pt([128, 128], "ident")
            cmasks.make_identity(nc, ident[:])
            ones_r = pt([1, T], "ones_r")
            nc.gpsimd.memset(ones_r[:], 1.0)

            # gather/const stationary: [128, 2, T] fp8; row (64, 0) is the
            # all-ones row pairing with the const row of w2c; rows 65..127
            # and all of j=1 stay zero.
            gathc8 = [pt([128, 2, T], f"gathc8{b}", dt8) for b in range(BPC)]
            enc8 = [pt([128, 2 * N_LAYERS, T], f"enc8{b}", dt8)
                    for b in range(BPC)]
            for b in range(BPC):
                nc.gpsimd.memset(gathc8[b][:], 0.0)
                nc.gpsimd.memset(gathc8[b][GP:GP + 1, 0, :], 1.0)

            def layernorm(xin, wtile, btile, outf, outb):
                P = xin.shape[0]
                s1 = wp.tile([P, 1], dtf, tag="lns", name="lns")
                nc.vector.reduce_sum(s1[:], xin[:], axis=AX.X)
                mu = wp.tile([P, 1], dtf, tag="lnm", name="lnm")
                nc.vector.tensor_scalar_mul(mu[:], s1[:], 1.0 / GP)
                xc = wp.tile([P, GP], dtf, tag="lnxc", name="lnxc")
                nc.vector.tensor_scalar(xc[:], xin[:], mu[:], None,
                                        op0=ALU.subtract)
                sq = wp.tile([P, GP], dtf, tag="lnsq", name="lnsq")
                vs = wp.tile([P, 1], dtf, tag="lnvs", name="lnvs")
                nc.scalar.activation(sq[:], xc[:], AF.Square, accum_out=vs[:])
                lnv = wp.tile([P, 1], dtf, tag="lnlv", name="lnlv")
                nc.scalar.activation(lnv[:], vs[:], AF.Ln, scale=1.0 / GP,
                                     bias=eps_s[:P])
                rstd = wp.tile([P, 1], dtf, tag="lnrs", name="lnrs")
                nc.scalar.activation(rstd[:], lnv[:], AF.Exp, scale=-0.5)
                nc.vector.scalar_tensor_tensor(
                    outf[:], xc[:], rstd[:], wtile[:], op0=ALU.mult, op1=ALU.mult)
                nc.vector.tensor_add(outf[:], outf[:], btile[:])
                nc.vector.tensor_copy(outb[:], outf[:])

            # ==== software-pipelined emission ====
            # Engines execute their instruction streams strictly in order,
            # so overlap between independent work REQUIRES interleaving at
            # emission time.  Schedule:
            #   front(b0) ; [post(b0) || front(b1)-begin] ;
            #   [intensity(b0) || front(b1)-rest + post(b1)] ; intensity(b1)
            ST = [{"curT": c0T[b], "curn": c0n[b],
                   "encn": [None] * N_LAYERS} for b in range(BPC)]

            def enc_gen(b):
                # Batch 0's front runs while ACT is otherwise idle, so its
                # softmax normalization uses the ACT ln/exp trick (balancing
                # ACT ~ exps+norm against DVE ~ copies+muls).  Batch 1's
                # front overlaps batch 0's ACT-bound intensity epilogue, so
                # its normalization stays on the DVE reciprocal.
                on_act = (b == 0)

                def stage(dst, src):
                    nc.vector.tensor_copy(dst, src)

                for l in range(N_LAYERS):
                    curT, curn = ST[b]["curT"], ST[b]["curn"]
                    qT = ap.tile([128, 2, T], dtb, tag="qT", bufs=2, name="qT")
                    kT = ap.tile([128, 2, T], dtb, tag="kT", bufs=2, name="kT")
                    for c2 in range(2):
                        psq = ps_f([128, T])
                        for c in range(2):
                            nc.tensor.matmul(
                                psq[:], wq[l][:, c, 128 * c2:128 * c2 + 128],
                                curT[:, c, :], start=(c == 0), stop=(c == 1))
                        stage(qT[:, c2, :], psq[:])
                        psk = ps_f([128, T])
                        for c in range(2):
                            nc.tensor.matmul(
                                psk[:], wk[l][:, c, 128 * c2:128 * c2 + 128],
                                xT[b][:, c, :], start=(c == 0), stop=(c == 1))
                        stage(kT[:, c2, :], psk[:])
                        yield
                    vcat = []
                    for m in range(NT):
                        psv = ps_f([128, D_MODEL])
                        for c in range(2):
                            nc.tensor.matmul(
                                psv[:], xT[b][:, c, 128 * m:128 * m + 128],
                                wv[l][:, c, :], start=(c == 0), stop=(c == 1))
                        vc = ap.tile([128, N_HEADS, 65], dtb, tag=f"vcat{m}",
                                     bufs=2, name=f"vcat{m}")
                        stage(vc[:, :, 0:64],
                              psv[:].rearrange("p (h e) -> p h e", h=N_HEADS))
                        nc.vector.memset(vc[:, :, 64:65], 1.0)
                        vcat.append(vc)
                        if m == 1:
                            yield
                    yield
                    # Scores phase: all (h, j) score+exp+mask units emitted
                    # back-to-back so no engine queue ever stalls on a long
                    # cross-engine round-trip (engines run their queues in
                    # order; a waiting instruction blocks everything behind
                    # it, including the other batch's interleaved intensity
                    # work).
                    aTs = [[None] * NT for _ in range(N_HEADS)]
                    cnt = 0
                    for h in range(N_HEADS):
                        bp, hc = 64 * (h % 2), h // 2
                        for j in range(NT):
                            q0 = 128 * j
                            nq = T - q0
                            pss = ps_f([128, T])
                            nc.tensor.matmul(
                                pss[:, :nq],
                                kT[bp:bp + 64, hc, q0:q0 + 128],
                                qT[bp:bp + 64, hc, q0:T])
                            aT = ap.tile([128, T], dtb, tag=f"aT{b}{h}{j}",
                                         bufs=1, name="aT")
                            nc.scalar.activation(aT[:, :nq], pss[:, :nq], AF.Exp)
                            nc.vector.tensor_mul(
                                aT[:, 0:128], aT[:, 0:128], tri_s[:])
                            aTs[h][j] = aT
                            cnt += 1
                            if cnt % 3 == 0:
                                yield
                    # po phase per head, then softmax normalization: 1/colsum
                    # on DVE (keeps ACT free for softplus), rank-1 broadcast
                    # matmul, multiply.
                    oT = ap.tile([128, 2, T], dtb, tag="oT", bufs=2, name="oT")
                    for h in range(N_HEADS):
                        bp, hc = 64 * (h % 2), h // 2
                        po = ps_po()
                        for j in range(NT):
                            q0 = 128 * j
                            nc.tensor.matmul(
                                po[:, q0:T], vcat[j][:, h, :],
                                aTs[h][j][:, :T - q0],
                                start=(j == 0), stop=(j == NT - 1),
                                skip_group_check=True)
                        oraw = wp.tile([64, T], dtb, tag="oraw", name="oraw")
                        stage(oraw[:], po[0:64, :])
                        rbf = wp.tile([1, T], dtb, tag="rbf", name="rbf")
                        if on_act or NORM_MODE == "act":
                            rsb = wp.tile([1, T], dtf, tag="rsb", name="rsb")
                            nc.scalar.activation(rsb[:], po[64:65, :], AF.Ln)
                            nc.scalar.activation(rbf[:], rsb[:], AF.Exp,
                                                 scale=-1.0)
                        else:
                            # reciprocal_approx_fast is a custom DVE op
                            # (BITWISE_NOT seed); it reads garbage from PSUM,
                            # so stage the colsum row into SBUF first.
                            rcs = wp.tile([1, T], dtf, tag="rcs", name="rcs")
                            nc.vector.tensor_copy(rcs[:], po[64:65, :])
                            rr = wp.tile([1, T], dtf, tag="rr", name="rr")
                            nc.vector.reciprocal_approx_fast(rr[:], rcs[:])
                            nc.vector.tensor_copy(rbf[:], rr[:])
                        pb = ps_f([64, T])
                        nc.tensor.matmul(pb[:], ones_r[0:1, 0:64], rbf[:])
                        nc.vector.tensor_mul(oT[bp:bp + 64, hc, :], oraw[:],
                                             pb[:])
                        yield
                    if debug_dump and b == 0:
                        nc.sync.dma_start(odbg_d[l], oT[:])
                    eT = pt([128, 2, T], f"encT{l}{b}")
                    en = pt([128, NT, D_MODEL], f"encn{l}{b}")
                    for c2 in range(2):
                        psp = ps_f([128, T])
                        for c in range(2):
                            nc.tensor.matmul(
                                psp[:], wo[l][:, c, 128 * c2:128 * c2 + 128],
                                oT[:, c, :], start=(c == 0), stop=(c == 1))
                        nc.vector.tensor_add(eT[:, c2, :], curT[:, c2, :],
                                             psp[:])
                    nc.vector.tensor_copy(enc8[b][:, 2 * l:2 * l + 2, :],
                                          eT[:])
                    if debug_dump and b == 0:
                        nc.sync.dma_start(encdbg_d[l], eT[:])
                    yield
                    for m in range(NT):
                        psn = ps_f([128, D_MODEL])
                        for c in range(2):
                            nc.tensor.matmul(
                                psn[:], oT[:, c, 128 * m:128 * m + 128],
                                wo[l][:, c, :], start=(c == 0), stop=(c == 1))
                        nc.vector.tensor_add(en[:, m, :], curn[:, m, :], psn[:])
                        if m == 1:
                            yield
                    ST[b]["encn"][l] = en
                    ST[b]["curT"], ST[b]["curn"] = eT, en
                    yield

            def post_gen(b):
                encn = ST[b]["encn"]
                # ---- group scatter A^T[d, seg] (batch-local 64 segs) ----
                AT_s = wp.tile([128, 4, NSEG], dtb, tag="AT", name="AT")
                for dt_i in range(4):
                    l, c2 = dt_i // 2, dt_i % 2
                    pa = ps_f([128, NSEG])
                    for m in range(NT):
                        nc.tensor.matmul(
                            pa[:], encn[l][:, m, 128 * c2:128 * c2 + 128],
                            mscT_s[:, NT * b + m, :],
                            start=(m == 0), stop=(m == NT - 1))
                    nc.vector.tensor_copy(AT_s[:, dt_i, :], pa[:])
                    if dt_i % 2:
                        yield
                # ---------------- group block ([64, *]) ----------------
                pg = ps_f([NSEG, GP])
                for dt_i in range(4):
                    nc.tensor.matmul(pg[:], AT_s[:, dt_i, :], gpw_s[:, dt_i, :],
                                     start=(dt_i == 0), stop=(dt_i == 3))
                gp_f = wp.tile([NSEG, GP], dtf, tag="gp_f", name="gp_f")
                nc.vector.tensor_add(gp_f[:], pg[:], gpbias_s[:, b, :])
                gp_bf = wp.tile([NSEG, GP], dtb, tag="gp_bf", name="gp_bf")
                nc.vector.tensor_copy(gp_bf[:], gp_f[:])
                ptr = ps_f([GP, NSEG], dtb)
                nc.tensor.transpose(ptr[:], gp_bf[:], ident[0:NSEG, 0:NSEG])
                gpT = wp.tile([GP, NSEG], dtb, tag="gpT", name="gpT")
                nc.vector.tensor_copy(gpT[:], ptr[:])
                yield
                qkvT = []
                for i in range(3):
                    pq = ps_f([GP, NSEG])
                    nc.tensor.matmul(pq[:], gain_s[:, GP * i:GP * i + GP],
                                     gpT[:])
                    tq = wp.tile([GP, NSEG], dtb, tag=f"qkvT{i}",
                                 name=f"qkvT{i}")
                    nc.vector.tensor_scalar(
                        tq[:], pq[:], gainb_s[:, i:i + 1], None, op0=ALU.add)
                    qkvT.append(tq)
                yield
                vc2 = wp.tile([GP, GH, 33], dtb, tag="vc2", name="vc2")
                for h in range(GH):
                    pv = ps_f([GP, 32], dtb)
                    nc.tensor.transpose(
                        pv[:], qkvT[2][32 * h:32 * h + 32, :],
                        ident[32 * h:32 * h + 32, 32 * h:32 * h + 32])
                    nc.vector.tensor_copy(vc2[:, h, 0:32], pv[:])
                nc.vector.memset(vc2[:, :, 32:33], 1.0)
                yield
                oT2 = wp.tile([GP, NSEG], dtb, tag="oT2", name="oT2")
                for h in range(GH):
                    ps1 = ps_f([64, 64])
                    nc.tensor.matmul(
                        ps1[:], qkvT[1][32 * h:32 * h + 32, :],
                        qkvT[0][32 * h:32 * h + 32, :])
                    a2 = wp.tile([64, 64], dtb, tag="a2", name="a2")
                    nc.scalar.activation(a2[:], ps1[:], AF.Exp)
                    po2 = ps_f([33, 64])
                    nc.tensor.matmul(po2[:], vc2[:, h, :], a2[:])
                    r2 = wp.tile([1, 64], dtf, tag="r2", name="r2")
                    nc.scalar.activation(r2[:], po2[32:33, :], AF.Ln)
                    r2b = wp.tile([1, 64], dtb, tag="r2b", name="r2b")
                    nc.scalar.activation(r2b[:], r2[:], AF.Exp, scale=-1.0)
                    pb2 = ps_f([32, 64])
                    nc.tensor.matmul(pb2[:], ones_r[0:1, 0:32], r2b[:])
                    orw2 = wp.tile([32, 64], dtb, tag="orw2", name="orw2")
                    nc.vector.tensor_copy(orw2[:], po2[0:32, :])
                    nc.vector.tensor_mul(oT2[32 * h:32 * h + 32, :], orw2[:],
                                         pb2[:])
                    yield
                pga = ps_f([GP, NSEG])
                nc.tensor.matmul(pga[:], gaout_s[:], oT2[:])
                t1 = wp.tile([GP, NSEG], dtb, tag="t1", name="t1")
                nc.vector.scalar_tensor_tensor(
                    t1[:], pga[:], gaoutb_s[:, 0:1], gpT[:],
                    op0=ALU.add, op1=ALU.add)
                px1 = ps_f([NSEG, GP], dtb)
                nc.tensor.transpose(px1[:], t1[:], ident[0:GP, 0:GP])
                x1 = wp.tile([NSEG, GP], dtf, tag="x1", name="x1")
                nc.vector.tensor_copy(x1[:], px1[:])
                gn_f = wp.tile([NSEG, GP], dtf, tag="gn_f", name="gn_f")
                gn_b = wp.tile([NSEG, GP], dtb, tag="gn_b", name="gn_b")
                layernorm(x1, ln_s["lnw1"][0:NSEG], ln_s["lnb1"][0:NSEG],
                          gn_f, gn_b)
                yield
                pgt = ps_f([GP, NSEG], dtb)
                nc.tensor.transpose(pgt[:], gn_b[:], ident[0:NSEG, 0:NSEG])
                gnT = wp.tile([GP, NSEG], dtb, tag="gnT", name="gnT")
                nc.vector.tensor_copy(gnT[:], pgt[:])
                ph1 = ps_f([NSEG, GP])
                nc.tensor.matmul(ph1[:], ones_r[0:1, 0:NSEG], fb1_s[:],
                                 start=True, stop=False)
                nc.tensor.matmul(ph1[:], gnT[:], fw1_s[:], start=False,
                                 stop=True)
                h1b = wp.tile([NSEG, GP], dtb, tag="h1b", name="h1b")
                nc.vector.tensor_scalar_max(h1b[:], ph1[:], 0.0)
                ph1t = ps_f([GP, NSEG], dtb)
                nc.tensor.transpose(ph1t[:], h1b[:], ident[0:NSEG, 0:NSEG])
                h1T = wp.tile([GP, NSEG], dtb, tag="h1T", name="h1T")
                nc.vector.tensor_copy(h1T[:], ph1t[:])
                yield
                ph2 = ps_f([NSEG, GP])
                nc.tensor.matmul(ph2[:], ones_r[0:1, 0:NSEG], fb2_s[:],
                                 start=True, stop=False)
                nc.tensor.matmul(ph2[:], h1T[:], fw2_s[:], start=False,
                                 stop=True)
                x2 = wp.tile([NSEG, GP], dtf, tag="x2", name="x2")
                nc.vector.tensor_add(x2[:], ph2[:], gn_f[:])
                go_f = wp.tile([NSEG, GP], dtf, tag="go_f", name="go_f")
                go_b = wp.tile([NSEG, GP], dtb, tag="go_b", name="go_b")
                layernorm(x2, ln_s["lnw2"][0:NSEG], ln_s["lnb2"][0:NSEG],
                          go_f, go_b)
                yield
                # ---- gather^T: gathc8[p, 0, t] = gout[gid(t), p]*fm(t) ----
                pgh = ps_f([GP, T])
                nc.tensor.matmul(pgh[:], go_b[:], mgath_s[:, b, :])
                nc.vector.tensor_copy(gathc8[b][0:GP, 0, :], pgh[:])
                yield

            def int_gen(b):
                # -------- intensity head (fp8 DoubleRow, K=512+65) --------
                for P in range(NPAIR):
                    for m in range(NT):
                        ot = op.tile([128, 2048], dtb, tag="ot", name="ot")
                        width = 0
                        for s2 in range(2):
                            sp = 2 * P + s2
                            ncs = min(1024, N_ENTITY - 1024 * sp)
                            pi = ps_pi()
                            for h2 in range(2):
                                col0 = 1024 * sp + 512 * h2
                                ncol = min(512, N_ENTITY - col0)
                                if ncol <= 0:
                                    continue
                                pv = pi[:, 512 * h2:512 * h2 + ncol]
                                for c in range(2):
                                    nc.tensor.matmul(
                                        pv,
                                        enc8[b][:, 2 * c:2 * c + 2,
                                                128 * m:128 * m + 128],
                                        w1_s[:, 2 * c:2 * c + 2,
                                             col0:col0 + ncol],
                                        perf_mode=DR, start=(c == 0),
                                        stop=False, skip_group_check=True)
                                nc.tensor.matmul(
                                    pv,
                                    gathc8[b][:, :, 128 * m:128 * m + 128],
                                    w2c_s[b][:, :, col0:col0 + ncol],
                                    perf_mode=DR, start=False, stop=True,
                                    skip_group_check=True)
                            nc.scalar.activation(
                                ot[:, 1024 * s2:1024 * s2 + ncs],
                                pi[:, :ncs], AF.Exp, scale=1.0 / INTW_SCALE)
                            width += ncs
                        # softplus second half: one Ln per strip-pair
                        nc.scalar.activation(ot[:, :width], ot[:, :width],
                                             AF.Ln, bias=1.0)
                        nc.sync.dma_start(
                            out_d[T * b + 128 * m:T * b + 128 * m + 128,
                                  2048 * P:2048 * P + width],
                            ot[:, :width])
                        yield

            def drain(g):
                for _ in g:
                    pass

            def zip_all(*gens):
                gens = list(gens)
                while gens:
                    for g in list(gens):
                        if next(g, StopIteration) is StopIteration:
                            gens.remove(g)

            def zip_until(primary, filler):
                # pace on `primary`; advance `filler` one stage per step
                for _ in primary:
                    next(filler, None)

            # Schedule: front(b0) alone; post(b0) paced against front(b1)'s
            # beginning (b1's short-dependency q/k/score chunks execute on
            # the otherwise-idle PE while post(b0)'s serial group-block chain
            # runs); then intensity(b0) interleaved 1:2 with the REST of
            # front(b1)+post(b1) -- intensity units lead so their ACT work
            # is never queued behind unissued front(b1) instructions, and
            # the 2x g1 stepping lets post(b1) finish before intensity(b0)
            # drains so intensity(b1) starts immediately after.
            drain(enc_gen(0))
            g1 = (_chain_gens(enc_gen(1), post_gen(1))
                  if BPC > 1 else iter(()))
            if debug_stop >= 3:
                # post(b0) paced against front(b1) at 2:1 -- just enough b1
                # prologue to keep the PE warm during post(b0)'s serial
                # group-block chain, but few enough that intensity(b0)'s
                # first units aren't queued behind half of b1's layer 0.
                k0 = 0
                for _ in post_gen(0):
                    if k0 % 2 == 1:
                        next(g1, None)
                    k0 += 1
                if debug_stop >= 5:
                    # Intensity(b0) clumped 2 units : 3 front(b1) chunks.
                    # Two units = 24 back-to-back DoubleRow matmuls on the
                    # PE -- long enough to hold the PE's continuous-busy
                    # p-state ramp -- while front(b1)'s cross-engine
                    # dependency hops resolve a clump ahead of where the
                    # in-order queues consume them.
                    gi0 = int_gen(0)
                    alive = True
                    while alive:
                        for _ in range(2):
                            if next(gi0, StopIteration) is StopIteration:
                                alive = False
                                break
                        for _ in range(3):
                            next(g1, None)
                    drain(g1)
                    for bb in range(1, BPC):
                        drain(int_gen(bb))
                else:
                    drain(g1)
            elif BPC > 1:
                drain(g1)
    nc.compile()
    return nc


def _get_nc():
    if "nc" not in _CACHED:
        _CACHED["nc"] = build_nc()
    return _CACHED["nc"]


def _install_ntff_hook():
    """Best-effort: register the axon NTFF profile hook so trace=True works."""
    import sys, types
    if "antenv.axon_hooks" in sys.modules:
        return
    try:
        import antenv  # noqa
        from trn_agent_boot.trn_boot import _ntff_profile_via_ctypes
        mod = types.ModuleType("antenv.axon_hooks")
        hook = [_ntff_profile_via_ctypes("/opt/axon/libaxon_pjrt.so")]
        mod.set_axon_ntff_profile_hook = lambda h: hook.__setitem__(0, h)
        mod.get_axon_ntff_profile_hook = lambda: hook[0]
        sys.modules["antenv.axon_hooks"] = mod
    except Exception:
        pass


def kernel(**inputs):
    global LAST_EXEC_NS, LAST_RESULTS
    from concourse.bass_utils import run_bass_kernel_spmd

    in_maps = prep_inputs(inputs)
    nc = _get_nc()
    trace = bool(os.environ.get("BASS_TRACE"))
    if trace:
        _install_ntff_hook()
    res = run_bass_kernel_spmd(
        nc, in_maps, core_ids=list(range(NCORES)), trace=trace)
    LAST_RESULTS = res
    LAST_EXEC_NS = res.exec_time_ns
    out = np.empty((B, Lh, N_ENTITY), np.float32)
    for core in range(NCORES):
        o = res.results[core]["out"]
        for b in range(BPC):
            out[core * BPC + b] = o[T * b:T * b + Lh, :].astype(np.float32)
    return out


# revision 25
# speedup vs baseline: 1.2018x; 1.2018x over previous
"""GAttNHP model as a Bass/Tile kernel on 8 Trainium2 NeuronCores.

Strategy: pure data-parallel over batch (B=16 -> 2 batches/core, no
collectives).  bf16 matmuls accumulating in fp32 PSUM; the dominant
intensity head runs fp8e4 with DoubleRow (2 weights/cell).

Key structural move vs the straightforward lowering: the merge Linear is
folded into the intensity head ON THE HOST.  With
  enhanced = [enc | s_emb | r_emb | gathered] @ mg_w + mg_b
  out      = softplus(enhanced @ int_w + int_b)
we precompute W1 = mg_w[:512] @ int_w (enc part), W2 = mg_w[1024:] @ int_w
(gathered part) and a per-batch constant row (s/r embeddings are constant
per batch).  The device-side intensity matmul then has K = 512 (enc, fp8
DoubleRow, 2 instructions) + 64+1 (gather one-hot + const row, packed into
one more DoubleRow instruction) instead of K = 1024 + a separate [*,1088]
merge matmul.  The gather chunk's stationary is padded to the full 128
partitions (zero rows) so every matmul in a PSUM accumulation group covers
the same PE row-group (disjoint row-groups race on the PSUM accumulate and
fault the exec unit).

The intensity epilogue softplus = ln(1+exp(x)) is two full ACT passes over
every output element and is the hard floor of the kernel (~110us/core); the
schedule is arranged so the Scalar engine never idles during the intensity
phases: Exp per 1024-col strip from a 2-bank PSUM tile, one Ln per 2048-col
strip-pair, attention softmax normalization moved off ACT onto the DVE
(reciprocal_approx_fast on the appended-ones column sum), and ACT keeps a
single activation table set (natural_log_exp_and_others) throughout -- see
_pin_act_tables.

Device pipeline, emitted as a complete chain PER BATCH so batch 1's
latency-bound front half overlaps batch 0's ACT-bound intensity epilogue:
  1. AttNHP encoder, 2 layers, activations kept in transposed [d, t]
     layout (plus a natural [t, d] copy for the group scatter).  Causal
     softmax in s^T layout: exp (no max-subtract, scores are tiny),
     triangular mask on the diagonal block, column sums via an appended
     ones-column on v, normalization via DVE reciprocal + rank-1
     broadcast matmul.  Encoder output additionally cast to fp8 (enc8)
     as the intensity stationary.
  2. Group scatter-mean as a matmul against a host-built one-hot
     matrix (64 batch-local segments on partitions 0..63).
  3. Tiny group transformer block (attn + ffn + 2 layernorms; rstd via
     ln/exp so the whole kernel uses ONE ACT table set).
  4. Gather back gathc8[p,t] = gout[gid(t),p]*fm(t) via one matmul
     against the one-hot gather matrix, cast fp8, plus a preset
     all-ones row (pairs with the per-batch const row of W2).
  5. Intensity head: per (strip-pair, t-tile): 3 fp8 DoubleRow matmuls
     per 512-col half into 2-bank [128,1024] PSUM strips; Exp per
     strip, Ln per strip-pair, DMA out per strip-pair.
"""

import os

import numpy as np
import ml_dtypes

bf16 = ml_dtypes.bfloat16
f8 = ml_dtypes.float8_e4m3
INTW_SCALE = 128.0

N_ENTITY = 8000
N_REL = 100
N_GROUPS = 64
HIDDEN = 256
D_MODEL = 256
N_LAYERS = 2
N_HEADS = 4
GP = 64
GH = 2
D_TOTAL = D_MODEL * N_LAYERS          # 512
D_FEAT = D_TOTAL + 2 * HIDDEN         # 1024
B, L = 16, 512
Lh = L - 1                            # 511
NCORES = 8
BPC = B // NCORES                     # 2 batches per core
T = 512                               # padded seq length
NT = T // 128                         # 4 t-tiles per batch
R = BPC * T                           # 1024 rows per core
NSEG = N_GROUPS                       # 64 batch-local segments
NE_PAD = 8192
NSTRIP = 8                            # 1024-col strips (last covers 832)
NPAIR = NSTRIP // 2                   # 2048-col strip-pairs

LAST_EXEC_NS = None
LAST_RESULTS = None
_CACHED = {}


def _time_enc(t, d=D_MODEL):
    i = np.arange(d // 2)
    freqs = np.exp(-np.log(10000.0) * (2.0 * i / d)).astype(np.float32)
    ang = t[..., None].astype(np.float32) * freqs
    return np.concatenate([np.sin(ang), np.cos(ang)], axis=-1).astype(np.float32)


def _pack_T(a):
    # [512, 256] natural -> [128, 2, 512] transposed tiles (d = c*128+p)
    return np.ascontiguousarray(a.T.reshape(2, 128, T).transpose(1, 0, 2))


def _pack_N(a):
    # [512, 256] natural -> [128, 4, 256] natural tiles (t = m*128+p)
    return np.ascontiguousarray(a.reshape(NT, 128, D_MODEL).transpose(1, 0, 2))


def _wpack(w):
    # [256, 256] -> [128, 2, 256]  (rows d = c*128+p)
    return np.ascontiguousarray(w.reshape(2, 128, D_MODEL).transpose(1, 0, 2))


def prep_inputs(inputs):
    """Returns in_maps per core."""
    f32 = np.float32
    subs = np.asarray(inputs["subs"])
    marks = np.asarray(inputs["marks"])
    objs = np.asarray(inputs["objs"])
    times = np.asarray(inputs["times"], f32)
    dt = np.asarray(inputs["dt"], f32)
    mask = np.asarray(inputs["mask"])
    group_map = np.asarray(inputs["group_map"])
    g = lambda k: np.asarray(inputs[k], f32)
    obj_embed = g("obj_embed")
    core_Wq, core_Wk, core_Wv, core_Wo = (
        g("core_Wq"), g("core_Wk"), g("core_Wv"), g("core_Wo"))
    sub_embed, rel_embed = g("sub_embed"), g("rel_embed")
    gp_w, gp_b = g("gp_w"), g("gp_b")
    ga_in_w, ga_in_b = g("ga_in_w"), g("ga_in_b")
    ga_out_w, ga_out_b = g("ga_out_w"), g("ga_out_b")
    ffn_w1, ffn_b1, ffn_w2, ffn_b2 = g("ffn_w1"), g("ffn_b1"), g("ffn_w2"), g("ffn_b2")
    n1_w, n1_b, n2_w, n2_b = g("n1_w"), g("n1_b"), g("n2_w"), g("n2_b")
    mg_w, mg_b = g("mg_w"), g("mg_b")
    int_w, int_b = g("int_w"), g("int_b")

    # ---- host-fused intensity weights ----
    # enhanced = [enc | sr | gathered] @ mg_w + mg_b; out = sp(enh @ int_w + int_b)
    w1f = mg_w[:D_TOTAL] @ int_w                     # [512, 8000]
    w2f = mg_w[D_FEAT:D_FEAT + GP] @ int_w           # [64, 8000]
    mgb_row = mg_b @ int_w + int_b                   # [8000]

    shared = {}
    shared["wq"] = np.stack([_wpack(core_Wq[l] / np.sqrt(64.0))
                             for l in range(N_LAYERS)]).astype(bf16)
    shared["wk"] = np.stack([_wpack(core_Wk[l]) for l in range(N_LAYERS)]).astype(bf16)
    shared["wv"] = np.stack([_wpack(core_Wv[l]) for l in range(N_LAYERS)]).astype(bf16)
    shared["wo"] = np.stack([_wpack(core_Wo[l]) for l in range(N_LAYERS)]).astype(bf16)
    shared["gpw"] = np.ascontiguousarray(
        gp_w[:D_TOTAL].reshape(4, 128, GP).transpose(1, 0, 2)).astype(bf16)
    gain = ga_in_w.copy()
    gainb = ga_in_b.copy().reshape(3, GP).T.copy()   # [64, 3] columns q/k/v
    gain[:, :GP] /= np.sqrt(32.0)
    gainb[:, 0] /= np.sqrt(32.0)
    shared["gain"] = gain.astype(bf16)
    shared["gainb"] = gainb.astype(f32)
    shared["gaout"] = ga_out_w.astype(bf16)
    shared["gaoutb"] = ga_out_b.reshape(GP, 1).astype(f32)
    shared["fw1"] = ffn_w1.astype(bf16)
    shared["fw2"] = ffn_w2.astype(bf16)
    shared["fb1"] = ffn_b1.reshape(1, GP).astype(bf16)
    shared["fb2"] = ffn_b2.reshape(1, GP).astype(bf16)
    shared["lnw1"] = np.tile(n1_w, (NSEG, 1)).astype(f32)
    shared["lnb1"] = np.tile(n1_b, (NSEG, 1)).astype(f32)
    shared["lnw2"] = np.tile(n2_w, (NSEG, 1)).astype(f32)
    shared["lnb2"] = np.tile(n2_b, (NSEG, 1)).astype(f32)
    w1pad = np.zeros((D_TOTAL, NE_PAD), np.float32)
    w1pad[:, :N_ENTITY] = w1f * INTW_SCALE
    # device tile [128, 4, NE_PAD]: w1[p, c, n] = W1[c*128+p, n]
    shared["w1"] = np.ascontiguousarray(
        w1pad.reshape(4, 128, NE_PAD).transpose(1, 0, 2)).astype(f8)
    tri = (np.arange(128)[None, :] >= np.arange(128)[:, None])
    shared["tri"] = tri.astype(bf16)

    in_maps = []
    for core in range(NCORES):
        m = dict(shared)
        xT = np.zeros((BPC, 128, 2, T), np.float32)
        c0T = np.zeros((BPC, 128, 2, T), np.float32)
        c0n = np.zeros((BPC, 128, NT, D_MODEL), np.float32)
        mscT = np.zeros((128, BPC * NT, NSEG), np.float32)  # [p, b*m, seg]
        mga = np.zeros((NSEG, BPC, T), np.float32)          # [seg, b, t]
        gpbias = np.zeros((NSEG, BPC, GP), np.float32)
        w2c = np.zeros((BPC, 128, 2, NE_PAD), np.float32)
        for b in range(BPC):
            gb = core * BPC + b
            hist = objs[gb, :Lh]
            x_nat = np.zeros((T, D_MODEL), np.float32)
            x_nat[:Lh] = (obj_embed[hist] + _time_enc(times[gb, :Lh])
                          + _time_enc(dt[gb, :Lh]))
            cur0 = np.zeros((T, D_MODEL), np.float32)
            cur0[:Lh] = _time_enc(times[gb, 1:])
            xT[b] = _pack_T(x_nat)
            c0T[b] = _pack_T(cur0)
            c0n[b] = _pack_N(cur0)

            gids = group_map[subs[gb] * N_REL + marks[gb]][:Lh]
            fm = mask[gb, :Lh].astype(np.float32)
            cnt = np.bincount(gids, weights=fm, minlength=NSEG)
            ts = np.arange(Lh)
            mga[gids, b, ts] = fm
            msc = np.zeros((T, NSEG), np.float32)      # [t, seg]
            msc[ts, gids] = fm / np.maximum(cnt, 1.0)[gids]
            mscT[:, NT * b:NT * b + NT, :] = msc.reshape(
                NT, 128, NSEG).transpose(1, 0, 2)
            sr = np.concatenate([sub_embed[subs[gb, 0]], rel_embed[marks[gb, 0]]])
            nz = (cnt > 0).astype(np.float32)
            gpbias[:, b, :] = (nz[:, None] * (sr @ gp_w[D_TOTAL:D_FEAT])[None, :]
                               + gp_b[None, :])
            const_row = (sr @ mg_w[D_TOTAL:D_FEAT]) @ int_w + mgb_row  # [8000]
            # gather/const chunk rhs: rows (p, j): (0..63, 0) = W2, (64, 0) =
            # const row, rest zero; pairs with gathc8 on the device.
            w2c[b, 0:GP, 0, :N_ENTITY] = w2f * INTW_SCALE
            w2c[b, GP, 0, :N_ENTITY] = const_row * INTW_SCALE
        m["xT"] = xT.astype(bf16)
        m["c0T"] = c0T.astype(bf16)
        m["c0n"] = c0n.astype(bf16)
        m["mscT"] = mscT.astype(bf16)
        m["mgath"] = mga.astype(bf16)
        m["gpbias"] = gpbias
        m["w2c"] = w2c.astype(f8)
        in_maps.append(m)
    return in_maps


def _chain_gens(*gens):
    for g in gens:
        yield from g


def _pin_act_tables():
    # bacc assigns each InstActivation a table set greedily, which makes a
    # mixed Exp/Ln instruction stream alternate between exp_and_others and
    # natural_log -> one ~1.3us ACT_TABLE_LOAD per switch.  Empty every set
    # except natural_log_exp_and_others (which contains Exp/Ln/Copy/Identity/
    # Square -- everything we use) so the chooser is forced onto one set;
    # positional set ids are preserved.
    import concourse.bacc as bacc
    from concourse import hw_specs
    if getattr(bacc.get_activation_tables, "_pinned", False):
        return
    orig = hw_specs.get_activation_tables
    KEEP = "natural_log_exp_and_others"

    def pinned(arch):
        t = dict(orig(arch))
        return {k: (v if k == KEEP else set()) for k, v in t.items()}

    pinned._pinned = True
    bacc.get_activation_tables = pinned


def build_nc(debug_stop=99):
    import concourse.bacc as bacc
    import concourse.mybir as mybir
    import concourse.tile as tile
    from concourse import masks as cmasks
    _pin_act_tables()
    NORM_MODE = os.environ.get("BASS_NORM", "recip")

    dtb = mybir.dt.bfloat16
    dtf = mybir.dt.float32
    dt8 = mybir.dt.float8e4
    AF = mybir.ActivationFunctionType
    ALU = mybir.AluOpType
    AX = mybir.AxisListType
    DR = mybir.MatmulPerfMode.DoubleRow

    nc = bacc.Bacc()

    def din(name, shape, dt=dtb):
        return nc.dram_tensor(name, shape, dt, kind="ExternalInput")

    xT_d = din("xT", [BPC, 128, 2, T])
    c0T_d = din("c0T", [BPC, 128, 2, T])
    c0n_d = din("c0n", [BPC, 128, NT, D_MODEL])
    wq_d = din("wq", [N_LAYERS, 128, 2, D_MODEL])
    wk_d = din("wk", [N_LAYERS, 128, 2, D_MODEL])
    wv_d = din("wv", [N_LAYERS, 128, 2, D_MODEL])
    wo_d = din("wo", [N_LAYERS, 128, 2, D_MODEL])
    mscT_d = din("mscT", [128, BPC * NT, NSEG])
    mgath_d = din("mgath", [NSEG, BPC, T])
    gpw_d = din("gpw", [128, 4, GP])
    gpbias_d = din("gpbias", [NSEG, BPC, GP], mybir.dt.float32)
    gain_d = din("gain", [GP, 3 * GP])
    gainb_d = din("gainb", [GP, 3], mybir.dt.float32)
    gaout_d = din("gaout", [GP, GP])
    gaoutb_d = din("gaoutb", [GP, 1], mybir.dt.float32)
    fw1_d = din("fw1", [GP, GP])
    fw2_d = din("fw2", [GP, GP])
    fb1_d = din("fb1", [1, GP])
    fb2_d = din("fb2", [1, GP])
    lnw1_d = din("lnw1", [NSEG, GP], mybir.dt.float32)
    lnb1_d = din("lnb1", [NSEG, GP], mybir.dt.float32)
    lnw2_d = din("lnw2", [NSEG, GP], mybir.dt.float32)
    lnb2_d = din("lnb2", [NSEG, GP], mybir.dt.float32)
    w1_d = din("w1", [128, 4, NE_PAD], dt8)
    w2c_d = din("w2c", [BPC, 128, 2, NE_PAD], dt8)
    tri_d = din("tri", [128, 128])
    out_d = nc.dram_tensor("out", [R, N_ENTITY], mybir.dt.bfloat16,
                           kind="ExternalOutput")
    debug_dump = bool(os.environ.get("BASS_DEBUG_DUMP"))
    if debug_dump:
        encdbg_d = nc.dram_tensor("encdbg", [N_LAYERS, 128, 2, T],
                                  mybir.dt.bfloat16, kind="ExternalOutput")
        odbg_d = nc.dram_tensor("odbg", [N_LAYERS, 128, 2, T],
                                mybir.dt.bfloat16, kind="ExternalOutput")

    with tile.TileContext(nc) as tc:
        with (
            tc.tile_pool(name="persist", bufs=1) as pp,
            tc.tile_pool(name="work", bufs=2) as wp,
            tc.tile_pool(name="acts", bufs=5) as ap,
            tc.tile_pool(name="outp", bufs=3) as op,
            tc.tile_pool(name="psum", bufs=1, space="PSUM") as ps,
        ):
            def pt(shape, tag, dt=dtb):
                return pp.tile(shape, dt, tag=tag, name=tag)

            def dma(dst, src):
                nc.sync.dma_start(dst, src)

            # PSUM budget (16KB/partition = 8 banks), by tag:
            #   pi 2x[128,1024]f32 = 8KB; f 2x[128,512]f32 = 4KB;
            #   po 2x[65,512]f32 = 4KB.  (pb lives in the f tag)
            def ps_pi():
                return ps.tile([128, 1024], dtf, tag="pi", bufs=2, name="pi")

            def ps_f(shape, dt=dtf):
                return ps.tile(shape, dt, tag="f", bufs=2, name="psf",
                               padded_shape=[128, 512])

            def ps_po():
                return ps.tile([65, T], dtf, tag="po", bufs=2, name="po")

            # ---- constants in ----
            # Input DMAs are split into small per-queue chunks, ordered so
            # batch 0 layer 0's operands land first: the front can start
            # ~5us in instead of waiting ~20us for monolithic transfers.
            xT = [pt([128, 2, T], f"xT{b}") for b in range(BPC)]
            c0T = [pt([128, 2, T], f"c0T{b}") for b in range(BPC)]
            c0n = [pt([128, NT, D_MODEL], f"c0n{b}") for b in range(BPC)]
            wq = [pt([128, 2, D_MODEL], f"wq{l}") for l in range(N_LAYERS)]
            wk = [pt([128, 2, D_MODEL], f"wk{l}") for l in range(N_LAYERS)]
            wo = [pt([128, 2, D_MODEL], f"wo{l}") for l in range(N_LAYERS)]
            wv = [pt([128, 2, D_MODEL], f"wv{l}") for l in range(N_LAYERS)]
            tri_s = pt([128, 128], "tri")

            def dma_qkv_w(l):
                dma(wq[l][:], wq_d[l])
                dma(wk[l][:], wk_d[l])
                dma(wv[l][:], wv_d[l])
                dma(wo[l][:], wo_d[l])

            def dma_batch_in(b):
                dma(c0T[b][:], c0T_d[b])
                dma(xT[b][:], xT_d[b])
                dma(c0n[b][:], c0n_d[b])

            dma_qkv_w(0)
            dma_batch_in(0)
            dma(tri_s[:], tri_d[:])
            dma_qkv_w(1)
            dma_batch_in(1)
            mscT_s = pt([128, BPC * NT, NSEG], "mscT")
            dma(mscT_s[:], mscT_d[:])
            mgath_s = pt([NSEG, BPC, T], "mgath")
            dma(mgath_s[:], mgath_d[:])
            gpw_s = pt([128, 4, GP], "gpw")
            dma(gpw_s[:], gpw_d[:])
            gpbias_s = pt([NSEG, BPC, GP], "gpbias", dtf)
            dma(gpbias_s[:], gpbias_d[:])
            gain_s = pt([GP, 3 * GP], "gain")
            dma(gain_s[:], gain_d[:])
            gainb_s = pt([GP, 3], "gainb", dtf)
            dma(gainb_s[:], gainb_d[:])
            gaout_s = pt([GP, GP], "gaout")
            dma(gaout_s[:], gaout_d[:])
            gaoutb_s = pt([GP, 1], "gaoutb", dtf)
            dma(gaoutb_s[:], gaoutb_d[:])
            fw1_s = pt([GP, GP], "fw1")
            dma(fw1_s[:], fw1_d[:])
            fw2_s = pt([GP, GP], "fw2")
            dma(fw2_s[:], fw2_d[:])
            fb1_s = pt([1, GP], "fb1")
            dma(fb1_s[:], fb1_d[:])
            fb2_s = pt([1, GP], "fb2")
            dma(fb2_s[:], fb2_d[:])
            ln_s = {}
            for nm, d in [("lnw1", lnw1_d), ("lnb1", lnb1_d),
                          ("lnw2", lnw2_d), ("lnb2", lnb2_d)]:
                ln_s[nm] = pt([NSEG, GP], nm, dtf)
                dma(ln_s[nm][:], d[:])

            # intensity weights, split into per-queue chunks so all DMA
            # rings pull in parallel and early strips land first
            w2c_s = [pt([128, 2, NE_PAD], f"w2c{b}", dt8) for b in range(BPC)]
            w1_s = pt([128, 4, NE_PAD], "w1", dt8)
            for c in range(2):
                for blk in range(4):
                    cs = slice(2048 * blk, 2048 * blk + 2048)
                    dma(w2c_s[0][:, c, cs], w2c_d[0, :, c, cs])
            for blk in range(4):
                for c in range(4):
                    cs = slice(2048 * blk, 2048 * blk + 2048)
                    dma(w1_s[:, c, cs], w1_d[:, c, cs])
            for c in range(2):
                for blk in range(4):
                    cs = slice(2048 * blk, 2048 * blk + 2048)
                    dma(w2c_s[1][:, c, cs], w2c_d[1, :, c, cs])

            eps_s = pt([NSEG, 1], "eps", dtf)
            nc.gpsimd.memset(eps_s[:], 1e-5)
            ident = pt([128, 128], "ident")
            cmasks.make_identity(nc, ident[:])
            ones_r = pt([1, T], "ones_r")
            nc.gpsimd.memset(ones_r[:], 1.0)

            # gather/const stationary: [128, 2, T] fp8; row (64, 0) is the
            # all-ones row pairing with the const row of w2c; rows 65..127
            # and all of j=1 stay zero.
            gathc8 = [pt([128, 2, T], f"gathc8{b}", dt8) for b in range(BPC)]
            enc8 = [pt([128, 2 * N_LAYERS, T], f"enc8{b}", dt8)
                    for b in range(BPC)]
            for b in range(BPC):
                nc.gpsimd.memset(gathc8[b][:], 0.0)
                nc.gpsimd.memset(gathc8[b][GP:GP + 1, 0, :], 1.0)

            def layernorm(xin, wtile, btile, outf, outb):
                P = xin.shape[0]
                s1 = wp.tile([P, 1], dtf, tag="lns", name="lns")
                nc.vector.reduce_sum(s1[:], xin[:], axis=AX.X)
                mu = wp.tile([P, 1], dtf, tag="lnm", name="lnm")
                nc.vector.tensor_scalar_mul(mu[:], s1[:], 1.0 / GP)
                xc = wp.tile([P, GP], dtf, tag="lnxc", name="lnxc")
                nc.vector.tensor_scalar(xc[:], xin[:], mu[:], None,
                                        op0=ALU.subtract)
                sq = wp.tile([P, GP], dtf, tag="lnsq", name="lnsq")
                vs = wp.tile([P, 1], dtf, tag="lnvs", name="lnvs")
                nc.scalar.activation(sq[:], xc[:], AF.Square, accum_out=vs[:])
                lnv = wp.tile([P, 1], dtf, tag="lnlv", name="lnlv")
                nc.scalar.activation(lnv[:], vs[:], AF.Ln, scale=1.0 / GP,
                                     bias=eps_s[:P])
                rstd = wp.tile([P, 1], dtf, tag="lnrs", name="lnrs")
                nc.scalar.activation(rstd[:], lnv[:], AF.Exp, scale=-0.5)
                nc.vector.scalar_tensor_tensor(
                    outf[:], xc[:], rstd[:], wtile[:], op0=ALU.mult, op1=ALU.mult)
                nc.vector.tensor_add(outf[:], outf[:], btile[:])
                nc.vector.tensor_copy(outb[:], outf[:])

            # ==== software-pipelined emission ====
            # Engines execute their instruction streams strictly in order,
            # so overlap between independent work REQUIRES interleaving at
            # emission time.  Schedule:
            #   front(b0) ; [post(b0) || front(b1)-begin] ;
            #   [intensity(b0) || front(b1)-rest + post(b1)] ; intensity(b1)
            ST = [{"curT": c0T[b], "curn": c0n[b],
                   "encn": [None] * N_LAYERS} for b in range(BPC)]

            def enc_gen(b):
                # Batch 0's front runs while ACT is idle: staging copies and
                # softmax normalization go on ACT to shorten the critical
                # chain.  Batch 1's front overlaps batch 0's ACT-bound
                # intensity epilogue, so everything movable stays on the DVE.
                on_act = (b == 0)

                def stage(dst, src):
                    if on_act:
                        nc.scalar.activation(dst, src, AF.Copy)
                    else:
                        nc.vector.tensor_copy(dst, src)

                for l in range(N_LAYERS):
                    curT, curn = ST[b]["curT"], ST[b]["curn"]
                    qT = ap.tile([128, 2, T], dtb, tag="qT", bufs=2, name="qT")
                    kT = ap.tile([128, 2, T], dtb, tag="kT", bufs=2, name="kT")
                    for c2 in range(2):
                        psq = ps_f([128, T])
                        for c in range(2):
                            nc.tensor.matmul(
                                psq[:], wq[l][:, c, 128 * c2:128 * c2 + 128],
                                curT[:, c, :], start=(c == 0), stop=(c == 1))
                        stage(qT[:, c2, :], psq[:])
                        psk = ps_f([128, T])
                        for c in range(2):
                            nc.tensor.matmul(
                                psk[:], wk[l][:, c, 128 * c2:128 * c2 + 128],
                                xT[b][:, c, :], start=(c == 0), stop=(c == 1))
                        stage(kT[:, c2, :], psk[:])
                        yield
                    vcat = []
                    for m in range(NT):
                        psv = ps_f([128, D_MODEL])
                        for c in range(2):
                            nc.tensor.matmul(
                                psv[:], xT[b][:, c, 128 * m:128 * m + 128],
                                wv[l][:, c, :], start=(c == 0), stop=(c == 1))
                        vc = ap.tile([128, N_HEADS, 65], dtb, tag=f"vcat{m}",
                                     bufs=2, name=f"vcat{m}")
                        stage(vc[:, :, 0:64],
                              psv[:].rearrange("p (h e) -> p h e", h=N_HEADS))
                        nc.vector.memset(vc[:, :, 64:65], 1.0)
                        vcat.append(vc)
                        if m == 1:
                            yield
                    yield
                    # Scores phase: all (h, j) score+exp+mask units emitted
                    # back-to-back so no engine queue ever stalls on a long
                    # cross-engine round-trip (engines run their queues in
                    # order; a waiting instruction blocks everything behind
                    # it, including the other batch's interleaved intensity
                    # work).
                    aTs = [[None] * NT for _ in range(N_HEADS)]
                    cnt = 0
                    for h in range(N_HEADS):
                        bp, hc = 64 * (h % 2), h // 2
                        for j in range(NT):
                            q0 = 128 * j
                            nq = T - q0
                            pss = ps_f([128, T])
                            nc.tensor.matmul(
                                pss[:, :nq],
                                kT[bp:bp + 64, hc, q0:q0 + 128],
                                qT[bp:bp + 64, hc, q0:T])
                            aT = ap.tile([128, T], dtb, tag=f"aT{b}{h}{j}",
                                         bufs=1, name="aT")
                            nc.scalar.activation(aT[:, :nq], pss[:, :nq], AF.Exp)
                            nc.vector.tensor_mul(
                                aT[:, 0:128], aT[:, 0:128], tri_s[:])
                            aTs[h][j] = aT
                            cnt += 1
                            if cnt % 3 == 0:
                                yield
                    # po phase per head, then softmax normalization: 1/colsum
                    # on DVE (keeps ACT free for softplus), rank-1 broadcast
                    # matmul, multiply.
                    oT = ap.tile([128, 2, T], dtb, tag="oT", bufs=2, name="oT")
                    for h in range(N_HEADS):
                        bp, hc = 64 * (h % 2), h // 2
                        po = ps_po()
                        for j in range(NT):
                            q0 = 128 * j
                            nc.tensor.matmul(
                                po[:, q0:T], vcat[j][:, h, :],
                                aTs[h][j][:, :T - q0],
                                start=(j == 0), stop=(j == NT - 1),
                                skip_group_check=True)
                        oraw = wp.tile([64, T], dtb, tag="oraw", name="oraw")
                        stage(oraw[:], po[0:64, :])
                        rbf = wp.tile([1, T], dtb, tag="rbf", name="rbf")
                        if on_act or NORM_MODE == "act":
                            rsb = wp.tile([1, T], dtf, tag="rsb", name="rsb")
                            nc.scalar.activation(rsb[:], po[64:65, :], AF.Ln)
                            nc.scalar.activation(rbf[:], rsb[:], AF.Exp,
                                                 scale=-1.0)
                        else:
                            # reciprocal_approx_fast is a custom DVE op
                            # (BITWISE_NOT seed); it reads garbage from PSUM,
                            # so stage the colsum row into SBUF first.
                            rcs = wp.tile([1, T], dtf, tag="rcs", name="rcs")
                            nc.vector.tensor_copy(rcs[:], po[64:65, :])
                            rr = wp.tile([1, T], dtf, tag="rr", name="rr")
                            nc.vector.reciprocal_approx_fast(rr[:], rcs[:])
                            nc.vector.tensor_copy(rbf[:], rr[:])
                        pb = ps_f([64, T])
                        nc.tensor.matmul(pb[:], ones_r[0:1, 0:64], rbf[:])
                        nc.vector.tensor_mul(oT[bp:bp + 64, hc, :], oraw[:],
                                             pb[:])
                        yield
                    if debug_dump and b == 0:
                        nc.sync.dma_start(odbg_d[l], oT[:])
                    eT = pt([128, 2, T], f"encT{l}{b}")
                    en = pt([128, NT, D_MODEL], f"encn{l}{b}")
                    for c2 in range(2):
                        psp = ps_f([128, T])
                        for c in range(2):
                            nc.tensor.matmul(
                                psp[:], wo[l][:, c, 128 * c2:128 * c2 + 128],
                                oT[:, c, :], start=(c == 0), stop=(c == 1))
                        nc.vector.tensor_add(eT[:, c2, :], curT[:, c2, :],
                                             psp[:])
                    nc.vector.tensor_copy(enc8[b][:, 2 * l:2 * l + 2, :],
                                          eT[:])
                    if debug_dump and b == 0:
                        nc.sync.dma_start(encdbg_d[l], eT[:])
                    yield
                    for m in range(NT):
                        psn = ps_f([128, D_MODEL])
                        for c in range(2):
                            nc.tensor.matmul(
                                psn[:], oT[:, c, 128 * m:128 * m + 128],
                                wo[l][:, c, :], start=(c == 0), stop=(c == 1))
                        nc.vector.tensor_add(en[:, m, :], curn[:, m, :], psn[:])
                        if m == 1:
                            yield
                    ST[b]["encn"][l] = en
                    ST[b]["curT"], ST[b]["curn"] = eT, en
                    yield

            def post_gen(b):
                encn = ST[b]["encn"]
                # ---- group scatter A^T[d, seg] (batch-local 64 segs) ----
                AT_s = wp.tile([128, 4, NSEG], dtb, tag="AT", name="AT")
                for dt_i in range(4):
                    l, c2 = dt_i // 2, dt_i % 2
                    pa = ps_f([128, NSEG])
                    for m in range(NT):
                        nc.tensor.matmul(
                            pa[:], encn[l][:, m, 128 * c2:128 * c2 + 128],
                            mscT_s[:, NT * b + m, :],
                            start=(m == 0), stop=(m == NT - 1))
                    nc.vector.tensor_copy(AT_s[:, dt_i, :], pa[:])
                    if dt_i % 2:
                        yield
                # ---------------- group block ([64, *]) ----------------
                pg = ps_f([NSEG, GP])
                for dt_i in range(4):
                    nc.tensor.matmul(pg[:], AT_s[:, dt_i, :], gpw_s[:, dt_i, :],
                                     start=(dt_i == 0), stop=(dt_i == 3))
                gp_f = wp.tile([NSEG, GP], dtf, tag="gp_f", name="gp_f")
                nc.vector.tensor_add(gp_f[:], pg[:], gpbias_s[:, b, :])
                gp_bf = wp.tile([NSEG, GP], dtb, tag="gp_bf", name="gp_bf")
                nc.vector.tensor_copy(gp_bf[:], gp_f[:])
                ptr = ps_f([GP, NSEG], dtb)
                nc.tensor.transpose(ptr[:], gp_bf[:], ident[0:NSEG, 0:NSEG])
                gpT = wp.tile([GP, NSEG], dtb, tag="gpT", name="gpT")
                nc.vector.tensor_copy(gpT[:], ptr[:])
                yield
                qkvT = []
                for i in range(3):
                    pq = ps_f([GP, NSEG])
                    nc.tensor.matmul(pq[:], gain_s[:, GP * i:GP * i + GP],
                                     gpT[:])
                    tq = wp.tile([GP, NSEG], dtb, tag=f"qkvT{i}",
                                 name=f"qkvT{i}")
                    nc.vector.tensor_scalar(
                        tq[:], pq[:], gainb_s[:, i:i + 1], None, op0=ALU.add)
                    qkvT.append(tq)
                yield
                vc2 = wp.tile([GP, GH, 33], dtb, tag="vc2", name="vc2")
                for h in range(GH):
                    pv = ps_f([GP, 32], dtb)
                    nc.tensor.transpose(
                        pv[:], qkvT[2][32 * h:32 * h + 32, :],
                        ident[32 * h:32 * h + 32, 32 * h:32 * h + 32])
                    nc.vector.tensor_copy(vc2[:, h, 0:32], pv[:])
                nc.vector.memset(vc2[:, :, 32:33], 1.0)
                yield
                oT2 = wp.tile([GP, NSEG], dtb, tag="oT2", name="oT2")
                for h in range(GH):
                    ps1 = ps_f([64, 64])
                    nc.tensor.matmul(
                        ps1[:], qkvT[1][32 * h:32 * h + 32, :],
                        qkvT[0][32 * h:32 * h + 32, :])
                    a2 = wp.tile([64, 64], dtb, tag="a2", name="a2")
                    nc.scalar.activation(a2[:], ps1[:], AF.Exp)
                    po2 = ps_f([33, 64])
                    nc.tensor.matmul(po2[:], vc2[:, h, :], a2[:])
                    r2 = wp.tile([1, 64], dtf, tag="r2", name="r2")
                    nc.scalar.activation(r2[:], po2[32:33, :], AF.Ln)
                    r2b = wp.tile([1, 64], dtb, tag="r2b", name="r2b")
                    nc.scalar.activation(r2b[:], r2[:], AF.Exp, scale=-1.0)
                    pb2 = ps_f([32, 64])
                    nc.tensor.matmul(pb2[:], ones_r[0:1, 0:32], r2b[:])
                    orw2 = wp.tile([32, 64], dtb, tag="orw2", name="orw2")
                    nc.vector.tensor_copy(orw2[:], po2[0:32, :])
                    nc.vector.tensor_mul(oT2[32 * h:32 * h + 32, :], orw2[:],
                                         pb2[:])
                    yield
                pga = ps_f([GP, NSEG])
                nc.tensor.matmul(pga[:], gaout_s[:], oT2[:])
                t1 = wp.tile([GP, NSEG], dtb, tag="t1", name="t1")
                nc.vector.scalar_tensor_tensor(
                    t1[:], pga[:], gaoutb_s[:, 0:1], gpT[:],
                    op0=ALU.add, op1=ALU.add)
                px1 = ps_f([NSEG, GP], dtb)
                nc.tensor.transpose(px1[:], t1[:], ident[0:GP, 0:GP])
                x1 = wp.tile([NSEG, GP], dtf, tag="x1", name="x1")
                nc.vector.tensor_copy(x1[:], px1[:])
                gn_f = wp.tile([NSEG, GP], dtf, tag="gn_f", name="gn_f")
                gn_b = wp.tile([NSEG, GP], dtb, tag="gn_b", name="gn_b")
                layernorm(x1, ln_s["lnw1"][0:NSEG], ln_s["lnb1"][0:NSEG],
                          gn_f, gn_b)
                yield
                pgt = ps_f([GP, NSEG], dtb)
                nc.tensor.transpose(pgt[:], gn_b[:], ident[0:NSEG, 0:NSEG])
                gnT = wp.tile([GP, NSEG], dtb, tag="gnT", name="gnT")
                nc.vector.tensor_copy(gnT[:], pgt[:])
                ph1 = ps_f([NSEG, GP])
                nc.tensor.matmul(ph1[:], ones_r[0:1, 0:NSEG], fb1_s[:],
                                 start=True, stop=False)
                nc.tensor.matmul(ph1[:], gnT[:], fw1_s[:], start=False,
                                 stop=True)
                h1b = wp.tile([NSEG, GP], dtb, tag="h1b", name="h1b")
                nc.vector.tensor_scalar_max(h1b[:], ph1[:], 0.0)
                ph1t = ps_f([GP, NSEG], dtb)
                nc.tensor.transpose(ph1t[:], h1b[:], ident[0:NSEG, 0:NSEG])
                h1T = wp.tile([GP, NSEG], dtb, tag="h1T", name="h1T")
                nc.vector.tensor_copy(h1T[:], ph1t[:])
                yield
                ph2 = ps_f([NSEG, GP])
                nc.tensor.matmul(ph2[:], ones_r[0:1, 0:NSEG], fb2_s[:],
                                 start=True, stop=False)
                nc.tensor.matmul(ph2[:], h1T[:], fw2_s[:], start=False,
                                 stop=True)
                x2 = wp.tile([NSEG, GP], dtf, tag="x2", name="x2")
                nc.vector.tensor_add(x2[:], ph2[:], gn_f[:])
                go_f = wp.tile([NSEG, GP], dtf, tag="go_f", name="go_f")
                go_b = wp.tile([NSEG, GP], dtb, tag="go_b", name="go_b")
                layernorm(x2, ln_s["lnw2"][0:NSEG], ln_s["lnb2"][0:NSEG],
                          go_f, go_b)
                yield
                # ---- gather^T: gathc8[p, 0, t] = gout[gid(t), p]*fm(t) ----
                pgh = ps_f([GP, T])
                nc.tensor.matmul(pgh[:], go_b[:], mgath_s[:, b, :])
                nc.vector.tensor_copy(gathc8[b][0:GP, 0, :], pgh[:])
                yield

            def int_gen(b):
                # -------- intensity head (fp8 DoubleRow, K=512+65) --------
                for P in range(NPAIR):
                    for m in range(NT):
                        ot = op.tile([128, 2048], dtb, tag="ot", name="ot")
                        width = 0
                        for s2 in range(2):
                            sp = 2 * P + s2
                            ncs = min(1024, N_ENTITY - 1024 * sp)
                            pi = ps_pi()
                            for h2 in range(2):
                                col0 = 1024 * sp + 512 * h2
                                ncol = min(512, N_ENTITY - col0)
                                if ncol <= 0:
                                    continue
                                pv = pi[:, 512 * h2:512 * h2 + ncol]
                                for c in range(2):
                                    nc.tensor.matmul(
                                        pv,
                                        enc8[b][:, 2 * c:2 * c + 2,
                                                128 * m:128 * m + 128],
                                        w1_s[:, 2 * c:2 * c + 2,
                                             col0:col0 + ncol],
                                        perf_mode=DR, start=(c == 0),
                                        stop=False, skip_group_check=True)
                                nc.tensor.matmul(
                                    pv,
                                    gathc8[b][:, :, 128 * m:128 * m + 128],
                                    w2c_s[b][:, :, col0:col0 + ncol],
                                    perf_mode=DR, start=False, stop=True,
                                    skip_group_check=True)
                            nc.scalar.activation(
                                ot[:, 1024 * s2:1024 * s2 + ncs],
                                pi[:, :ncs], AF.Exp, scale=1.0 / INTW_SCALE)
                            width += ncs
                        # softplus second half: one Ln per strip-pair
                        nc.scalar.activation(ot[:, :width], ot[:, :width],
                                             AF.Ln, bias=1.0)
                        nc.sync.dma_start(
                            out_d[T * b + 128 * m:T * b + 128 * m + 128,
                                  2048 * P:2048 * P + width],
                            ot[:, :width])
                        yield

            def drain(g):
                for _ in g:
                    pass

            def zip_all(*gens):
                gens = list(gens)
                while gens:
                    for g in list(gens):
                        if next(g, StopIteration) is StopIteration:
                            gens.remove(g)

            def zip_until(primary, filler):
                # pace on `primary`; advance `filler` one stage per step
                for _ in primary:
                    next(filler, None)

            # Schedule: front(b0) alone; post(b0) paced against front(b1)'s
            # beginning (b1's short-dependency q/k/score chunks execute on
            # the otherwise-idle PE while post(b0)'s serial group-block chain
            # runs); then intensity(b0) interleaved 1:2 with the REST of
            # front(b1)+post(b1) -- intensity units lead so their ACT work
            # is never queued behind unissued front(b1) instructions, and
            # the 2x g1 stepping lets post(b1) finish before intensity(b0)
            # drains so intensity(b1) starts immediately after.
            drain(enc_gen(0))
            g1 = (_chain_gens(enc_gen(1), post_gen(1))
                  if BPC > 1 else iter(()))
            if debug_stop >= 3:
                zip_until(post_gen(0), g1)
                if debug_stop >= 5:
                    # Intensity(b0) units lead the remaining front(b1)
                    # chunks 1:2 -- the units' ACT work is never queued
                    # behind unissued front(b1) instructions, and the double
                    # g1 stepping lets post(b1) finish before intensity(b0)
                    # drains.
                    gi0 = int_gen(0)
                    next(gi0, None)
                    next(gi0, None)
                    while True:
                        if next(gi0, StopIteration) is StopIteration:
                            gi0 = None
                            break
                        if (next(g1, StopIteration) is StopIteration or
                                next(g1, StopIteration) is StopIteration):
                            break
                    drain(g1)
                    if gi0 is not None:
                        drain(gi0)
                    for bb in range(1, BPC):
                        drain(int_gen(bb))
                else:
                    drain(g1)
            elif BPC > 1:
                drain(g1)
    nc.compile()
    return nc


def _get_nc():
    if "nc" not in _CACHED:
        _CACHED["nc"] = build_nc()
    return _CACHED["nc"]


def _install_ntff_hook():
    """Best-effort: register the axon NTFF profile hook so trace=True works."""
    import sys, types
    if "antenv.axon_hooks" in sys.modules:
        return
    try:
        import antenv  # noqa
        from trn_agent_boot.trn_boot import _ntff_profile_via_ctypes
        mod = types.ModuleType("antenv.axon_hooks")
        hook = [_ntff_profile_via_ctypes("/opt/axon/libaxon_pjrt.so")]
        mod.set_axon_ntff_profile_hook = lambda h: hook.__setitem__(0, h)
        mod.get_axon_ntff_profile_hook = lambda: hook[0]
        sys.modules["antenv.axon_hooks"] = mod
    except Exception:
        pass


def kernel(**inputs):
    global LAST_EXEC_NS, LAST_RESULTS
    from concourse.bass_utils import run_bass_kernel_spmd

    in_maps = prep_inputs(inputs)
    nc = _get_nc()
    trace = bool(os.environ.get("BASS_TRACE"))
    if trace:
        _install_ntff_hook()
    res = run_bass_kernel_spmd(
        nc, in_maps, core_ids=list(range(NCORES)), trace=trace)
    LAST_RESULTS = res
    LAST_EXEC_NS = res.exec_time_ns
    out = np.empty((B, Lh, N_ENTITY), np.float32)
    for core in range(NCORES):
        o = res.results[core]["out"]
        for b in range(BPC):
            out[core * BPC + b] = o[T * b:T * b + Lh, :].astype(np.float32)
    return out


# revision 26
# speedup vs baseline: 1.2116x; 1.0082x over previous
"""GAttNHP model as a Bass/Tile kernel on 8 Trainium2 NeuronCores.

Strategy: pure data-parallel over batch (B=16 -> 2 batches/core, no
collectives).  bf16 matmuls accumulating in fp32 PSUM; the dominant
intensity head runs fp8e4 with DoubleRow (2 weights/cell).

Key structural move vs the straightforward lowering: the merge Linear is
folded into the intensity head ON THE HOST.  With
  enhanced = [enc | s_emb | r_emb | gathered] @ mg_w + mg_b
  out      = softplus(enhanced @ int_w + int_b)
we precompute W1 = mg_w[:512] @ int_w (enc part), W2 = mg_w[1024:] @ int_w
(gathered part) and a per-batch constant row (s/r embeddings are constant
per batch).  The device-side intensity matmul then has K = 512 (enc, fp8
DoubleRow, 2 instructions) + 64+1 (gather one-hot + const row, packed into
one more DoubleRow instruction) instead of K = 1024 + a separate [*,1088]
merge matmul.  The gather chunk's stationary is padded to the full 128
partitions (zero rows) so every matmul in a PSUM accumulation group covers
the same PE row-group (disjoint row-groups race on the PSUM accumulate and
fault the exec unit).

The intensity epilogue softplus = ln(1+exp(x)) is two full ACT passes over
every output element and is the hard floor of the kernel (~110us/core); the
schedule is arranged so the Scalar engine never idles during the intensity
phases: Exp per 1024-col strip from a 2-bank PSUM tile, one Ln per 2048-col
strip-pair, attention softmax normalization moved off ACT onto the DVE
(reciprocal_approx_fast on the appended-ones column sum), and ACT keeps a
single activation table set (natural_log_exp_and_others) throughout -- see
_pin_act_tables.

Device pipeline, emitted as a complete chain PER BATCH so batch 1's
latency-bound front half overlaps batch 0's ACT-bound intensity epilogue:
  1. AttNHP encoder, 2 layers, activations kept in transposed [d, t]
     layout (plus a natural [t, d] copy for the group scatter).  Causal
     softmax in s^T layout: exp (no max-subtract, scores are tiny),
     triangular mask on the diagonal block, column sums via an appended
     ones-column on v, normalization via DVE reciprocal + rank-1
     broadcast matmul.  Encoder output additionally cast to fp8 (enc8)
     as the intensity stationary.
  2. Group scatter-mean as a matmul against a host-built one-hot
     matrix (64 batch-local segments on partitions 0..63).
  3. Tiny group transformer block (attn + ffn + 2 layernorms; rstd via
     ln/exp so the whole kernel uses ONE ACT table set).
  4. Gather back gathc8[p,t] = gout[gid(t),p]*fm(t) via one matmul
     against the one-hot gather matrix, cast fp8, plus a preset
     all-ones row (pairs with the per-batch const row of W2).
  5. Intensity head: per (strip-pair, t-tile): 3 fp8 DoubleRow matmuls
     per 512-col half into 2-bank [128,1024] PSUM strips; Exp per
     strip, Ln per strip-pair, DMA out per strip-pair.
"""

import os

import numpy as np
import ml_dtypes

bf16 = ml_dtypes.bfloat16
f8 = ml_dtypes.float8_e4m3
INTW_SCALE = 128.0

N_ENTITY = 8000
N_REL = 100
N_GROUPS = 64
HIDDEN = 256
D_MODEL = 256
N_LAYERS = 2
N_HEADS = 4
GP = 64
GH = 2
D_TOTAL = D_MODEL * N_LAYERS          # 512
D_FEAT = D_TOTAL + 2 * HIDDEN         # 1024
B, L = 16, 512
Lh = L - 1                            # 511
NCORES = 8
BPC = B // NCORES                     # 2 batches per core
T = 512                               # padded seq length
NT = T // 128                         # 4 t-tiles per batch
R = BPC * T                           # 1024 rows per core
NSEG = N_GROUPS                       # 64 batch-local segments
NE_PAD = 8192
NSTRIP = 8                            # 1024-col strips (last covers 832)
NPAIR = NSTRIP // 2                   # 2048-col strip-pairs

LAST_EXEC_NS = None
LAST_RESULTS = None
_CACHED = {}


def _time_enc(t, d=D_MODEL):
    i = np.arange(d // 2)
    freqs = np.exp(-np.log(10000.0) * (2.0 * i / d)).astype(np.float32)
    ang = t[..., None].astype(np.float32) * freqs
    return np.concatenate([np.sin(ang), np.cos(ang)], axis=-1).astype(np.float32)


def _pack_T(a):
    # [512, 256] natural -> [128, 2, 512] transposed tiles (d = c*128+p)
    return np.ascontiguousarray(a.T.reshape(2, 128, T).transpose(1, 0, 2))


def _pack_N(a):
    # [512, 256] natural -> [128, 4, 256] natural tiles (t = m*128+p)
    return np.ascontiguousarray(a.reshape(NT, 128, D_MODEL).transpose(1, 0, 2))


def _wpack(w):
    # [256, 256] -> [128, 2, 256]  (rows d = c*128+p)
    return np.ascontiguousarray(w.reshape(2, 128, D_MODEL).transpose(1, 0, 2))


def prep_inputs(inputs):
    """Returns in_maps per core."""
    f32 = np.float32
    subs = np.asarray(inputs["subs"])
    marks = np.asarray(inputs["marks"])
    objs = np.asarray(inputs["objs"])
    times = np.asarray(inputs["times"], f32)
    dt = np.asarray(inputs["dt"], f32)
    mask = np.asarray(inputs["mask"])
    group_map = np.asarray(inputs["group_map"])
    g = lambda k: np.asarray(inputs[k], f32)
    obj_embed = g("obj_embed")
    core_Wq, core_Wk, core_Wv, core_Wo = (
        g("core_Wq"), g("core_Wk"), g("core_Wv"), g("core_Wo"))
    sub_embed, rel_embed = g("sub_embed"), g("rel_embed")
    gp_w, gp_b = g("gp_w"), g("gp_b")
    ga_in_w, ga_in_b = g("ga_in_w"), g("ga_in_b")
    ga_out_w, ga_out_b = g("ga_out_w"), g("ga_out_b")
    ffn_w1, ffn_b1, ffn_w2, ffn_b2 = g("ffn_w1"), g("ffn_b1"), g("ffn_w2"), g("ffn_b2")
    n1_w, n1_b, n2_w, n2_b = g("n1_w"), g("n1_b"), g("n2_w"), g("n2_b")
    mg_w, mg_b = g("mg_w"), g("mg_b")
    int_w, int_b = g("int_w"), g("int_b")

    # ---- host-fused intensity weights ----
    # enhanced = [enc | sr | gathered] @ mg_w + mg_b; out = sp(enh @ int_w + int_b)
    w1f = mg_w[:D_TOTAL] @ int_w                     # [512, 8000]
    w2f = mg_w[D_FEAT:D_FEAT + GP] @ int_w           # [64, 8000]
    mgb_row = mg_b @ int_w + int_b                   # [8000]

    shared = {}
    shared["wq"] = np.stack([_wpack(core_Wq[l] / np.sqrt(64.0))
                             for l in range(N_LAYERS)]).astype(bf16)
    shared["wk"] = np.stack([_wpack(core_Wk[l]) for l in range(N_LAYERS)]).astype(bf16)
    shared["wv"] = np.stack([_wpack(core_Wv[l]) for l in range(N_LAYERS)]).astype(bf16)
    shared["wo"] = np.stack([_wpack(core_Wo[l]) for l in range(N_LAYERS)]).astype(bf16)
    shared["gpw"] = np.ascontiguousarray(
        gp_w[:D_TOTAL].reshape(4, 128, GP).transpose(1, 0, 2)).astype(bf16)
    gain = ga_in_w.copy()
    gainb = ga_in_b.copy().reshape(3, GP).T.copy()   # [64, 3] columns q/k/v
    gain[:, :GP] /= np.sqrt(32.0)
    gainb[:, 0] /= np.sqrt(32.0)
    shared["gain"] = gain.astype(bf16)
    shared["gainb"] = gainb.astype(f32)
    shared["gaout"] = ga_out_w.astype(bf16)
    shared["gaoutb"] = ga_out_b.reshape(GP, 1).astype(f32)
    shared["fw1"] = ffn_w1.astype(bf16)
    shared["fw2"] = ffn_w2.astype(bf16)
    shared["fb1"] = ffn_b1.reshape(1, GP).astype(bf16)
    shared["fb2"] = ffn_b2.reshape(1, GP).astype(bf16)
    shared["lnw1"] = np.tile(n1_w, (NSEG, 1)).astype(f32)
    shared["lnb1"] = np.tile(n1_b, (NSEG, 1)).astype(f32)
    shared["lnw2"] = np.tile(n2_w, (NSEG, 1)).astype(f32)
    shared["lnb2"] = np.tile(n2_b, (NSEG, 1)).astype(f32)
    w1pad = np.zeros((D_TOTAL, NE_PAD), np.float32)
    w1pad[:, :N_ENTITY] = w1f * INTW_SCALE
    # device tile [128, 4, NE_PAD]: w1[p, c, n] = W1[c*128+p, n]
    shared["w1"] = np.ascontiguousarray(
        w1pad.reshape(4, 128, NE_PAD).transpose(1, 0, 2)).astype(f8)
    tri = (np.arange(128)[None, :] >= np.arange(128)[:, None])
    shared["tri"] = tri.astype(bf16)

    in_maps = []
    for core in range(NCORES):
        m = dict(shared)
        xT = np.zeros((BPC, 128, 2, T), np.float32)
        c0T = np.zeros((BPC, 128, 2, T), np.float32)
        c0n = np.zeros((BPC, 128, NT, D_MODEL), np.float32)
        mscT = np.zeros((128, BPC * NT, NSEG), np.float32)  # [p, b*m, seg]
        mga = np.zeros((NSEG, BPC, T), np.float32)          # [seg, b, t]
        gpbias = np.zeros((NSEG, BPC, GP), np.float32)
        w2c = np.zeros((BPC, 128, 2, NE_PAD), np.float32)
        for b in range(BPC):
            gb = core * BPC + b
            hist = objs[gb, :Lh]
            x_nat = np.zeros((T, D_MODEL), np.float32)
            x_nat[:Lh] = (obj_embed[hist] + _time_enc(times[gb, :Lh])
                          + _time_enc(dt[gb, :Lh]))
            cur0 = np.zeros((T, D_MODEL), np.float32)
            cur0[:Lh] = _time_enc(times[gb, 1:])
            xT[b] = _pack_T(x_nat)
            c0T[b] = _pack_T(cur0)
            c0n[b] = _pack_N(cur0)

            gids = group_map[subs[gb] * N_REL + marks[gb]][:Lh]
            fm = mask[gb, :Lh].astype(np.float32)
            cnt = np.bincount(gids, weights=fm, minlength=NSEG)
            ts = np.arange(Lh)
            mga[gids, b, ts] = fm
            msc = np.zeros((T, NSEG), np.float32)      # [t, seg]
            msc[ts, gids] = fm / np.maximum(cnt, 1.0)[gids]
            mscT[:, NT * b:NT * b + NT, :] = msc.reshape(
                NT, 128, NSEG).transpose(1, 0, 2)
            sr = np.concatenate([sub_embed[subs[gb, 0]], rel_embed[marks[gb, 0]]])
            nz = (cnt > 0).astype(np.float32)
            gpbias[:, b, :] = (nz[:, None] * (sr @ gp_w[D_TOTAL:D_FEAT])[None, :]
                               + gp_b[None, :])
            const_row = (sr @ mg_w[D_TOTAL:D_FEAT]) @ int_w + mgb_row  # [8000]
            # gather/const chunk rhs: rows (p, j): (0..63, 0) = W2, (64, 0) =
            # const row, rest zero; pairs with gathc8 on the device.
            w2c[b, 0:GP, 0, :N_ENTITY] = w2f * INTW_SCALE
            w2c[b, GP, 0, :N_ENTITY] = const_row * INTW_SCALE
        m["xT"] = xT.astype(bf16)
        m["c0T"] = c0T.astype(bf16)
        m["c0n"] = c0n.astype(bf16)
        m["mscT"] = mscT.astype(bf16)
        m["mgath"] = mga.astype(bf16)
        m["gpbias"] = gpbias
        m["w2c"] = w2c.astype(f8)
        in_maps.append(m)
    return in_maps


def _chain_gens(*gens):
    for g in gens:
        yield from g


def _pin_act_tables():
    # bacc assigns each InstActivation a table set greedily, which makes a
    # mixed Exp/Ln instruction stream alternate between exp_and_others and
    # natural_log -> one ~1.3us ACT_TABLE_LOAD per switch.  Empty every set
    # except natural_log_exp_and_others (which contains Exp/Ln/Copy/Identity/
    # Square -- everything we use) so the chooser is forced onto one set;
    # positional set ids are preserved.
    import concourse.bacc as bacc
    from concourse import hw_specs
    if getattr(bacc.get_activation_tables, "_pinned", False):
        return
    orig = hw_specs.get_activation_tables
    KEEP = "natural_log_exp_and_others"

    def pinned(arch):
        t = dict(orig(arch))
        return {k: (v if k == KEEP else set()) for k, v in t.items()}

    pinned._pinned = True
    bacc.get_activation_tables = pinned


def build_nc(debug_stop=99):
    import concourse.bacc as bacc
    import concourse.mybir as mybir
    import concourse.tile as tile
    from concourse import masks as cmasks
    _pin_act_tables()
    NORM_MODE = os.environ.get("BASS_NORM", "recip")

    dtb = mybir.dt.bfloat16
    dtf = mybir.dt.float32
    dt8 = mybir.dt.float8e4
    AF = mybir.ActivationFunctionType
    ALU = mybir.AluOpType
    AX = mybir.AxisListType
    DR = mybir.MatmulPerfMode.DoubleRow

    nc = bacc.Bacc()

    def din(name, shape, dt=dtb):
        return nc.dram_tensor(name, shape, dt, kind="ExternalInput")

    xT_d = din("xT", [BPC, 128, 2, T])
    c0T_d = din("c0T", [BPC, 128, 2, T])
    c0n_d = din("c0n", [BPC, 128, NT, D_MODEL])
    wq_d = din("wq", [N_LAYERS, 128, 2, D_MODEL])
    wk_d = din("wk", [N_LAYERS, 128, 2, D_MODEL])
    wv_d = din("wv", [N_LAYERS, 128, 2, D_MODEL])
    wo_d = din("wo", [N_LAYERS, 128, 2, D_MODEL])
    mscT_d = din("mscT", [128, BPC * NT, NSEG])
    mgath_d = din("mgath", [NSEG, BPC, T])
    gpw_d = din("gpw", [128, 4, GP])
    gpbias_d = din("gpbias", [NSEG, BPC, GP], mybir.dt.float32)
    gain_d = din("gain", [GP, 3 * GP])
    gainb_d = din("gainb", [GP, 3], mybir.dt.float32)
    gaout_d = din("gaout", [GP, GP])
    gaoutb_d = din("gaoutb", [GP, 1], mybir.dt.float32)
    fw1_d = din("fw1", [GP, GP])
    fw2_d = din("fw2", [GP, GP])
    fb1_d = din("fb1", [1, GP])
    fb2_d = din("fb2", [1, GP])
    lnw1_d = din("lnw1", [NSEG, GP], mybir.dt.float32)
    lnb1_d = din("lnb1", [NSEG, GP], mybir.dt.float32)
    lnw2_d = din("lnw2", [NSEG, GP], mybir.dt.float32)
    lnb2_d = din("lnb2", [NSEG, GP], mybir.dt.float32)
    w1_d = din("w1", [128, 4, NE_PAD], dt8)
    w2c_d = din("w2c", [BPC, 128, 2, NE_PAD], dt8)
    tri_d = din("tri", [128, 128])
    out_d = nc.dram_tensor("out", [R, N_ENTITY], mybir.dt.bfloat16,
                           kind="ExternalOutput")
    debug_dump = bool(os.environ.get("BASS_DEBUG_DUMP"))
    if debug_dump:
        encdbg_d = nc.dram_tensor("encdbg", [N_LAYERS, 128, 2, T],
                                  mybir.dt.bfloat16, kind="ExternalOutput")
        odbg_d = nc.dram_tensor("odbg", [N_LAYERS, 128, 2, T],
                                mybir.dt.bfloat16, kind="ExternalOutput")

    with tile.TileContext(nc) as tc:
        with (
            tc.tile_pool(name="persist", bufs=1) as pp,
            tc.tile_pool(name="work", bufs=2) as wp,
            tc.tile_pool(name="acts", bufs=5) as ap,
            tc.tile_pool(name="outp", bufs=3) as op,
            tc.tile_pool(name="psum", bufs=1, space="PSUM") as ps,
        ):
            def pt(shape, tag, dt=dtb):
                return pp.tile(shape, dt, tag=tag, name=tag)

            def dma(dst, src):
                nc.sync.dma_start(dst, src)

            # PSUM budget (16KB/partition = 8 banks), by tag:
            #   pi 2x[128,1024]f32 = 8KB; f 2x[128,512]f32 = 4KB;
            #   po 2x[65,512]f32 = 4KB.  (pb lives in the f tag)
            def ps_pi():
                return ps.tile([128, 1024], dtf, tag="pi", bufs=2, name="pi")

            def ps_f(shape, dt=dtf):
                return ps.tile(shape, dt, tag="f", bufs=2, name="psf",
                               padded_shape=[128, 512])

            def ps_po():
                return ps.tile([65, T], dtf, tag="po", bufs=2, name="po")

            # ---- constants in ----
            # Input DMAs are split into small per-queue chunks, ordered so
            # batch 0 layer 0's operands land first: the front can start
            # ~5us in instead of waiting ~20us for monolithic transfers.
            xT = [pt([128, 2, T], f"xT{b}") for b in range(BPC)]
            c0T = [pt([128, 2, T], f"c0T{b}") for b in range(BPC)]
            c0n = [pt([128, NT, D_MODEL], f"c0n{b}") for b in range(BPC)]
            wq = [pt([128, 2, D_MODEL], f"wq{l}") for l in range(N_LAYERS)]
            wk = [pt([128, 2, D_MODEL], f"wk{l}") for l in range(N_LAYERS)]
            wo = [pt([128, 2, D_MODEL], f"wo{l}") for l in range(N_LAYERS)]
            wv = [pt([128, 2, D_MODEL], f"wv{l}") for l in range(N_LAYERS)]
            tri_s = pt([128, 128], "tri")

            def dma_qkv_w(l):
                dma(wq[l][:], wq_d[l])
                dma(wk[l][:], wk_d[l])
                dma(wv[l][:], wv_d[l])
                dma(wo[l][:], wo_d[l])

            def dma_batch_in(b):
                dma(c0T[b][:], c0T_d[b])
                dma(xT[b][:], xT_d[b])
                dma(c0n[b][:], c0n_d[b])

            dma_qkv_w(0)
            dma_batch_in(0)
            dma(tri_s[:], tri_d[:])
            dma_qkv_w(1)
            dma_batch_in(1)
            mscT_s = pt([128, BPC * NT, NSEG], "mscT")
            dma(mscT_s[:], mscT_d[:])
            mgath_s = pt([NSEG, BPC, T], "mgath")
            dma(mgath_s[:], mgath_d[:])
            gpw_s = pt([128, 4, GP], "gpw")
            dma(gpw_s[:], gpw_d[:])
            gpbias_s = pt([NSEG, BPC, GP], "gpbias", dtf)
            dma(gpbias_s[:], gpbias_d[:])
            gain_s = pt([GP, 3 * GP], "gain")
            dma(gain_s[:], gain_d[:])
            gainb_s = pt([GP, 3], "gainb", dtf)
            dma(gainb_s[:], gainb_d[:])
            gaout_s = pt([GP, GP], "gaout")
            dma(gaout_s[:], gaout_d[:])
            gaoutb_s = pt([GP, 1], "gaoutb", dtf)
            dma(gaoutb_s[:], gaoutb_d[:])
            fw1_s = pt([GP, GP], "fw1")
            dma(fw1_s[:], fw1_d[:])
            fw2_s = pt([GP, GP], "fw2")
            dma(fw2_s[:], fw2_d[:])
            fb1_s = pt([1, GP], "fb1")
            dma(fb1_s[:], fb1_d[:])
            fb2_s = pt([1, GP], "fb2")
            dma(fb2_s[:], fb2_d[:])
            ln_s = {}
            for nm, d in [("lnw1", lnw1_d), ("lnb1", lnb1_d),
                          ("lnw2", lnw2_d), ("lnb2", lnb2_d)]:
                ln_s[nm] = pt([NSEG, GP], nm, dtf)
                dma(ln_s[nm][:], d[:])

            # intensity weights, split into per-queue chunks so all DMA
            # rings pull in parallel and early strips land first
            w2c_s = [pt([128, 2, NE_PAD], f"w2c{b}", dt8) for b in range(BPC)]
            w1_s = pt([128, 4, NE_PAD], "w1", dt8)
            for c in range(2):
                for blk in range(4):
                    cs = slice(2048 * blk, 2048 * blk + 2048)
                    dma(w2c_s[0][:, c, cs], w2c_d[0, :, c, cs])
            for blk in range(4):
                for c in range(4):
                    cs = slice(2048 * blk, 2048 * blk + 2048)
                    dma(w1_s[:, c, cs], w1_d[:, c, cs])
            for c in range(2):
                for blk in range(4):
                    cs = slice(2048 * blk, 2048 * blk + 2048)
                    dma(w2c_s[1][:, c, cs], w2c_d[1, :, c, cs])

            eps_s = pt([NSEG, 1], "eps", dtf)
            nc.gpsimd.memset(eps_s[:], 1e-5)
            ident = pt([128, 128], "ident")
            cmasks.make_identity(nc, ident[:])
            ones_r = pt([1, T], "ones_r")
            nc.gpsimd.memset(ones_r[:], 1.0)

            # gather/const stationary: [128, 2, T] fp8; row (64, 0) is the
            # all-ones row pairing with the const row of w2c; rows 65..127
            # and all of j=1 stay zero.
            gathc8 = [pt([128, 2, T], f"gathc8{b}", dt8) for b in range(BPC)]
            enc8 = [pt([128, 2 * N_LAYERS, T], f"enc8{b}", dt8)
                    for b in range(BPC)]
            for b in range(BPC):
                nc.gpsimd.memset(gathc8[b][:], 0.0)
                nc.gpsimd.memset(gathc8[b][GP:GP + 1, 0, :], 1.0)

            def layernorm(xin, wtile, btile, outf, outb):
                P = xin.shape[0]
                s1 = wp.tile([P, 1], dtf, tag="lns", name="lns")
                nc.vector.reduce_sum(s1[:], xin[:], axis=AX.X)
                mu = wp.tile([P, 1], dtf, tag="lnm", name="lnm")
                nc.vector.tensor_scalar_mul(mu[:], s1[:], 1.0 / GP)
                xc = wp.tile([P, GP], dtf, tag="lnxc", name="lnxc")
                nc.vector.tensor_scalar(xc[:], xin[:], mu[:], None,
                                        op0=ALU.subtract)
                sq = wp.tile([P, GP], dtf, tag="lnsq", name="lnsq")
                vs = wp.tile([P, 1], dtf, tag="lnvs", name="lnvs")
                nc.scalar.activation(sq[:], xc[:], AF.Square, accum_out=vs[:])
                lnv = wp.tile([P, 1], dtf, tag="lnlv", name="lnlv")
                nc.scalar.activation(lnv[:], vs[:], AF.Ln, scale=1.0 / GP,
                                     bias=eps_s[:P])
                rstd = wp.tile([P, 1], dtf, tag="lnrs", name="lnrs")
                nc.scalar.activation(rstd[:], lnv[:], AF.Exp, scale=-0.5)
                nc.vector.scalar_tensor_tensor(
                    outf[:], xc[:], rstd[:], wtile[:], op0=ALU.mult, op1=ALU.mult)
                nc.vector.tensor_add(outf[:], outf[:], btile[:])
                nc.vector.tensor_copy(outb[:], outf[:])

            # ==== software-pipelined emission ====
            # Engines execute their instruction streams strictly in order,
            # so overlap between independent work REQUIRES interleaving at
            # emission time.  Schedule:
            #   front(b0) ; [post(b0) || front(b1)-begin] ;
            #   [intensity(b0) || front(b1)-rest + post(b1)] ; intensity(b1)
            ST = [{"curT": c0T[b], "curn": c0n[b],
                   "encn": [None] * N_LAYERS} for b in range(BPC)]

            def enc_gen(b):
                # Batch 0's front runs while ACT is idle: staging copies and
                # softmax normalization go on ACT to shorten the critical
                # chain.  Batch 1's front overlaps batch 0's ACT-bound
                # intensity epilogue, so everything movable stays on the DVE.
                on_act = (b == 0)

                def stage(dst, src):
                    nc.vector.tensor_copy(dst, src)

                for l in range(N_LAYERS):
                    curT, curn = ST[b]["curT"], ST[b]["curn"]
                    qT = ap.tile([128, 2, T], dtb, tag="qT", bufs=2, name="qT")
                    kT = ap.tile([128, 2, T], dtb, tag="kT", bufs=2, name="kT")
                    for c2 in range(2):
                        psq = ps_f([128, T])
                        for c in range(2):
                            nc.tensor.matmul(
                                psq[:], wq[l][:, c, 128 * c2:128 * c2 + 128],
                                curT[:, c, :], start=(c == 0), stop=(c == 1))
                        stage(qT[:, c2, :], psq[:])
                        psk = ps_f([128, T])
                        for c in range(2):
                            nc.tensor.matmul(
                                psk[:], wk[l][:, c, 128 * c2:128 * c2 + 128],
                                xT[b][:, c, :], start=(c == 0), stop=(c == 1))
                        stage(kT[:, c2, :], psk[:])
                        yield
                    vcat = []
                    for m in range(NT):
                        psv = ps_f([128, D_MODEL])
                        for c in range(2):
                            nc.tensor.matmul(
                                psv[:], xT[b][:, c, 128 * m:128 * m + 128],
                                wv[l][:, c, :], start=(c == 0), stop=(c == 1))
                        vc = ap.tile([128, N_HEADS, 65], dtb, tag=f"vcat{m}",
                                     bufs=2, name=f"vcat{m}")
                        stage(vc[:, :, 0:64],
                              psv[:].rearrange("p (h e) -> p h e", h=N_HEADS))
                        nc.vector.memset(vc[:, :, 64:65], 1.0)
                        vcat.append(vc)
                        if m == 1:
                            yield
                    yield
                    # Scores phase: all (h, j) score+exp+mask units emitted
                    # back-to-back so no engine queue ever stalls on a long
                    # cross-engine round-trip (engines run their queues in
                    # order; a waiting instruction blocks everything behind
                    # it, including the other batch's interleaved intensity
                    # work).
                    aTs = [[None] * NT for _ in range(N_HEADS)]
                    cnt = 0
                    for h in range(N_HEADS):
                        bp, hc = 64 * (h % 2), h // 2
                        for j in range(NT):
                            q0 = 128 * j
                            nq = T - q0
                            pss = ps_f([128, T])
                            nc.tensor.matmul(
                                pss[:, :nq],
                                kT[bp:bp + 64, hc, q0:q0 + 128],
                                qT[bp:bp + 64, hc, q0:T])
                            aT = ap.tile([128, T], dtb, tag=f"aT{b}{h}{j}",
                                         bufs=1, name="aT")
                            nc.scalar.activation(aT[:, :nq], pss[:, :nq], AF.Exp)
                            nc.vector.tensor_mul(
                                aT[:, 0:128], aT[:, 0:128], tri_s[:])
                            aTs[h][j] = aT
                            cnt += 1
                            if cnt % 3 == 0:
                                yield
                    # po phase per head, then softmax normalization: 1/colsum
                    # on DVE (keeps ACT free for softplus), rank-1 broadcast
                    # matmul, multiply.
                    oT = ap.tile([128, 2, T], dtb, tag="oT", bufs=2, name="oT")
                    for h in range(N_HEADS):
                        bp, hc = 64 * (h % 2), h // 2
                        po = ps_po()
                        for j in range(NT):
                            q0 = 128 * j
                            nc.tensor.matmul(
                                po[:, q0:T], vcat[j][:, h, :],
                                aTs[h][j][:, :T - q0],
                                start=(j == 0), stop=(j == NT - 1),
                                skip_group_check=True)
                        oraw = wp.tile([64, T], dtb, tag="oraw", name="oraw")
                        stage(oraw[:], po[0:64, :])
                        rbf = wp.tile([1, T], dtb, tag="rbf", name="rbf")
                        if on_act or NORM_MODE == "act":
                            rsb = wp.tile([1, T], dtf, tag="rsb", name="rsb")
                            nc.scalar.activation(rsb[:], po[64:65, :], AF.Ln)
                            nc.scalar.activation(rbf[:], rsb[:], AF.Exp,
                                                 scale=-1.0)
                        else:
                            # reciprocal_approx_fast is a custom DVE op
                            # (BITWISE_NOT seed); it reads garbage from PSUM,
                            # so stage the colsum row into SBUF first.
                            rcs = wp.tile([1, T], dtf, tag="rcs", name="rcs")
                            nc.vector.tensor_copy(rcs[:], po[64:65, :])
                            rr = wp.tile([1, T], dtf, tag="rr", name="rr")
                            nc.vector.reciprocal_approx_fast(rr[:], rcs[:])
                            nc.vector.tensor_copy(rbf[:], rr[:])
                        pb = ps_f([64, T])
                        nc.tensor.matmul(pb[:], ones_r[0:1, 0:64], rbf[:])
                        nc.vector.tensor_mul(oT[bp:bp + 64, hc, :], oraw[:],
                                             pb[:])
                        yield
                    if debug_dump and b == 0:
                        nc.sync.dma_start(odbg_d[l], oT[:])
                    eT = pt([128, 2, T], f"encT{l}{b}")
                    en = pt([128, NT, D_MODEL], f"encn{l}{b}")
                    for c2 in range(2):
                        psp = ps_f([128, T])
                        for c in range(2):
                            nc.tensor.matmul(
                                psp[:], wo[l][:, c, 128 * c2:128 * c2 + 128],
                                oT[:, c, :], start=(c == 0), stop=(c == 1))
                        nc.vector.tensor_add(eT[:, c2, :], curT[:, c2, :],
                                             psp[:])
                    nc.vector.tensor_copy(enc8[b][:, 2 * l:2 * l + 2, :],
                                          eT[:])
                    if debug_dump and b == 0:
                        nc.sync.dma_start(encdbg_d[l], eT[:])
                    yield
                    for m in range(NT):
                        psn = ps_f([128, D_MODEL])
                        for c in range(2):
                            nc.tensor.matmul(
                                psn[:], oT[:, c, 128 * m:128 * m + 128],
                                wo[l][:, c, :], start=(c == 0), stop=(c == 1))
                        nc.vector.tensor_add(en[:, m, :], curn[:, m, :], psn[:])
                        if m == 1:
                            yield
                    ST[b]["encn"][l] = en
                    ST[b]["curT"], ST[b]["curn"] = eT, en
                    yield

            def post_gen(b):
                encn = ST[b]["encn"]
                # ---- group scatter A^T[d, seg] (batch-local 64 segs) ----
                AT_s = wp.tile([128, 4, NSEG], dtb, tag="AT", name="AT")
                for dt_i in range(4):
                    l, c2 = dt_i // 2, dt_i % 2
                    pa = ps_f([128, NSEG])
                    for m in range(NT):
                        nc.tensor.matmul(
                            pa[:], encn[l][:, m, 128 * c2:128 * c2 + 128],
                            mscT_s[:, NT * b + m, :],
                            start=(m == 0), stop=(m == NT - 1))
                    nc.vector.tensor_copy(AT_s[:, dt_i, :], pa[:])
                    if dt_i % 2:
                        yield
                # ---------------- group block ([64, *]) ----------------
                pg = ps_f([NSEG, GP])
                for dt_i in range(4):
                    nc.tensor.matmul(pg[:], AT_s[:, dt_i, :], gpw_s[:, dt_i, :],
                                     start=(dt_i == 0), stop=(dt_i == 3))
                gp_f = wp.tile([NSEG, GP], dtf, tag="gp_f", name="gp_f")
                nc.vector.tensor_add(gp_f[:], pg[:], gpbias_s[:, b, :])
                gp_bf = wp.tile([NSEG, GP], dtb, tag="gp_bf", name="gp_bf")
                nc.vector.tensor_copy(gp_bf[:], gp_f[:])
                ptr = ps_f([GP, NSEG], dtb)
                nc.tensor.transpose(ptr[:], gp_bf[:], ident[0:NSEG, 0:NSEG])
                gpT = wp.tile([GP, NSEG], dtb, tag="gpT", name="gpT")
                nc.vector.tensor_copy(gpT[:], ptr[:])
                yield
                qkvT = []
                for i in range(3):
                    pq = ps_f([GP, NSEG])
                    nc.tensor.matmul(pq[:], gain_s[:, GP * i:GP * i + GP],
                                     gpT[:])
                    tq = wp.tile([GP, NSEG], dtb, tag=f"qkvT{i}",
                                 name=f"qkvT{i}")
                    nc.vector.tensor_scalar(
                        tq[:], pq[:], gainb_s[:, i:i + 1], None, op0=ALU.add)
                    qkvT.append(tq)
                yield
                vc2 = wp.tile([GP, GH, 33], dtb, tag="vc2", name="vc2")
                for h in range(GH):
                    pv = ps_f([GP, 32], dtb)
                    nc.tensor.transpose(
                        pv[:], qkvT[2][32 * h:32 * h + 32, :],
                        ident[32 * h:32 * h + 32, 32 * h:32 * h + 32])
                    nc.vector.tensor_copy(vc2[:, h, 0:32], pv[:])
                nc.vector.memset(vc2[:, :, 32:33], 1.0)
                yield
                oT2 = wp.tile([GP, NSEG], dtb, tag="oT2", name="oT2")
                for h in range(GH):
                    ps1 = ps_f([64, 64])
                    nc.tensor.matmul(
                        ps1[:], qkvT[1][32 * h:32 * h + 32, :],
                        qkvT[0][32 * h:32 * h + 32, :])
                    a2 = wp.tile([64, 64], dtb, tag="a2", name="a2")
                    nc.scalar.activation(a2[:], ps1[:], AF.Exp)
                    po2 = ps_f([33, 64])
                    nc.tensor.matmul(po2[:], vc2[:, h, :], a2[:])
                    r2 = wp.tile([1, 64], dtf, tag="r2", name="r2")
                    nc.scalar.activation(r2[:], po2[32:33, :], AF.Ln)
                    r2b = wp.tile([1, 64], dtb, tag="r2b", name="r2b")
                    nc.scalar.activation(r2b[:], r2[:], AF.Exp, scale=-1.0)
                    pb2 = ps_f([32, 64])
                    nc.tensor.matmul(pb2[:], ones_r[0:1, 0:32], r2b[:])
                    orw2 = wp.tile([32, 64], dtb, tag="orw2", name="orw2")
                    nc.vector.tensor_copy(orw2[:], po2[0:32, :])
                    nc.vector.tensor_mul(oT2[32 * h:32 * h + 32, :], orw2[:],
                                         pb2[:])
                    yield
                pga = ps_f([GP, NSEG])
                nc.tensor.matmul(pga[:], gaout_s[:], oT2[:])
                t1 = wp.tile([GP, NSEG], dtb, tag="t1", name="t1")
                nc.vector.scalar_tensor_tensor(
                    t1[:], pga[:], gaoutb_s[:, 0:1], gpT[:],
                    op0=ALU.add, op1=ALU.add)
                px1 = ps_f([NSEG, GP], dtb)
                nc.tensor.transpose(px1[:], t1[:], ident[0:GP, 0:GP])
                x1 = wp.tile([NSEG, GP], dtf, tag="x1", name="x1")
                nc.vector.tensor_copy(x1[:], px1[:])
                gn_f = wp.tile([NSEG, GP], dtf, tag="gn_f", name="gn_f")
                gn_b = wp.tile([NSEG, GP], dtb, tag="gn_b", name="gn_b")
                layernorm(x1, ln_s["lnw1"][0:NSEG], ln_s["lnb1"][0:NSEG],
                          gn_f, gn_b)
                yield
                pgt = ps_f([GP, NSEG], dtb)
                nc.tensor.transpose(pgt[:], gn_b[:], ident[0:NSEG, 0:NSEG])
                gnT = wp.tile([GP, NSEG], dtb, tag="gnT", name="gnT")
                nc.vector.tensor_copy(gnT[:], pgt[:])
                ph1 = ps_f([NSEG, GP])
                nc.tensor.matmul(ph1[:], ones_r[0:1, 0:NSEG], fb1_s[:],
                                 start=True, stop=False)
                nc.tensor.matmul(ph1[:], gnT[:], fw1_s[:], start=False,
                                 stop=True)
                h1b = wp.tile([NSEG, GP], dtb, tag="h1b", name="h1b")
                nc.vector.tensor_scalar_max(h1b[:], ph1[:], 0.0)
                ph1t = ps_f([GP, NSEG], dtb)
                nc.tensor.transpose(ph1t[:], h1b[:], ident[0:NSEG, 0:NSEG])
                h1T = wp.tile([GP, NSEG], dtb, tag="h1T", name="h1T")
                nc.vector.tensor_copy(h1T[:], ph1t[:])
                yield
                ph2 = ps_f([NSEG, GP])
                nc.tensor.matmul(ph2[:], ones_r[0:1, 0:NSEG], fb2_s[:],
                                 start=True, stop=False)
                nc.tensor.matmul(ph2[:], h1T[:], fw2_s[:], start=False,
                                 stop=True)
                x2 = wp.tile([NSEG, GP], dtf, tag="x2", name="x2")
                nc.vector.tensor_add(x2[:], ph2[:], gn_f[:])
                go_f = wp.tile([NSEG, GP], dtf, tag="go_f", name="go_f")
                go_b = wp.tile([NSEG, GP], dtb, tag="go_b", name="go_b")
                layernorm(x2, ln_s["lnw2"][0:NSEG], ln_s["lnb2"][0:NSEG],
                          go_f, go_b)
                yield
                # ---- gather^T: gathc8[p, 0, t] = gout[gid(t), p]*fm(t) ----
                pgh = ps_f([GP, T])
                nc.tensor.matmul(pgh[:], go_b[:], mgath_s[:, b, :])
                nc.vector.tensor_copy(gathc8[b][0:GP, 0, :], pgh[:])
                yield

            def int_gen(b):
                # -------- intensity head (fp8 DoubleRow, K=512+65) --------
                for P in range(NPAIR):
                    for m in range(NT):
                        ot = op.tile([128, 2048], dtb, tag="ot", name="ot")
                        width = 0
                        for s2 in range(2):
                            sp = 2 * P + s2
                            ncs = min(1024, N_ENTITY - 1024 * sp)
                            pi = ps_pi()
                            for h2 in range(2):
                                col0 = 1024 * sp + 512 * h2
                                ncol = min(512, N_ENTITY - col0)
                                if ncol <= 0:
                                    continue
                                pv = pi[:, 512 * h2:512 * h2 + ncol]
                                for c in range(2):
                                    nc.tensor.matmul(
                                        pv,
                                        enc8[b][:, 2 * c:2 * c + 2,
                                                128 * m:128 * m + 128],
                                        w1_s[:, 2 * c:2 * c + 2,
                                             col0:col0 + ncol],
                                        perf_mode=DR, start=(c == 0),
                                        stop=False, skip_group_check=True)
                                nc.tensor.matmul(
                                    pv,
                                    gathc8[b][:, :, 128 * m:128 * m + 128],
                                    w2c_s[b][:, :, col0:col0 + ncol],
                                    perf_mode=DR, start=False, stop=True,
                                    skip_group_check=True)
                            nc.scalar.activation(
                                ot[:, 1024 * s2:1024 * s2 + ncs],
                                pi[:, :ncs], AF.Exp, scale=1.0 / INTW_SCALE)
                            width += ncs
                        # softplus second half: one Ln per strip-pair
                        nc.scalar.activation(ot[:, :width], ot[:, :width],
                                             AF.Ln, bias=1.0)
                        nc.sync.dma_start(
                            out_d[T * b + 128 * m:T * b + 128 * m + 128,
                                  2048 * P:2048 * P + width],
                            ot[:, :width])
                        yield

            def drain(g):
                for _ in g:
                    pass

            def zip_all(*gens):
                gens = list(gens)
                while gens:
                    for g in list(gens):
                        if next(g, StopIteration) is StopIteration:
                            gens.remove(g)

            def zip_until(primary, filler):
                # pace on `primary`; advance `filler` one stage per step
                for _ in primary:
                    next(filler, None)

            # Schedule: front(b0) alone; post(b0) paced against front(b1)'s
            # beginning (b1's short-dependency q/k/score chunks execute on
            # the otherwise-idle PE while post(b0)'s serial group-block chain
            # runs); then intensity(b0) interleaved 1:2 with the REST of
            # front(b1)+post(b1) -- intensity units lead so their ACT work
            # is never queued behind unissued front(b1) instructions, and
            # the 2x g1 stepping lets post(b1) finish before intensity(b0)
            # drains so intensity(b1) starts immediately after.
            drain(enc_gen(0))
            g1 = (_chain_gens(enc_gen(1), post_gen(1))
                  if BPC > 1 else iter(()))
            if debug_stop >= 3:
                zip_until(post_gen(0), g1)
                if debug_stop >= 5:
                    # Intensity(b0) units lead the remaining front(b1)
                    # chunks 1:2 -- the units' ACT work is never queued
                    # behind unissued front(b1) instructions, and the double
                    # g1 stepping lets post(b1) finish before intensity(b0)
                    # drains.
                    gi0 = int_gen(0)
                    next(gi0, None)
                    next(gi0, None)
                    while True:
                        if next(gi0, StopIteration) is StopIteration:
                            gi0 = None
                            break
                        if (next(g1, StopIteration) is StopIteration or
                                next(g1, StopIteration) is StopIteration):
                            break
                    drain(g1)
                    if gi0 is not None:
                        drain(gi0)
                    for bb in range(1, BPC):
                        drain(int_gen(bb))
                else:
                    drain(g1)
            elif BPC > 1:
                drain(g1)
    nc.compile()
    return nc


def _get_nc():
    if "nc" not in _CACHED:
        _CACHED["nc"] = build_nc()
    return _CACHED["nc"]


def _install_ntff_hook():
    """Best-effort: register the axon NTFF profile hook so trace=True works."""
    import sys, types
    if "antenv.axon_hooks" in sys.modules:
        return
    try:
        import antenv  # noqa
        from trn_agent_boot.trn_boot import _ntff_profile_via_ctypes
        mod = types.ModuleType("antenv.axon_hooks")
        hook = [_ntff_profile_via_ctypes("/opt/axon/libaxon_pjrt.so")]
        mod.set_axon_ntff_profile_hook = lambda h: hook.__setitem__(0, h)
        mod.get_axon_ntff_profile_hook = lambda: hook[0]
        sys.modules["antenv.axon_hooks"] = mod
    except Exception:
        pass


def kernel(**inputs):
    global LAST_EXEC_NS, LAST_RESULTS
    from concourse.bass_utils import run_bass_kernel_spmd

    in_maps = prep_inputs(inputs)
    nc = _get_nc()
    trace = bool(os.environ.get("BASS_TRACE"))
    if trace:
        _install_ntff_hook()
    res = run_bass_kernel_spmd(
        nc, in_maps, core_ids=list(range(NCORES)), trace=trace)
    LAST_RESULTS = res
    LAST_EXEC_NS = res.exec_time_ns
    out = np.empty((B, Lh, N_ENTITY), np.float32)
    for core in range(NCORES):
        o = res.results[core]["out"]
        for b in range(BPC):
            out[core * BPC + b] = o[T * b:T * b + Lh, :].astype(np.float32)
    return out


# revision 27
# speedup vs baseline: 1.2214x; 1.0080x over previous
"""GAttNHP model as a Bass/Tile kernel on 8 Trainium2 NeuronCores.

Strategy: pure data-parallel over batch (B=16 -> 2 batches/core, no
collectives).  bf16 matmuls accumulating in fp32 PSUM; the dominant
intensity head runs fp8e4 with DoubleRow (2 weights/cell).

Key structural move vs the straightforward lowering: the merge Linear is
folded into the intensity head ON THE HOST.  With
  enhanced = [enc | s_emb | r_emb | gathered] @ mg_w + mg_b
  out      = softplus(enhanced @ int_w + int_b)
we precompute W1 = mg_w[:512] @ int_w (enc part), W2 = mg_w[1024:] @ int_w
(gathered part) and a per-batch constant row (s/r embeddings are constant
per batch).  The device-side intensity matmul then has K = 512 (enc, fp8
DoubleRow, 2 instructions) + 64+1 (gather one-hot + const row, packed into
one more DoubleRow instruction) instead of K = 1024 + a separate [*,1088]
merge matmul.  The gather chunk's stationary is padded to the full 128
partitions (zero rows) so every matmul in a PSUM accumulation group covers
the same PE row-group (disjoint row-groups race on the PSUM accumulate and
fault the exec unit).

The intensity epilogue softplus = ln(1+exp(x)) is two full ACT passes over
every output element and is the hard floor of the kernel (~110us/core); the
schedule is arranged so the Scalar engine never idles during the intensity
phases: Exp per 1024-col strip from a 2-bank PSUM tile, one Ln per 2048-col
strip-pair, attention softmax normalization moved off ACT onto the DVE
(reciprocal_approx_fast on the appended-ones column sum), and ACT keeps a
single activation table set (natural_log_exp_and_others) throughout -- see
_pin_act_tables.

Device pipeline, emitted as a complete chain PER BATCH so batch 1's
latency-bound front half overlaps batch 0's ACT-bound intensity epilogue:
  1. AttNHP encoder, 2 layers, activations kept in transposed [d, t]
     layout (plus a natural [t, d] copy for the group scatter).  Causal
     softmax in s^T layout: exp (no max-subtract, scores are tiny),
     triangular mask on the diagonal block, column sums via an appended
     ones-column on v, normalization via DVE reciprocal + rank-1
     broadcast matmul.  Encoder output additionally cast to fp8 (enc8)
     as the intensity stationary.
  2. Group scatter-mean as a matmul against a host-built one-hot
     matrix (64 batch-local segments on partitions 0..63).
  3. Tiny group transformer block (attn + ffn + 2 layernorms; rstd via
     ln/exp so the whole kernel uses ONE ACT table set).
  4. Gather back gathc8[p,t] = gout[gid(t),p]*fm(t) via one matmul
     against the one-hot gather matrix, cast fp8, plus a preset
     all-ones row (pairs with the per-batch const row of W2).
  5. Intensity head: per (strip-pair, t-tile): 3 fp8 DoubleRow matmuls
     per 512-col half into 2-bank [128,1024] PSUM strips; Exp per
     strip, Ln per strip-pair, DMA out per strip-pair.
"""

import os

import numpy as np
import ml_dtypes

bf16 = ml_dtypes.bfloat16
f8 = ml_dtypes.float8_e4m3
INTW_SCALE = 128.0

N_ENTITY = 8000
N_REL = 100
N_GROUPS = 64
HIDDEN = 256
D_MODEL = 256
N_LAYERS = 2
N_HEADS = 4
GP = 64
GH = 2
D_TOTAL = D_MODEL * N_LAYERS          # 512
D_FEAT = D_TOTAL + 2 * HIDDEN         # 1024
B, L = 16, 512
Lh = L - 1                            # 511
NCORES = 8
BPC = B // NCORES                     # 2 batches per core
T = 512                               # padded seq length
NT = T // 128                         # 4 t-tiles per batch
R = BPC * T                           # 1024 rows per core
NSEG = N_GROUPS                       # 64 batch-local segments
NE_PAD = 8192
NSTRIP = 8                            # 1024-col strips (last covers 832)
NPAIR = NSTRIP // 2                   # 2048-col strip-pairs

LAST_EXEC_NS = None
LAST_RESULTS = None
_CACHED = {}


def _time_enc(t, d=D_MODEL):
    i = np.arange(d // 2)
    freqs = np.exp(-np.log(10000.0) * (2.0 * i / d)).astype(np.float32)
    ang = t[..., None].astype(np.float32) * freqs
    return np.concatenate([np.sin(ang), np.cos(ang)], axis=-1).astype(np.float32)


def _pack_T(a):
    # [512, 256] natural -> [128, 2, 512] transposed tiles (d = c*128+p)
    return np.ascontiguousarray(a.T.reshape(2, 128, T).transpose(1, 0, 2))


def _pack_N(a):
    # [512, 256] natural -> [128, 4, 256] natural tiles (t = m*128+p)
    return np.ascontiguousarray(a.reshape(NT, 128, D_MODEL).transpose(1, 0, 2))


def _wpack(w):
    # [256, 256] -> [128, 2, 256]  (rows d = c*128+p)
    return np.ascontiguousarray(w.reshape(2, 128, D_MODEL).transpose(1, 0, 2))


def prep_inputs(inputs):
    """Returns in_maps per core."""
    f32 = np.float32
    subs = np.asarray(inputs["subs"])
    marks = np.asarray(inputs["marks"])
    objs = np.asarray(inputs["objs"])
    times = np.asarray(inputs["times"], f32)
    dt = np.asarray(inputs["dt"], f32)
    mask = np.asarray(inputs["mask"])
    group_map = np.asarray(inputs["group_map"])
    g = lambda k: np.asarray(inputs[k], f32)
    obj_embed = g("obj_embed")
    core_Wq, core_Wk, core_Wv, core_Wo = (
        g("core_Wq"), g("core_Wk"), g("core_Wv"), g("core_Wo"))
    sub_embed, rel_embed = g("sub_embed"), g("rel_embed")
    gp_w, gp_b = g("gp_w"), g("gp_b")
    ga_in_w, ga_in_b = g("ga_in_w"), g("ga_in_b")
    ga_out_w, ga_out_b = g("ga_out_w"), g("ga_out_b")
    ffn_w1, ffn_b1, ffn_w2, ffn_b2 = g("ffn_w1"), g("ffn_b1"), g("ffn_w2"), g("ffn_b2")
    n1_w, n1_b, n2_w, n2_b = g("n1_w"), g("n1_b"), g("n2_w"), g("n2_b")
    mg_w, mg_b = g("mg_w"), g("mg_b")
    int_w, int_b = g("int_w"), g("int_b")

    # ---- host-fused intensity weights ----
    # enhanced = [enc | sr | gathered] @ mg_w + mg_b; out = sp(enh @ int_w + int_b)
    w1f = mg_w[:D_TOTAL] @ int_w                     # [512, 8000]
    w2f = mg_w[D_FEAT:D_FEAT + GP] @ int_w           # [64, 8000]
    mgb_row = mg_b @ int_w + int_b                   # [8000]

    shared = {}
    shared["wq"] = np.stack([_wpack(core_Wq[l] / np.sqrt(64.0))
                             for l in range(N_LAYERS)]).astype(bf16)
    shared["wk"] = np.stack([_wpack(core_Wk[l]) for l in range(N_LAYERS)]).astype(bf16)
    shared["wv"] = np.stack([_wpack(core_Wv[l]) for l in range(N_LAYERS)]).astype(bf16)
    shared["wo"] = np.stack([_wpack(core_Wo[l]) for l in range(N_LAYERS)]).astype(bf16)
    shared["gpw"] = np.ascontiguousarray(
        gp_w[:D_TOTAL].reshape(4, 128, GP).transpose(1, 0, 2)).astype(bf16)
    gain = ga_in_w.copy()
    gainb = ga_in_b.copy().reshape(3, GP).T.copy()   # [64, 3] columns q/k/v
    gain[:, :GP] /= np.sqrt(32.0)
    gainb[:, 0] /= np.sqrt(32.0)
    shared["gain"] = gain.astype(bf16)
    shared["gainb"] = gainb.astype(f32)
    shared["gaout"] = ga_out_w.astype(bf16)
    shared["gaoutb"] = ga_out_b.reshape(GP, 1).astype(f32)
    shared["fw1"] = ffn_w1.astype(bf16)
    shared["fw2"] = ffn_w2.astype(bf16)
    shared["fb1"] = ffn_b1.reshape(1, GP).astype(bf16)
    shared["fb2"] = ffn_b2.reshape(1, GP).astype(bf16)
    shared["lnw1"] = np.tile(n1_w, (NSEG, 1)).astype(f32)
    shared["lnb1"] = np.tile(n1_b, (NSEG, 1)).astype(f32)
    shared["lnw2"] = np.tile(n2_w, (NSEG, 1)).astype(f32)
    shared["lnb2"] = np.tile(n2_b, (NSEG, 1)).astype(f32)
    w1pad = np.zeros((D_TOTAL, NE_PAD), np.float32)
    w1pad[:, :N_ENTITY] = w1f * INTW_SCALE
    # device tile [128, 4, NE_PAD]: w1[p, c, n] = W1[c*128+p, n]
    shared["w1"] = np.ascontiguousarray(
        w1pad.reshape(4, 128, NE_PAD).transpose(1, 0, 2)).astype(f8)
    tri = (np.arange(128)[None, :] >= np.arange(128)[:, None])
    shared["tri"] = tri.astype(bf16)

    in_maps = []
    for core in range(NCORES):
        m = dict(shared)
        xT = np.zeros((BPC, 128, 2, T), np.float32)
        c0T = np.zeros((BPC, 128, 2, T), np.float32)
        c0n = np.zeros((BPC, 128, NT, D_MODEL), np.float32)
        mscT = np.zeros((128, BPC * NT, NSEG), np.float32)  # [p, b*m, seg]
        mga = np.zeros((NSEG, BPC, T), np.float32)          # [seg, b, t]
        gpbias = np.zeros((NSEG, BPC, GP), np.float32)
        w2c = np.zeros((BPC, 128, 2, NE_PAD), np.float32)
        for b in range(BPC):
            gb = core * BPC + b
            hist = objs[gb, :Lh]
            x_nat = np.zeros((T, D_MODEL), np.float32)
            x_nat[:Lh] = (obj_embed[hist] + _time_enc(times[gb, :Lh])
                          + _time_enc(dt[gb, :Lh]))
            cur0 = np.zeros((T, D_MODEL), np.float32)
            cur0[:Lh] = _time_enc(times[gb, 1:])
            xT[b] = _pack_T(x_nat)
            c0T[b] = _pack_T(cur0)
            c0n[b] = _pack_N(cur0)

            gids = group_map[subs[gb] * N_REL + marks[gb]][:Lh]
            fm = mask[gb, :Lh].astype(np.float32)
            cnt = np.bincount(gids, weights=fm, minlength=NSEG)
            ts = np.arange(Lh)
            mga[gids, b, ts] = fm
            msc = np.zeros((T, NSEG), np.float32)      # [t, seg]
            msc[ts, gids] = fm / np.maximum(cnt, 1.0)[gids]
            mscT[:, NT * b:NT * b + NT, :] = msc.reshape(
                NT, 128, NSEG).transpose(1, 0, 2)
            sr = np.concatenate([sub_embed[subs[gb, 0]], rel_embed[marks[gb, 0]]])
            nz = (cnt > 0).astype(np.float32)
            gpbias[:, b, :] = (nz[:, None] * (sr @ gp_w[D_TOTAL:D_FEAT])[None, :]
                               + gp_b[None, :])
            const_row = (sr @ mg_w[D_TOTAL:D_FEAT]) @ int_w + mgb_row  # [8000]
            # gather/const chunk rhs: rows (p, j): (0..63, 0) = W2, (64, 0) =
            # const row, rest zero; pairs with gathc8 on the device.
            w2c[b, 0:GP, 0, :N_ENTITY] = w2f * INTW_SCALE
            w2c[b, GP, 0, :N_ENTITY] = const_row * INTW_SCALE
        m["xT"] = xT.astype(bf16)
        m["c0T"] = c0T.astype(bf16)
        m["c0n"] = c0n.astype(bf16)
        m["mscT"] = mscT.astype(bf16)
        m["mgath"] = mga.astype(bf16)
        m["gpbias"] = gpbias
        m["w2c"] = w2c.astype(f8)
        in_maps.append(m)
    return in_maps


def _chain_gens(*gens):
    for g in gens:
        yield from g


def _pin_act_tables():
    # bacc assigns each InstActivation a table set greedily, which makes a
    # mixed Exp/Ln instruction stream alternate between exp_and_others and
    # natural_log -> one ~1.3us ACT_TABLE_LOAD per switch.  Empty every set
    # except natural_log_exp_and_others (which contains Exp/Ln/Copy/Identity/
    # Square -- everything we use) so the chooser is forced onto one set;
    # positional set ids are preserved.
    import concourse.bacc as bacc
    from concourse import hw_specs
    if getattr(bacc.get_activation_tables, "_pinned", False):
        return
    orig = hw_specs.get_activation_tables
    KEEP = "natural_log_exp_and_others"

    def pinned(arch):
        t = dict(orig(arch))
        return {k: (v if k == KEEP else set()) for k, v in t.items()}

    pinned._pinned = True
    bacc.get_activation_tables = pinned


def build_nc(debug_stop=99):
    import concourse.bacc as bacc
    import concourse.mybir as mybir
    import concourse.tile as tile
    from concourse import masks as cmasks
    _pin_act_tables()
    NORM_MODE = os.environ.get("BASS_NORM", "recip")

    dtb = mybir.dt.bfloat16
    dtf = mybir.dt.float32
    dt8 = mybir.dt.float8e4
    AF = mybir.ActivationFunctionType
    ALU = mybir.AluOpType
    AX = mybir.AxisListType
    DR = mybir.MatmulPerfMode.DoubleRow

    nc = bacc.Bacc()

    def din(name, shape, dt=dtb):
        return nc.dram_tensor(name, shape, dt, kind="ExternalInput")

    xT_d = din("xT", [BPC, 128, 2, T])
    c0T_d = din("c0T", [BPC, 128, 2, T])
    c0n_d = din("c0n", [BPC, 128, NT, D_MODEL])
    wq_d = din("wq", [N_LAYERS, 128, 2, D_MODEL])
    wk_d = din("wk", [N_LAYERS, 128, 2, D_MODEL])
    wv_d = din("wv", [N_LAYERS, 128, 2, D_MODEL])
    wo_d = din("wo", [N_LAYERS, 128, 2, D_MODEL])
    mscT_d = din("mscT", [128, BPC * NT, NSEG])
    mgath_d = din("mgath", [NSEG, BPC, T])
    gpw_d = din("gpw", [128, 4, GP])
    gpbias_d = din("gpbias", [NSEG, BPC, GP], mybir.dt.float32)
    gain_d = din("gain", [GP, 3 * GP])
    gainb_d = din("gainb", [GP, 3], mybir.dt.float32)
    gaout_d = din("gaout", [GP, GP])
    gaoutb_d = din("gaoutb", [GP, 1], mybir.dt.float32)
    fw1_d = din("fw1", [GP, GP])
    fw2_d = din("fw2", [GP, GP])
    fb1_d = din("fb1", [1, GP])
    fb2_d = din("fb2", [1, GP])
    lnw1_d = din("lnw1", [NSEG, GP], mybir.dt.float32)
    lnb1_d = din("lnb1", [NSEG, GP], mybir.dt.float32)
    lnw2_d = din("lnw2", [NSEG, GP], mybir.dt.float32)
    lnb2_d = din("lnb2", [NSEG, GP], mybir.dt.float32)
    w1_d = din("w1", [128, 4, NE_PAD], dt8)
    w2c_d = din("w2c", [BPC, 128, 2, NE_PAD], dt8)
    tri_d = din("tri", [128, 128])
    out_d = nc.dram_tensor("out", [R, N_ENTITY], mybir.dt.bfloat16,
                           kind="ExternalOutput")
    debug_dump = bool(os.environ.get("BASS_DEBUG_DUMP"))
    if debug_dump:
        encdbg_d = nc.dram_tensor("encdbg", [N_LAYERS, 128, 2, T],
                                  mybir.dt.bfloat16, kind="ExternalOutput")
        odbg_d = nc.dram_tensor("odbg", [N_LAYERS, 128, 2, T],
                                mybir.dt.bfloat16, kind="ExternalOutput")

    with tile.TileContext(nc) as tc:
        with (
            tc.tile_pool(name="persist", bufs=1) as pp,
            tc.tile_pool(name="work", bufs=2) as wp,
            tc.tile_pool(name="acts", bufs=5) as ap,
            tc.tile_pool(name="outp", bufs=3) as op,
            tc.tile_pool(name="psum", bufs=1, space="PSUM") as ps,
        ):
            def pt(shape, tag, dt=dtb):
                return pp.tile(shape, dt, tag=tag, name=tag)

            def dma(dst, src):
                nc.sync.dma_start(dst, src)

            # PSUM budget (16KB/partition = 8 banks), by tag:
            #   pi 2x[128,1024]f32 = 8KB; f 2x[128,512]f32 = 4KB;
            #   po 2x[65,512]f32 = 4KB.  (pb lives in the f tag)
            def ps_pi():
                return ps.tile([128, 1024], dtf, tag="pi", bufs=2, name="pi")

            def ps_f(shape, dt=dtf):
                return ps.tile(shape, dt, tag="f", bufs=2, name="psf",
                               padded_shape=[128, 512])

            def ps_po():
                return ps.tile([65, T], dtf, tag="po", bufs=2, name="po")

            # ---- constants in ----
            # Input DMAs are split into small per-queue chunks, ordered so
            # batch 0 layer 0's operands land first: the front can start
            # ~5us in instead of waiting ~20us for monolithic transfers.
            xT = [pt([128, 2, T], f"xT{b}") for b in range(BPC)]
            c0T = [pt([128, 2, T], f"c0T{b}") for b in range(BPC)]
            c0n = [pt([128, NT, D_MODEL], f"c0n{b}") for b in range(BPC)]
            wq = [pt([128, 2, D_MODEL], f"wq{l}") for l in range(N_LAYERS)]
            wk = [pt([128, 2, D_MODEL], f"wk{l}") for l in range(N_LAYERS)]
            wo = [pt([128, 2, D_MODEL], f"wo{l}") for l in range(N_LAYERS)]
            wv = [pt([128, 2, D_MODEL], f"wv{l}") for l in range(N_LAYERS)]
            tri_s = pt([128, 128], "tri")

            def dma_qkv_w(l):
                dma(wq[l][:], wq_d[l])
                dma(wk[l][:], wk_d[l])
                dma(wv[l][:], wv_d[l])
                dma(wo[l][:], wo_d[l])

            def dma_batch_in(b):
                dma(c0T[b][:], c0T_d[b])
                dma(xT[b][:], xT_d[b])
                dma(c0n[b][:], c0n_d[b])

            dma_qkv_w(0)
            dma_batch_in(0)
            dma(tri_s[:], tri_d[:])
            dma_qkv_w(1)
            dma_batch_in(1)
            mscT_s = pt([128, BPC * NT, NSEG], "mscT")
            dma(mscT_s[:], mscT_d[:])
            mgath_s = pt([NSEG, BPC, T], "mgath")
            dma(mgath_s[:], mgath_d[:])
            gpw_s = pt([128, 4, GP], "gpw")
            dma(gpw_s[:], gpw_d[:])
            gpbias_s = pt([NSEG, BPC, GP], "gpbias", dtf)
            dma(gpbias_s[:], gpbias_d[:])
            gain_s = pt([GP, 3 * GP], "gain")
            dma(gain_s[:], gain_d[:])
            gainb_s = pt([GP, 3], "gainb", dtf)
            dma(gainb_s[:], gainb_d[:])
            gaout_s = pt([GP, GP], "gaout")
            dma(gaout_s[:], gaout_d[:])
            gaoutb_s = pt([GP, 1], "gaoutb", dtf)
            dma(gaoutb_s[:], gaoutb_d[:])
            fw1_s = pt([GP, GP], "fw1")
            dma(fw1_s[:], fw1_d[:])
            fw2_s = pt([GP, GP], "fw2")
            dma(fw2_s[:], fw2_d[:])
            fb1_s = pt([1, GP], "fb1")
            dma(fb1_s[:], fb1_d[:])
            fb2_s = pt([1, GP], "fb2")
            dma(fb2_s[:], fb2_d[:])
            ln_s = {}
            for nm, d in [("lnw1", lnw1_d), ("lnb1", lnb1_d),
                          ("lnw2", lnw2_d), ("lnb2", lnb2_d)]:
                ln_s[nm] = pt([NSEG, GP], nm, dtf)
                dma(ln_s[nm][:], d[:])

            # intensity weights, split into per-queue chunks so all DMA
            # rings pull in parallel and early strips land first
            w2c_s = [pt([128, 2, NE_PAD], f"w2c{b}", dt8) for b in range(BPC)]
            w1_s = pt([128, 4, NE_PAD], "w1", dt8)
            for c in range(2):
                for blk in range(4):
                    cs = slice(2048 * blk, 2048 * blk + 2048)
                    dma(w2c_s[0][:, c, cs], w2c_d[0, :, c, cs])
            for blk in range(4):
                for c in range(4):
                    cs = slice(2048 * blk, 2048 * blk + 2048)
                    dma(w1_s[:, c, cs], w1_d[:, c, cs])
            for c in range(2):
                for blk in range(4):
                    cs = slice(2048 * blk, 2048 * blk + 2048)
                    dma(w2c_s[1][:, c, cs], w2c_d[1, :, c, cs])

            eps_s = pt([NSEG, 1], "eps", dtf)
            nc.gpsimd.memset(eps_s[:], 1e-5)
            ident = pt([128, 128], "ident")
            cmasks.make_identity(nc, ident[:])
            ones_r = pt([1, T], "ones_r")
            nc.gpsimd.memset(ones_r[:], 1.0)

            # gather/const stationary: [128, 2, T] fp8; row (64, 0) is the
            # all-ones row pairing with the const row of w2c; rows 65..127
            # and all of j=1 stay zero.
            gathc8 = [pt([128, 2, T], f"gathc8{b}", dt8) for b in range(BPC)]
            enc8 = [pt([128, 2 * N_LAYERS, T], f"enc8{b}", dt8)
                    for b in range(BPC)]
            for b in range(BPC):
                nc.gpsimd.memset(gathc8[b][:], 0.0)
                nc.gpsimd.memset(gathc8[b][GP:GP + 1, 0, :], 1.0)

            def layernorm(xin, wtile, btile, outf, outb):
                P = xin.shape[0]
                s1 = wp.tile([P, 1], dtf, tag="lns", name="lns")
                nc.vector.reduce_sum(s1[:], xin[:], axis=AX.X)
                mu = wp.tile([P, 1], dtf, tag="lnm", name="lnm")
                nc.vector.tensor_scalar_mul(mu[:], s1[:], 1.0 / GP)
                xc = wp.tile([P, GP], dtf, tag="lnxc", name="lnxc")
                nc.vector.tensor_scalar(xc[:], xin[:], mu[:], None,
                                        op0=ALU.subtract)
                sq = wp.tile([P, GP], dtf, tag="lnsq", name="lnsq")
                vs = wp.tile([P, 1], dtf, tag="lnvs", name="lnvs")
                nc.scalar.activation(sq[:], xc[:], AF.Square, accum_out=vs[:])
                lnv = wp.tile([P, 1], dtf, tag="lnlv", name="lnlv")
                nc.scalar.activation(lnv[:], vs[:], AF.Ln, scale=1.0 / GP,
                                     bias=eps_s[:P])
                rstd = wp.tile([P, 1], dtf, tag="lnrs", name="lnrs")
                nc.scalar.activation(rstd[:], lnv[:], AF.Exp, scale=-0.5)
                nc.vector.scalar_tensor_tensor(
                    outf[:], xc[:], rstd[:], wtile[:], op0=ALU.mult, op1=ALU.mult)
                nc.vector.tensor_add(outf[:], outf[:], btile[:])
                nc.vector.tensor_copy(outb[:], outf[:])

            # ==== software-pipelined emission ====
            # Engines execute their instruction streams strictly in order,
            # so overlap between independent work REQUIRES interleaving at
            # emission time.  Schedule:
            #   front(b0) ; [post(b0) || front(b1)-begin] ;
            #   [intensity(b0) || front(b1)-rest + post(b1)] ; intensity(b1)
            ST = [{"curT": c0T[b], "curn": c0n[b],
                   "encn": [None] * N_LAYERS} for b in range(BPC)]

            def enc_gen(b):
                # Batch 0's front runs while ACT is idle: staging copies and
                # softmax normalization go on ACT to shorten the critical
                # chain.  Batch 1's front overlaps batch 0's ACT-bound
                # intensity epilogue, so everything movable stays on the DVE.
                on_act = (b == 0)

                def stage(dst, src):
                    nc.vector.tensor_copy(dst, src)

                for l in range(N_LAYERS):
                    curT, curn = ST[b]["curT"], ST[b]["curn"]
                    qT = ap.tile([128, 2, T], dtb, tag="qT", bufs=2, name="qT")
                    kT = ap.tile([128, 2, T], dtb, tag="kT", bufs=2, name="kT")
                    for c2 in range(2):
                        psq = ps_f([128, T])
                        for c in range(2):
                            nc.tensor.matmul(
                                psq[:], wq[l][:, c, 128 * c2:128 * c2 + 128],
                                curT[:, c, :], start=(c == 0), stop=(c == 1))
                        stage(qT[:, c2, :], psq[:])
                        psk = ps_f([128, T])
                        for c in range(2):
                            nc.tensor.matmul(
                                psk[:], wk[l][:, c, 128 * c2:128 * c2 + 128],
                                xT[b][:, c, :], start=(c == 0), stop=(c == 1))
                        stage(kT[:, c2, :], psk[:])
                        yield
                    vcat = []
                    for m in range(NT):
                        psv = ps_f([128, D_MODEL])
                        for c in range(2):
                            nc.tensor.matmul(
                                psv[:], xT[b][:, c, 128 * m:128 * m + 128],
                                wv[l][:, c, :], start=(c == 0), stop=(c == 1))
                        vc = ap.tile([128, N_HEADS, 65], dtb, tag=f"vcat{m}",
                                     bufs=2, name=f"vcat{m}")
                        stage(vc[:, :, 0:64],
                              psv[:].rearrange("p (h e) -> p h e", h=N_HEADS))
                        nc.vector.memset(vc[:, :, 64:65], 1.0)
                        vcat.append(vc)
                        if m == 1:
                            yield
                    yield
                    # Scores phase: all (h, j) score+exp+mask units emitted
                    # back-to-back so no engine queue ever stalls on a long
                    # cross-engine round-trip (engines run their queues in
                    # order; a waiting instruction blocks everything behind
                    # it, including the other batch's interleaved intensity
                    # work).
                    aTs = [[None] * NT for _ in range(N_HEADS)]
                    cnt = 0
                    for h in range(N_HEADS):
                        bp, hc = 64 * (h % 2), h // 2
                        for j in range(2):
                            q0 = 128 * j
                            nq = T - q0
                            pss = ps_f([128, T])
                            nc.tensor.matmul(
                                pss[:, :nq],
                                kT[bp:bp + 64, hc, q0:q0 + 128],
                                qT[bp:bp + 64, hc, q0:T])
                            aT = ap.tile([128, T], dtb, tag=f"aT{b}{h}{j}",
                                         bufs=1, name="aT")
                            nc.scalar.activation(aT[:, :nq], pss[:, :nq], AF.Exp)
                            nc.vector.tensor_mul(
                                aT[:, 0:128], aT[:, 0:128], tri_s[:])
                            aTs[h][j] = aT[:, :nq]
                            cnt += 1
                            if cnt % 3 == 0:
                                yield
                        # j=2 (256 cols) and j=3 (128 cols) share one PSUM
                        # tile and one Exp: j2 at cols [0:256], j3 at
                        # [256:384] (partition dim = k-pos local to each j).
                        pss = ps_f([128, T])
                        nc.tensor.matmul(
                            pss[:, 0:256], kT[bp:bp + 64, hc, 256:384],
                            qT[bp:bp + 64, hc, 256:T],
                            skip_group_check=True)
                        nc.tensor.matmul(
                            pss[:, 256:384], kT[bp:bp + 64, hc, 384:512],
                            qT[bp:bp + 64, hc, 384:T],
                            skip_group_check=True)
                        aT23 = ap.tile([128, T], dtb, tag=f"aT{b}{h}2",
                                       bufs=1, name="aT23")
                        nc.scalar.activation(aT23[:, 0:384], pss[:, 0:384],
                                             AF.Exp)
                        nc.vector.tensor_mul(
                            aT23[:, 0:128], aT23[:, 0:128], tri_s[:])
                        nc.vector.tensor_mul(
                            aT23[:, 256:384], aT23[:, 256:384], tri_s[:])
                        aTs[h][2] = aT23[:, 0:256]
                        aTs[h][3] = aT23[:, 256:384]
                        cnt += 1
                        if cnt % 3 == 0:
                            yield
                    # po phase per head, then softmax normalization: 1/colsum
                    # on DVE (keeps ACT free for softplus), rank-1 broadcast
                    # matmul, multiply.
                    oT = ap.tile([128, 2, T], dtb, tag="oT", bufs=2, name="oT")
                    for h in range(N_HEADS):
                        bp, hc = 64 * (h % 2), h // 2
                        po = ps_po()
                        for j in range(NT):
                            q0 = 128 * j
                            nc.tensor.matmul(
                                po[:, q0:T], vcat[j][:, h, :],
                                aTs[h][j][:, :T - q0],
                                start=(j == 0), stop=(j == NT - 1),
                                skip_group_check=True)
                        oraw = wp.tile([64, T], dtb, tag="oraw", name="oraw")
                        stage(oraw[:], po[0:64, :])
                        rbf = wp.tile([1, T], dtb, tag="rbf", name="rbf")
                        if on_act or NORM_MODE == "act":
                            rsb = wp.tile([1, T], dtf, tag="rsb", name="rsb")
                            nc.scalar.activation(rsb[:], po[64:65, :], AF.Ln)
                            nc.scalar.activation(rbf[:], rsb[:], AF.Exp,
                                                 scale=-1.0)
                        else:
                            # reciprocal_approx_fast is a custom DVE op
                            # (BITWISE_NOT seed); it reads garbage from PSUM,
                            # so stage the colsum row into SBUF first.
                            rcs = wp.tile([1, T], dtf, tag="rcs", name="rcs")
                            nc.vector.tensor_copy(rcs[:], po[64:65, :])
                            rr = wp.tile([1, T], dtf, tag="rr", name="rr")
                            nc.vector.reciprocal_approx_fast(rr[:], rcs[:])
                            nc.vector.tensor_copy(rbf[:], rr[:])
                        pb = ps_f([64, T])
                        nc.tensor.matmul(pb[:], ones_r[0:1, 0:64], rbf[:])
                        nc.vector.tensor_mul(oT[bp:bp + 64, hc, :], oraw[:],
                                             pb[:])
                        yield
                    if debug_dump and b == 0:
                        nc.sync.dma_start(odbg_d[l], oT[:])
                    eT = pt([128, 2, T], f"encT{l}{b}")
                    en = pt([128, NT, D_MODEL], f"encn{l}{b}")
                    for c2 in range(2):
                        psp = ps_f([128, T])
                        for c in range(2):
                            nc.tensor.matmul(
                                psp[:], wo[l][:, c, 128 * c2:128 * c2 + 128],
                                oT[:, c, :], start=(c == 0), stop=(c == 1))
                        nc.vector.tensor_add(eT[:, c2, :], curT[:, c2, :],
                                             psp[:])
                    nc.vector.tensor_copy(enc8[b][:, 2 * l:2 * l + 2, :],
                                          eT[:])
                    if debug_dump and b == 0:
                        nc.sync.dma_start(encdbg_d[l], eT[:])
                    yield
                    for m in range(NT):
                        psn = ps_f([128, D_MODEL])
                        for c in range(2):
                            nc.tensor.matmul(
                                psn[:], oT[:, c, 128 * m:128 * m + 128],
                                wo[l][:, c, :], start=(c == 0), stop=(c == 1))
                        nc.vector.tensor_add(en[:, m, :], curn[:, m, :], psn[:])
                        if m == 1:
                            yield
                    ST[b]["encn"][l] = en
                    ST[b]["curT"], ST[b]["curn"] = eT, en
                    yield

            def post_gen(b):
                encn = ST[b]["encn"]
                # ---- group scatter A^T[d, seg] (batch-local 64 segs) ----
                AT_s = wp.tile([128, 4, NSEG], dtb, tag="AT", name="AT")
                for dt_i in range(4):
                    l, c2 = dt_i // 2, dt_i % 2
                    pa = ps_f([128, NSEG])
                    for m in range(NT):
                        nc.tensor.matmul(
                            pa[:], encn[l][:, m, 128 * c2:128 * c2 + 128],
                            mscT_s[:, NT * b + m, :],
                            start=(m == 0), stop=(m == NT - 1))
                    nc.vector.tensor_copy(AT_s[:, dt_i, :], pa[:])
                    if dt_i % 2:
                        yield
                # ---------------- group block ([64, *]) ----------------
                pg = ps_f([NSEG, GP])
                for dt_i in range(4):
                    nc.tensor.matmul(pg[:], AT_s[:, dt_i, :], gpw_s[:, dt_i, :],
                                     start=(dt_i == 0), stop=(dt_i == 3))
                gp_f = wp.tile([NSEG, GP], dtf, tag="gp_f", name="gp_f")
                nc.vector.tensor_add(gp_f[:], pg[:], gpbias_s[:, b, :])
                gp_bf = wp.tile([NSEG, GP], dtb, tag="gp_bf", name="gp_bf")
                nc.vector.tensor_copy(gp_bf[:], gp_f[:])
                ptr = ps_f([GP, NSEG], dtb)
                nc.tensor.transpose(ptr[:], gp_bf[:], ident[0:NSEG, 0:NSEG])
                gpT = wp.tile([GP, NSEG], dtb, tag="gpT", name="gpT")
                nc.vector.tensor_copy(gpT[:], ptr[:])
                yield
                qkvT = []
                for i in range(3):
                    pq = ps_f([GP, NSEG])
                    nc.tensor.matmul(pq[:], gain_s[:, GP * i:GP * i + GP],
                                     gpT[:])
                    tq = wp.tile([GP, NSEG], dtb, tag=f"qkvT{i}",
                                 name=f"qkvT{i}")
                    nc.vector.tensor_scalar(
                        tq[:], pq[:], gainb_s[:, i:i + 1], None, op0=ALU.add)
                    qkvT.append(tq)
                yield
                vc2 = wp.tile([GP, GH, 33], dtb, tag="vc2", name="vc2")
                for h in range(GH):
                    pv = ps_f([GP, 32], dtb)
                    nc.tensor.transpose(
                        pv[:], qkvT[2][32 * h:32 * h + 32, :],
                        ident[32 * h:32 * h + 32, 32 * h:32 * h + 32])
                    nc.vector.tensor_copy(vc2[:, h, 0:32], pv[:])
                nc.vector.memset(vc2[:, :, 32:33], 1.0)
                yield
                oT2 = wp.tile([GP, NSEG], dtb, tag="oT2", name="oT2")
                for h in range(GH):
                    ps1 = ps_f([64, 64])
                    nc.tensor.matmul(
                        ps1[:], qkvT[1][32 * h:32 * h + 32, :],
                        qkvT[0][32 * h:32 * h + 32, :])
                    a2 = wp.tile([64, 64], dtb, tag="a2", name="a2")
                    nc.scalar.activation(a2[:], ps1[:], AF.Exp)
                    po2 = ps_f([33, 64])
                    nc.tensor.matmul(po2[:], vc2[:, h, :], a2[:])
                    r2 = wp.tile([1, 64], dtf, tag="r2", name="r2")
                    nc.scalar.activation(r2[:], po2[32:33, :], AF.Ln)
                    r2b = wp.tile([1, 64], dtb, tag="r2b", name="r2b")
                    nc.scalar.activation(r2b[:], r2[:], AF.Exp, scale=-1.0)
                    pb2 = ps_f([32, 64])
                    nc.tensor.matmul(pb2[:], ones_r[0:1, 0:32], r2b[:])
                    orw2 = wp.tile([32, 64], dtb, tag="orw2", name="orw2")
                    nc.vector.tensor_copy(orw2[:], po2[0:32, :])
                    nc.vector.tensor_mul(oT2[32 * h:32 * h + 32, :], orw2[:],
                                         pb2[:])
                    yield
                pga = ps_f([GP, NSEG])
                nc.tensor.matmul(pga[:], gaout_s[:], oT2[:])
                t1 = wp.tile([GP, NSEG], dtb, tag="t1", name="t1")
                nc.vector.scalar_tensor_tensor(
                    t1[:], pga[:], gaoutb_s[:, 0:1], gpT[:],
                    op0=ALU.add, op1=ALU.add)
                px1 = ps_f([NSEG, GP], dtb)
                nc.tensor.transpose(px1[:], t1[:], ident[0:GP, 0:GP])
                x1 = wp.tile([NSEG, GP], dtf, tag="x1", name="x1")
                nc.vector.tensor_copy(x1[:], px1[:])
                gn_f = wp.tile([NSEG, GP], dtf, tag="gn_f", name="gn_f")
                gn_b = wp.tile([NSEG, GP], dtb, tag="gn_b", name="gn_b")
                layernorm(x1, ln_s["lnw1"][0:NSEG], ln_s["lnb1"][0:NSEG],
                          gn_f, gn_b)
                yield
                pgt = ps_f([GP, NSEG], dtb)
                nc.tensor.transpose(pgt[:], gn_b[:], ident[0:NSEG, 0:NSEG])
                gnT = wp.tile([GP, NSEG], dtb, tag="gnT", name="gnT")
                nc.vector.tensor_copy(gnT[:], pgt[:])
                ph1 = ps_f([NSEG, GP])
                nc.tensor.matmul(ph1[:], ones_r[0:1, 0:NSEG], fb1_s[:],
                                 start=True, stop=False)
                nc.tensor.matmul(ph1[:], gnT[:], fw1_s[:], start=False,
                                 stop=True)
                h1b = wp.tile([NSEG, GP], dtb, tag="h1b", name="h1b")
                nc.vector.tensor_scalar_max(h1b[:], ph1[:], 0.0)
                ph1t = ps_f([GP, NSEG], dtb)
                nc.tensor.transpose(ph1t[:], h1b[:], ident[0:NSEG, 0:NSEG])
                h1T = wp.tile([GP, NSEG], dtb, tag="h1T", name="h1T")
                nc.vector.tensor_copy(h1T[:], ph1t[:])
                yield
                ph2 = ps_f([NSEG, GP])
                nc.tensor.matmul(ph2[:], ones_r[0:1, 0:NSEG], fb2_s[:],
                                 start=True, stop=False)
                nc.tensor.matmul(ph2[:], h1T[:], fw2_s[:], start=False,
                                 stop=True)
                x2 = wp.tile([NSEG, GP], dtf, tag="x2", name="x2")
                nc.vector.tensor_add(x2[:], ph2[:], gn_f[:])
                go_f = wp.tile([NSEG, GP], dtf, tag="go_f", name="go_f")
                go_b = wp.tile([NSEG, GP], dtb, tag="go_b", name="go_b")
                layernorm(x2, ln_s["lnw2"][0:NSEG], ln_s["lnb2"][0:NSEG],
                          go_f, go_b)
                yield
                # ---- gather^T: gathc8[p, 0, t] = gout[gid(t), p]*fm(t) ----
                pgh = ps_f([GP, T])
                nc.tensor.matmul(pgh[:], go_b[:], mgath_s[:, b, :])
                nc.vector.tensor_copy(gathc8[b][0:GP, 0, :], pgh[:])
                yield

            def int_gen(b):
                # -------- intensity head (fp8 DoubleRow, K=512+65) --------
                for P in range(NPAIR):
                    for m in range(NT):
                        ot = op.tile([128, 2048], dtb, tag="ot", name="ot")
                        width = 0
                        for s2 in range(2):
                            sp = 2 * P + s2
                            ncs = min(1024, N_ENTITY - 1024 * sp)
                            pi = ps_pi()
                            for h2 in range(2):
                                col0 = 1024 * sp + 512 * h2
                                ncol = min(512, N_ENTITY - col0)
                                if ncol <= 0:
                                    continue
                                pv = pi[:, 512 * h2:512 * h2 + ncol]
                                for c in range(2):
                                    nc.tensor.matmul(
                                        pv,
                                        enc8[b][:, 2 * c:2 * c + 2,
                                                128 * m:128 * m + 128],
                                        w1_s[:, 2 * c:2 * c + 2,
                                             col0:col0 + ncol],
                                        perf_mode=DR, start=(c == 0),
                                        stop=False, skip_group_check=True)
                                nc.tensor.matmul(
                                    pv,
                                    gathc8[b][:, :, 128 * m:128 * m + 128],
                                    w2c_s[b][:, :, col0:col0 + ncol],
                                    perf_mode=DR, start=False, stop=True,
                                    skip_group_check=True)
                            nc.scalar.activation(
                                ot[:, 1024 * s2:1024 * s2 + ncs],
                                pi[:, :ncs], AF.Exp, scale=1.0 / INTW_SCALE)
                            width += ncs
                        # softplus second half: one Ln per strip-pair
                        nc.scalar.activation(ot[:, :width], ot[:, :width],
                                             AF.Ln, bias=1.0)
                        nc.sync.dma_start(
                            out_d[T * b + 128 * m:T * b + 128 * m + 128,
                                  2048 * P:2048 * P + width],
                            ot[:, :width])
                        yield

            def drain(g):
                for _ in g:
                    pass

            def zip_all(*gens):
                gens = list(gens)
                while gens:
                    for g in list(gens):
                        if next(g, StopIteration) is StopIteration:
                            gens.remove(g)

            def zip_until(primary, filler):
                # pace on `primary`; advance `filler` one stage per step
                for _ in primary:
                    next(filler, None)

            # Schedule: front(b0) alone; post(b0) paced against front(b1)'s
            # beginning (b1's short-dependency q/k/score chunks execute on
            # the otherwise-idle PE while post(b0)'s serial group-block chain
            # runs); then intensity(b0) interleaved 1:2 with the REST of
            # front(b1)+post(b1) -- intensity units lead so their ACT work
            # is never queued behind unissued front(b1) instructions, and
            # the 2x g1 stepping lets post(b1) finish before intensity(b0)
            # drains so intensity(b1) starts immediately after.
            drain(enc_gen(0))
            g1 = (_chain_gens(enc_gen(1), post_gen(1))
                  if BPC > 1 else iter(()))
            if debug_stop >= 3:
                zip_until(post_gen(0), g1)
                if debug_stop >= 5:
                    # Intensity(b0) units lead the remaining front(b1)
                    # chunks 1:2 -- the units' ACT work is never queued
                    # behind unissued front(b1) instructions, and the double
                    # g1 stepping lets post(b1) finish before intensity(b0)
                    # drains.
                    gi0 = int_gen(0)
                    next(gi0, None)
                    next(gi0, None)
                    while True:
                        if next(gi0, StopIteration) is StopIteration:
                            gi0 = None
                            break
                        if (next(g1, StopIteration) is StopIteration or
                                next(g1, StopIteration) is StopIteration):
                            break
                    drain(g1)
                    if gi0 is not None:
                        drain(gi0)
                    for bb in range(1, BPC):
                        drain(int_gen(bb))
                else:
                    drain(g1)
            elif BPC > 1:
                drain(g1)
    nc.compile()
    return nc


def _get_nc():
    if "nc" not in _CACHED:
        _CACHED["nc"] = build_nc()
    return _CACHED["nc"]


def _install_ntff_hook():
    """Best-effort: register the axon NTFF profile hook so trace=True works."""
    import sys, types
    if "antenv.axon_hooks" in sys.modules:
        return
    try:
        import antenv  # noqa
        from trn_agent_boot.trn_boot import _ntff_profile_via_ctypes
        mod = types.ModuleType("antenv.axon_hooks")
        hook = [_ntff_profile_via_ctypes("/opt/axon/libaxon_pjrt.so")]
        mod.set_axon_ntff_profile_hook = lambda h: hook.__setitem__(0, h)
        mod.get_axon_ntff_profile_hook = lambda: hook[0]
        sys.modules["antenv.axon_hooks"] = mod
    except Exception:
        pass


def kernel(**inputs):
    global LAST_EXEC_NS, LAST_RESULTS
    from concourse.bass_utils import run_bass_kernel_spmd

    in_maps = prep_inputs(inputs)
    nc = _get_nc()
    trace = bool(os.environ.get("BASS_TRACE"))
    if trace:
        _install_ntff_hook()
    res = run_bass_kernel_spmd(
        nc, in_maps, core_ids=list(range(NCORES)), trace=trace)
    LAST_RESULTS = res
    LAST_EXEC_NS = res.exec_time_ns
    out = np.empty((B, Lh, N_ENTITY), np.float32)
    for core in range(NCORES):
        o = res.results[core]["out"]
        for b in range(BPC):
            out[core * BPC + b] = o[T * b:T * b + Lh, :].astype(np.float32)
    return out


# revision 28
# speedup vs baseline: 1.2373x; 1.0130x over previous
"""GAttNHP model as a Bass/Tile kernel on 8 Trainium2 NeuronCores.

Strategy: pure data-parallel over batch (B=16 -> 2 batches/core, no
collectives).  bf16 matmuls accumulating in fp32 PSUM; the dominant
intensity head runs fp8e4 with DoubleRow (2 weights/cell).

Key structural move vs the straightforward lowering: the merge Linear is
folded into the intensity head ON THE HOST.  With
  enhanced = [enc | s_emb | r_emb | gathered] @ mg_w + mg_b
  out      = softplus(enhanced @ int_w + int_b)
we precompute W1 = mg_w[:512] @ int_w (enc part), W2 = mg_w[1024:] @ int_w
(gathered part) and a per-batch constant row (s/r embeddings are constant
per batch).  The device-side intensity matmul then has K = 512 (enc, fp8
DoubleRow, 2 instructions) + 64+1 (gather one-hot + const row, packed into
one more DoubleRow instruction) instead of K = 1024 + a separate [*,1088]
merge matmul.  The gather chunk's stationary is padded to the full 128
partitions (zero rows) so every matmul in a PSUM accumulation group covers
the same PE row-group (disjoint row-groups race on the PSUM accumulate and
fault the exec unit).

The intensity epilogue softplus = ln(1+exp(x)) is two full ACT passes over
every output element and is the hard floor of the kernel (~110us/core); the
schedule is arranged so the Scalar engine never idles during the intensity
phases: Exp per 1024-col strip from a 2-bank PSUM tile, one Ln per 2048-col
strip-pair, attention softmax normalization moved off ACT onto the DVE
(reciprocal_approx_fast on the appended-ones column sum), and ACT keeps a
single activation table set (natural_log_exp_and_others) throughout -- see
_pin_act_tables.

Device pipeline, emitted as a complete chain PER BATCH so batch 1's
latency-bound front half overlaps batch 0's ACT-bound intensity epilogue:
  1. AttNHP encoder, 2 layers, activations kept in transposed [d, t]
     layout (plus a natural [t, d] copy for the group scatter).  Causal
     softmax in s^T layout: exp (no max-subtract, scores are tiny),
     triangular mask on the diagonal block, column sums via an appended
     ones-column on v, normalization via DVE reciprocal + rank-1
     broadcast matmul.  Encoder output additionally cast to fp8 (enc8)
     as the intensity stationary.
  2. Group scatter-mean as a matmul against a host-built one-hot
     matrix (64 batch-local segments on partitions 0..63).
  3. Tiny group transformer block (attn + ffn + 2 layernorms; rstd via
     ln/exp so the whole kernel uses ONE ACT table set).
  4. Gather back gathc8[p,t] = gout[gid(t),p]*fm(t) via one matmul
     against the one-hot gather matrix, cast fp8, plus a preset
     all-ones row (pairs with the per-batch const row of W2).
  5. Intensity head: per (strip-pair, t-tile): 3 fp8 DoubleRow matmuls
     per 512-col half into 2-bank [128,1024] PSUM strips; Exp per
     strip, Ln per strip-pair, DMA out per strip-pair.
"""

import os

import numpy as np
import ml_dtypes

bf16 = ml_dtypes.bfloat16
f8 = ml_dtypes.float8_e4m3
INTW_SCALE = 128.0

N_ENTITY = 8000
N_REL = 100
N_GROUPS = 64
HIDDEN = 256
D_MODEL = 256
N_LAYERS = 2
N_HEADS = 4
GP = 64
GH = 2
D_TOTAL = D_MODEL * N_LAYERS          # 512
D_FEAT = D_TOTAL + 2 * HIDDEN         # 1024
B, L = 16, 512
Lh = L - 1                            # 511
NCORES = 8
BPC = B // NCORES                     # 2 batches per core
T = 512                               # padded seq length
NT = T // 128                         # 4 t-tiles per batch
R = BPC * T                           # 1024 rows per core
NSEG = N_GROUPS                       # 64 batch-local segments
NE_PAD = 8192
NSTRIP = 8                            # 1024-col strips (last covers 832)
NPAIR = NSTRIP // 2                   # 2048-col strip-pairs

LAST_EXEC_NS = None
LAST_RESULTS = None
_CACHED = {}


def _time_enc(t, d=D_MODEL):
    i = np.arange(d // 2)
    freqs = np.exp(-np.log(10000.0) * (2.0 * i / d)).astype(np.float32)
    ang = t[..., None].astype(np.float32) * freqs
    return np.concatenate([np.sin(ang), np.cos(ang)], axis=-1).astype(np.float32)


def _pack_T(a):
    # [512, 256] natural -> [128, 2, 512] transposed tiles (d = c*128+p)
    return np.ascontiguousarray(a.T.reshape(2, 128, T).transpose(1, 0, 2))


def _pack_N(a):
    # [512, 256] natural -> [128, 4, 256] natural tiles (t = m*128+p)
    return np.ascontiguousarray(a.reshape(NT, 128, D_MODEL).transpose(1, 0, 2))


def _wpack(w):
    # [256, 256] -> [128, 2, 256]  (rows d = c*128+p)
    return np.ascontiguousarray(w.reshape(2, 128, D_MODEL).transpose(1, 0, 2))


def prep_inputs(inputs):
    """Returns in_maps per core."""
    f32 = np.float32
    subs = np.asarray(inputs["subs"])
    marks = np.asarray(inputs["marks"])
    objs = np.asarray(inputs["objs"])
    times = np.asarray(inputs["times"], f32)
    dt = np.asarray(inputs["dt"], f32)
    mask = np.asarray(inputs["mask"])
    group_map = np.asarray(inputs["group_map"])
    g = lambda k: np.asarray(inputs[k], f32)
    obj_embed = g("obj_embed")
    core_Wq, core_Wk, core_Wv, core_Wo = (
        g("core_Wq"), g("core_Wk"), g("core_Wv"), g("core_Wo"))
    sub_embed, rel_embed = g("sub_embed"), g("rel_embed")
    gp_w, gp_b = g("gp_w"), g("gp_b")
    ga_in_w, ga_in_b = g("ga_in_w"), g("ga_in_b")
    ga_out_w, ga_out_b = g("ga_out_w"), g("ga_out_b")
    ffn_w1, ffn_b1, ffn_w2, ffn_b2 = g("ffn_w1"), g("ffn_b1"), g("ffn_w2"), g("ffn_b2")
    n1_w, n1_b, n2_w, n2_b = g("n1_w"), g("n1_b"), g("n2_w"), g("n2_b")
    mg_w, mg_b = g("mg_w"), g("mg_b")
    int_w, int_b = g("int_w"), g("int_b")

    # ---- host-fused intensity weights ----
    # enhanced = [enc | sr | gathered] @ mg_w + mg_b; out = sp(enh @ int_w + int_b)
    w1f = mg_w[:D_TOTAL] @ int_w                     # [512, 8000]
    w2f = mg_w[D_FEAT:D_FEAT + GP] @ int_w           # [64, 8000]
    mgb_row = mg_b @ int_w + int_b                   # [8000]

    shared = {}
    shared["wq"] = np.stack([_wpack(core_Wq[l] / np.sqrt(64.0))
                             for l in range(N_LAYERS)]).astype(bf16)
    shared["wk"] = np.stack([_wpack(core_Wk[l]) for l in range(N_LAYERS)]).astype(bf16)
    shared["wv"] = np.stack([_wpack(core_Wv[l]) for l in range(N_LAYERS)]).astype(bf16)
    shared["wo"] = np.stack([_wpack(core_Wo[l]) for l in range(N_LAYERS)]).astype(bf16)
    shared["gpw"] = np.ascontiguousarray(
        gp_w[:D_TOTAL].reshape(4, 128, GP).transpose(1, 0, 2)).astype(bf16)
    gain = ga_in_w.copy()
    gainb = ga_in_b.copy().reshape(3, GP).T.copy()   # [64, 3] columns q/k/v
    gain[:, :GP] /= np.sqrt(32.0)
    gainb[:, 0] /= np.sqrt(32.0)
    shared["gain"] = gain.astype(bf16)
    shared["gainb"] = gainb.astype(f32)
    shared["gaout"] = ga_out_w.astype(bf16)
    shared["gaoutb"] = ga_out_b.reshape(GP, 1).astype(f32)
    shared["fw1"] = ffn_w1.astype(bf16)
    shared["fw2"] = ffn_w2.astype(bf16)
    shared["fb1"] = ffn_b1.reshape(1, GP).astype(bf16)
    shared["fb2"] = ffn_b2.reshape(1, GP).astype(bf16)
    shared["lnw1"] = np.tile(n1_w, (NSEG, 1)).astype(f32)
    shared["lnb1"] = np.tile(n1_b, (NSEG, 1)).astype(f32)
    shared["lnw2"] = np.tile(n2_w, (NSEG, 1)).astype(f32)
    shared["lnb2"] = np.tile(n2_b, (NSEG, 1)).astype(f32)
    w1pad = np.zeros((D_TOTAL, NE_PAD), np.float32)
    w1pad[:, :N_ENTITY] = w1f * INTW_SCALE
    # device tile [128, 4, NE_PAD]: w1[p, c, n] = W1[c*128+p, n]
    shared["w1"] = np.ascontiguousarray(
        w1pad.reshape(4, 128, NE_PAD).transpose(1, 0, 2)).astype(f8)
    tri = (np.arange(128)[None, :] >= np.arange(128)[:, None])
    shared["tri"] = tri.astype(bf16)

    in_maps = []
    for core in range(NCORES):
        m = dict(shared)
        xT = np.zeros((BPC, 128, 2, T), np.float32)
        c0T = np.zeros((BPC, 128, 2, T), np.float32)
        c0n = np.zeros((BPC, 128, NT, D_MODEL), np.float32)
        mscT = np.zeros((128, BPC * NT, NSEG), np.float32)  # [p, b*m, seg]
        mga = np.zeros((NSEG, BPC, T), np.float32)          # [seg, b, t]
        gpbias = np.zeros((NSEG, BPC, GP), np.float32)
        w2c = np.zeros((BPC, 128, 2, NE_PAD), np.float32)
        for b in range(BPC):
            gb = core * BPC + b
            hist = objs[gb, :Lh]
            x_nat = np.zeros((T, D_MODEL), np.float32)
            x_nat[:Lh] = (obj_embed[hist] + _time_enc(times[gb, :Lh])
                          + _time_enc(dt[gb, :Lh]))
            cur0 = np.zeros((T, D_MODEL), np.float32)
            cur0[:Lh] = _time_enc(times[gb, 1:])
            xT[b] = _pack_T(x_nat)
            c0T[b] = _pack_T(cur0)
            c0n[b] = _pack_N(cur0)

            gids = group_map[subs[gb] * N_REL + marks[gb]][:Lh]
            fm = mask[gb, :Lh].astype(np.float32)
            cnt = np.bincount(gids, weights=fm, minlength=NSEG)
            ts = np.arange(Lh)
            mga[gids, b, ts] = fm
            msc = np.zeros((T, NSEG), np.float32)      # [t, seg]
            msc[ts, gids] = fm / np.maximum(cnt, 1.0)[gids]
            mscT[:, NT * b:NT * b + NT, :] = msc.reshape(
                NT, 128, NSEG).transpose(1, 0, 2)
            sr = np.concatenate([sub_embed[subs[gb, 0]], rel_embed[marks[gb, 0]]])
            nz = (cnt > 0).astype(np.float32)
            gpbias[:, b, :] = (nz[:, None] * (sr @ gp_w[D_TOTAL:D_FEAT])[None, :]
                               + gp_b[None, :])
            const_row = (sr @ mg_w[D_TOTAL:D_FEAT]) @ int_w + mgb_row  # [8000]
            # gather/const chunk rhs: rows (p, j): (0..63, 0) = W2, (64, 0) =
            # const row, rest zero; pairs with gathc8 on the device.
            w2c[b, 0:GP, 0, :N_ENTITY] = w2f * INTW_SCALE
            w2c[b, GP, 0, :N_ENTITY] = const_row * INTW_SCALE
        m["xT"] = xT.astype(bf16)
        m["c0T"] = c0T.astype(bf16)
        m["c0n"] = c0n.astype(bf16)
        m["mscT"] = mscT.astype(bf16)
        m["mgath"] = mga.astype(bf16)
        m["gpbias"] = gpbias
        m["w2c"] = w2c.astype(f8)
        in_maps.append(m)
    return in_maps


def _chain_gens(*gens):
    for g in gens:
        yield from g


def _pin_act_tables():
    # bacc assigns each InstActivation a table set greedily, which makes a
    # mixed Exp/Ln instruction stream alternate between exp_and_others and
    # natural_log -> one ~1.3us ACT_TABLE_LOAD per switch.  Empty every set
    # except natural_log_exp_and_others (which contains Exp/Ln/Copy/Identity/
    # Square -- everything we use) so the chooser is forced onto one set;
    # positional set ids are preserved.
    import concourse.bacc as bacc
    from concourse import hw_specs
    if getattr(bacc.get_activation_tables, "_pinned", False):
        return
    orig = hw_specs.get_activation_tables
    KEEP = "natural_log_exp_and_others"

    def pinned(arch):
        t = dict(orig(arch))
        return {k: (v if k == KEEP else set()) for k, v in t.items()}

    pinned._pinned = True
    bacc.get_activation_tables = pinned


def build_nc(debug_stop=99):
    import concourse.bacc as bacc
    import concourse.mybir as mybir
    import concourse.tile as tile
    from concourse import masks as cmasks
    _pin_act_tables()
    NORM_MODE = os.environ.get("BASS_NORM", "recip")

    dtb = mybir.dt.bfloat16
    dtf = mybir.dt.float32
    dt8 = mybir.dt.float8e4
    AF = mybir.ActivationFunctionType
    ALU = mybir.AluOpType
    AX = mybir.AxisListType
    DR = mybir.MatmulPerfMode.DoubleRow

    nc = bacc.Bacc()

    def din(name, shape, dt=dtb):
        return nc.dram_tensor(name, shape, dt, kind="ExternalInput")

    xT_d = din("xT", [BPC, 128, 2, T])
    c0T_d = din("c0T", [BPC, 128, 2, T])
    c0n_d = din("c0n", [BPC, 128, NT, D_MODEL])
    wq_d = din("wq", [N_LAYERS, 128, 2, D_MODEL])
    wk_d = din("wk", [N_LAYERS, 128, 2, D_MODEL])
    wv_d = din("wv", [N_LAYERS, 128, 2, D_MODEL])
    wo_d = din("wo", [N_LAYERS, 128, 2, D_MODEL])
    mscT_d = din("mscT", [128, BPC * NT, NSEG])
    mgath_d = din("mgath", [NSEG, BPC, T])
    gpw_d = din("gpw", [128, 4, GP])
    gpbias_d = din("gpbias", [NSEG, BPC, GP], mybir.dt.float32)
    gain_d = din("gain", [GP, 3 * GP])
    gainb_d = din("gainb", [GP, 3], mybir.dt.float32)
    gaout_d = din("gaout", [GP, GP])
    gaoutb_d = din("gaoutb", [GP, 1], mybir.dt.float32)
    fw1_d = din("fw1", [GP, GP])
    fw2_d = din("fw2", [GP, GP])
    fb1_d = din("fb1", [1, GP])
    fb2_d = din("fb2", [1, GP])
    lnw1_d = din("lnw1", [NSEG, GP], mybir.dt.float32)
    lnb1_d = din("lnb1", [NSEG, GP], mybir.dt.float32)
    lnw2_d = din("lnw2", [NSEG, GP], mybir.dt.float32)
    lnb2_d = din("lnb2", [NSEG, GP], mybir.dt.float32)
    w1_d = din("w1", [128, 4, NE_PAD], dt8)
    w2c_d = din("w2c", [BPC, 128, 2, NE_PAD], dt8)
    tri_d = din("tri", [128, 128])
    out_d = nc.dram_tensor("out", [R, N_ENTITY], mybir.dt.bfloat16,
                           kind="ExternalOutput")
    debug_dump = bool(os.environ.get("BASS_DEBUG_DUMP"))
    if debug_dump:
        encdbg_d = nc.dram_tensor("encdbg", [N_LAYERS, 128, 2, T],
                                  mybir.dt.bfloat16, kind="ExternalOutput")
        odbg_d = nc.dram_tensor("odbg", [N_LAYERS, 128, 2, T],
                                mybir.dt.bfloat16, kind="ExternalOutput")

    with tile.TileContext(nc) as tc:
        with (
            tc.tile_pool(name="persist", bufs=1) as pp,
            tc.tile_pool(name="work", bufs=2) as wp,
            tc.tile_pool(name="acts", bufs=5) as ap,
            tc.tile_pool(name="outp", bufs=3) as op,
            tc.tile_pool(name="psum", bufs=1, space="PSUM") as ps,
        ):
            def pt(shape, tag, dt=dtb):
                return pp.tile(shape, dt, tag=tag, name=tag)

            def dma(dst, src):
                nc.sync.dma_start(dst, src)

            # PSUM budget (16KB/partition = 8 banks), by tag:
            #   pi 2x[128,1024]f32 = 8KB; f 2x[128,512]f32 = 4KB;
            #   po 2x[65,512]f32 = 4KB.  (pb lives in the f tag)
            def ps_pi():
                return ps.tile([128, 1024], dtf, tag="pi", bufs=2, name="pi")

            def ps_f(shape, dt=dtf):
                return ps.tile(shape, dt, tag="f", bufs=2, name="psf",
                               padded_shape=[128, 512])

            def ps_po():
                return ps.tile([65, T], dtf, tag="po", bufs=2, name="po")

            # ---- constants in ----
            # Input DMAs are split into small per-queue chunks, ordered so
            # batch 0 layer 0's operands land first: the front can start
            # ~5us in instead of waiting ~20us for monolithic transfers.
            xT = [pt([128, 2, T], f"xT{b}") for b in range(BPC)]
            c0T = [pt([128, 2, T], f"c0T{b}") for b in range(BPC)]
            c0n = [pt([128, NT, D_MODEL], f"c0n{b}") for b in range(BPC)]
            wq = [pt([128, 2, D_MODEL], f"wq{l}") for l in range(N_LAYERS)]
            wk = [pt([128, 2, D_MODEL], f"wk{l}") for l in range(N_LAYERS)]
            wo = [pt([128, 2, D_MODEL], f"wo{l}") for l in range(N_LAYERS)]
            wv = [pt([128, 2, D_MODEL], f"wv{l}") for l in range(N_LAYERS)]
            tri_s = pt([128, 128], "tri")

            def dma_qkv_w(l):
                dma(wq[l][:], wq_d[l])
                dma(wk[l][:], wk_d[l])
                dma(wv[l][:], wv_d[l])
                dma(wo[l][:], wo_d[l])

            def dma_batch_in(b):
                dma(c0T[b][:], c0T_d[b])
                dma(xT[b][:], xT_d[b])
                dma(c0n[b][:], c0n_d[b])

            dma_qkv_w(0)
            dma_batch_in(0)
            dma(tri_s[:], tri_d[:])
            dma_qkv_w(1)
            dma_batch_in(1)
            mscT_s = pt([128, BPC * NT, NSEG], "mscT")
            dma(mscT_s[:], mscT_d[:])
            mgath_s = pt([NSEG, BPC, T], "mgath")
            dma(mgath_s[:], mgath_d[:])
            gpw_s = pt([128, 4, GP], "gpw")
            dma(gpw_s[:], gpw_d[:])
            gpbias_s = pt([NSEG, BPC, GP], "gpbias", dtf)
            dma(gpbias_s[:], gpbias_d[:])
            gain_s = pt([GP, 3 * GP], "gain")
            dma(gain_s[:], gain_d[:])
            gainb_s = pt([GP, 3], "gainb", dtf)
            dma(gainb_s[:], gainb_d[:])
            gaout_s = pt([GP, GP], "gaout")
            dma(gaout_s[:], gaout_d[:])
            gaoutb_s = pt([GP, 1], "gaoutb", dtf)
            dma(gaoutb_s[:], gaoutb_d[:])
            fw1_s = pt([GP, GP], "fw1")
            dma(fw1_s[:], fw1_d[:])
            fw2_s = pt([GP, GP], "fw2")
            dma(fw2_s[:], fw2_d[:])
            fb1_s = pt([1, GP], "fb1")
            dma(fb1_s[:], fb1_d[:])
            fb2_s = pt([1, GP], "fb2")
            dma(fb2_s[:], fb2_d[:])
            ln_s = {}
            for nm, d in [("lnw1", lnw1_d), ("lnb1", lnb1_d),
                          ("lnw2", lnw2_d), ("lnb2", lnb2_d)]:
                ln_s[nm] = pt([NSEG, GP], nm, dtf)
                dma(ln_s[nm][:], d[:])

            # intensity weights, split into per-queue chunks so all DMA
            # rings pull in parallel and early strips land first
            w2c_s = [pt([128, 2, NE_PAD], f"w2c{b}", dt8) for b in range(BPC)]
            w1_s = pt([128, 4, NE_PAD], "w1", dt8)
            for c in range(2):
                for blk in range(4):
                    cs = slice(2048 * blk, 2048 * blk + 2048)
                    dma(w2c_s[0][:, c, cs], w2c_d[0, :, c, cs])
            for blk in range(4):
                for c in range(4):
                    cs = slice(2048 * blk, 2048 * blk + 2048)
                    dma(w1_s[:, c, cs], w1_d[:, c, cs])
            for c in range(2):
                for blk in range(4):
                    cs = slice(2048 * blk, 2048 * blk + 2048)
                    dma(w2c_s[1][:, c, cs], w2c_d[1, :, c, cs])

            eps_s = pt([NSEG, 1], "eps", dtf)
            nc.gpsimd.memset(eps_s[:], 1e-5)
            ident = pt([128, 128], "ident")
            cmasks.make_identity(nc, ident[:])
            ones_r = pt([1, T], "ones_r")
            nc.gpsimd.memset(ones_r[:], 1.0)

            # gather/const stationary: [128, 2, T] fp8; row (64, 0) is the
            # all-ones row pairing with the const row of w2c; rows 65..127
            # and all of j=1 stay zero.
            gathc8 = [pt([128, 2, T], f"gathc8{b}", dt8) for b in range(BPC)]
            enc8 = [pt([128, 2 * N_LAYERS, T], f"enc8{b}", dt8)
                    for b in range(BPC)]
            for b in range(BPC):
                nc.gpsimd.memset(gathc8[b][:], 0.0)
                nc.gpsimd.memset(gathc8[b][GP:GP + 1, 0, :], 1.0)

            def layernorm(xin, wtile, btile, outf, outb):
                P = xin.shape[0]
                s1 = wp.tile([P, 1], dtf, tag="lns", name="lns")
                nc.vector.reduce_sum(s1[:], xin[:], axis=AX.X)
                mu = wp.tile([P, 1], dtf, tag="lnm", name="lnm")
                nc.vector.tensor_scalar_mul(mu[:], s1[:], 1.0 / GP)
                xc = wp.tile([P, GP], dtf, tag="lnxc", name="lnxc")
                nc.vector.tensor_scalar(xc[:], xin[:], mu[:], None,
                                        op0=ALU.subtract)
                sq = wp.tile([P, GP], dtf, tag="lnsq", name="lnsq")
                vs = wp.tile([P, 1], dtf, tag="lnvs", name="lnvs")
                nc.scalar.activation(sq[:], xc[:], AF.Square, accum_out=vs[:])
                lnv = wp.tile([P, 1], dtf, tag="lnlv", name="lnlv")
                nc.scalar.activation(lnv[:], vs[:], AF.Ln, scale=1.0 / GP,
                                     bias=eps_s[:P])
                rstd = wp.tile([P, 1], dtf, tag="lnrs", name="lnrs")
                nc.scalar.activation(rstd[:], lnv[:], AF.Exp, scale=-0.5)
                nc.vector.scalar_tensor_tensor(
                    outf[:], xc[:], rstd[:], wtile[:], op0=ALU.mult, op1=ALU.mult)
                nc.vector.tensor_add(outf[:], outf[:], btile[:])
                nc.vector.tensor_copy(outb[:], outf[:])

            # ==== software-pipelined emission ====
            # Engines execute their instruction streams strictly in order,
            # so overlap between independent work REQUIRES interleaving at
            # emission time.  Schedule:
            #   front(b0) ; [post(b0) || front(b1)-begin] ;
            #   [intensity(b0) || front(b1)-rest + post(b1)] ; intensity(b1)
            ST = [{"curT": c0T[b], "curn": c0n[b],
                   "encn": [None] * N_LAYERS} for b in range(BPC)]

            def enc_gen(b):
                # Batch 0's front runs while ACT is idle: staging copies and
                # softmax normalization go on ACT to shorten the critical
                # chain.  Batch 1's front overlaps batch 0's ACT-bound
                # intensity epilogue, so everything movable stays on the DVE.
                on_act = (b == 0)

                def stage(dst, src):
                    nc.vector.tensor_copy(dst, src)

                for l in range(N_LAYERS):
                    curT, curn = ST[b]["curT"], ST[b]["curn"]
                    qT = ap.tile([128, 2, T], dtb, tag="qT", bufs=2, name="qT")
                    kT = ap.tile([128, 2, T], dtb, tag="kT", bufs=2, name="kT")
                    for c2 in range(2):
                        psq = ps_f([128, T])
                        for c in range(2):
                            nc.tensor.matmul(
                                psq[:], wq[l][:, c, 128 * c2:128 * c2 + 128],
                                curT[:, c, :], start=(c == 0), stop=(c == 1))
                        stage(qT[:, c2, :], psq[:])
                        psk = ps_f([128, T])
                        for c in range(2):
                            nc.tensor.matmul(
                                psk[:], wk[l][:, c, 128 * c2:128 * c2 + 128],
                                xT[b][:, c, :], start=(c == 0), stop=(c == 1))
                        stage(kT[:, c2, :], psk[:])
                        yield
                    vcat = []
                    for m in range(NT):
                        psv = ps_f([128, D_MODEL])
                        for c in range(2):
                            nc.tensor.matmul(
                                psv[:], xT[b][:, c, 128 * m:128 * m + 128],
                                wv[l][:, c, :], start=(c == 0), stop=(c == 1))
                        vc = ap.tile([128, N_HEADS, 65], dtb, tag=f"vcat{m}",
                                     bufs=2, name=f"vcat{m}")
                        stage(vc[:, :, 0:64],
                              psv[:].rearrange("p (h e) -> p h e", h=N_HEADS))
                        nc.vector.memset(vc[:, :, 64:65], 1.0)
                        vcat.append(vc)
                        if m == 1:
                            yield
                    yield
                    # Scores phase: all (h, j) score+exp+mask units emitted
                    # back-to-back so no engine queue ever stalls on a long
                    # cross-engine round-trip (engines run their queues in
                    # order; a waiting instruction blocks everything behind
                    # it, including the other batch's interleaved intensity
                    # work).
                    aTs = [[None] * NT for _ in range(N_HEADS)]
                    cnt = 0
                    for h in range(N_HEADS):
                        bp, hc = 64 * (h % 2), h // 2
                        for j in range(2):
                            q0 = 128 * j
                            nq = T - q0
                            pss = ps_f([128, T])
                            nc.tensor.matmul(
                                pss[:, :nq],
                                kT[bp:bp + 64, hc, q0:q0 + 128],
                                qT[bp:bp + 64, hc, q0:T])
                            aT = ap.tile([128, T], dtb, tag=f"aT{b}{h}{j}",
                                         bufs=1, name="aT")
                            nc.scalar.activation(aT[:, :nq], pss[:, :nq], AF.Exp)
                            nc.vector.tensor_mul(
                                aT[:, 0:128], aT[:, 0:128], tri_s[:])
                            aTs[h][j] = aT[:, :nq]
                            cnt += 1
                            if cnt % 3 == 0:
                                yield
                        # j=2 (256 cols) and j=3 (128 cols) share one PSUM
                        # tile and one Exp: j2 at cols [0:256], j3 at
                        # [256:384] (partition dim = k-pos local to each j).
                        pss = ps_f([128, T])
                        nc.tensor.matmul(
                            pss[:, 0:256], kT[bp:bp + 64, hc, 256:384],
                            qT[bp:bp + 64, hc, 256:T],
                            skip_group_check=True)
                        nc.tensor.matmul(
                            pss[:, 256:384], kT[bp:bp + 64, hc, 384:512],
                            qT[bp:bp + 64, hc, 384:T],
                            skip_group_check=True)
                        aT23 = ap.tile([128, T], dtb, tag=f"aT{b}{h}2",
                                       bufs=1, name="aT23")
                        nc.scalar.activation(aT23[:, 0:384], pss[:, 0:384],
                                             AF.Exp)
                        nc.vector.tensor_mul(
                            aT23[:, 0:128], aT23[:, 0:128], tri_s[:])
                        nc.vector.tensor_mul(
                            aT23[:, 256:384], aT23[:, 256:384], tri_s[:])
                        aTs[h][2] = aT23[:, 0:256]
                        aTs[h][3] = aT23[:, 256:384]
                        cnt += 1
                        if cnt % 3 == 0:
                            yield
                    # po phase per head, then softmax normalization: 1/colsum
                    # on DVE (keeps ACT free for softplus), rank-1 broadcast
                    # matmul, multiply.
                    oT = ap.tile([128, 2, T], dtb, tag="oT", bufs=2, name="oT")
                    for h in range(N_HEADS):
                        bp, hc = 64 * (h % 2), h // 2
                        po = ps_po()
                        for j in range(NT):
                            q0 = 128 * j
                            nc.tensor.matmul(
                                po[:, q0:T], vcat[j][:, h, :],
                                aTs[h][j][:, :T - q0],
                                start=(j == 0), stop=(j == NT - 1),
                                skip_group_check=True)
                        oraw = wp.tile([64, T], dtb, tag="oraw", name="oraw")
                        stage(oraw[:], po[0:64, :])
                        rbf = wp.tile([1, T], dtb, tag="rbf", name="rbf")
                        if on_act or NORM_MODE == "act":
                            rsb = wp.tile([1, T], dtf, tag="rsb", name="rsb")
                            nc.scalar.activation(rsb[:], po[64:65, :], AF.Ln)
                            nc.scalar.activation(rbf[:], rsb[:], AF.Exp,
                                                 scale=-1.0)
                        else:
                            # reciprocal_approx_fast is a custom DVE op
                            # (BITWISE_NOT seed); it reads garbage from PSUM,
                            # so stage the colsum row into SBUF first.
                            rcs = wp.tile([1, T], dtf, tag="rcs", name="rcs")
                            nc.vector.tensor_copy(rcs[:], po[64:65, :])
                            rr = wp.tile([1, T], dtf, tag="rr", name="rr")
                            nc.vector.reciprocal_approx_fast(rr[:], rcs[:])
                            nc.vector.tensor_copy(rbf[:], rr[:])
                        pb = ps_f([64, T])
                        nc.tensor.matmul(pb[:], ones_r[0:1, 0:64], rbf[:])
                        nc.vector.tensor_mul(oT[bp:bp + 64, hc, :], oraw[:],
                                             pb[:])
                        yield
                    if debug_dump and b == 0:
                        nc.sync.dma_start(odbg_d[l], oT[:])
                    eT = pt([128, 2, T], f"encT{l}{b}")
                    en = pt([128, NT, D_MODEL], f"encn{l}{b}")
                    for c2 in range(2):
                        psp = ps_f([128, T])
                        for c in range(2):
                            nc.tensor.matmul(
                                psp[:], wo[l][:, c, 128 * c2:128 * c2 + 128],
                                oT[:, c, :], start=(c == 0), stop=(c == 1))
                        nc.vector.tensor_add(eT[:, c2, :], curT[:, c2, :],
                                             psp[:])
                    nc.vector.tensor_copy(enc8[b][:, 2 * l:2 * l + 2, :],
                                          eT[:])
                    if debug_dump and b == 0:
                        nc.sync.dma_start(encdbg_d[l], eT[:])
                    yield
                    for m in range(NT):
                        psn = ps_f([128, D_MODEL])
                        for c in range(2):
                            nc.tensor.matmul(
                                psn[:], oT[:, c, 128 * m:128 * m + 128],
                                wo[l][:, c, :], start=(c == 0), stop=(c == 1))
                        nc.vector.tensor_add(en[:, m, :], curn[:, m, :], psn[:])
                        if m == 1:
                            yield
                    ST[b]["encn"][l] = en
                    ST[b]["curT"], ST[b]["curn"] = eT, en
                    yield

            def post_gen(b):
                encn = ST[b]["encn"]
                # ---- group scatter A^T[d, seg] (batch-local 64 segs) ----
                AT_s = wp.tile([128, 4, NSEG], dtb, tag="AT", name="AT")
                for dt_i in range(4):
                    l, c2 = dt_i // 2, dt_i % 2
                    pa = ps_f([128, NSEG])
                    for m in range(NT):
                        nc.tensor.matmul(
                            pa[:], encn[l][:, m, 128 * c2:128 * c2 + 128],
                            mscT_s[:, NT * b + m, :],
                            start=(m == 0), stop=(m == NT - 1))
                    nc.vector.tensor_copy(AT_s[:, dt_i, :], pa[:])
                    if dt_i % 2:
                        yield
                # ---------------- group block ([64, *]) ----------------
                pg = ps_f([NSEG, GP])
                for dt_i in range(4):
                    nc.tensor.matmul(pg[:], AT_s[:, dt_i, :], gpw_s[:, dt_i, :],
                                     start=(dt_i == 0), stop=(dt_i == 3))
                gp_f = wp.tile([NSEG, GP], dtf, tag="gp_f", name="gp_f")
                nc.vector.tensor_add(gp_f[:], pg[:], gpbias_s[:, b, :])
                gp_bf = wp.tile([NSEG, GP], dtb, tag="gp_bf", name="gp_bf")
                nc.vector.tensor_copy(gp_bf[:], gp_f[:])
                ptr = ps_f([GP, NSEG], dtb)
                nc.tensor.transpose(ptr[:], gp_bf[:], ident[0:NSEG, 0:NSEG])
                gpT = wp.tile([GP, NSEG], dtb, tag="gpT", name="gpT")
                nc.vector.tensor_copy(gpT[:], ptr[:])
                yield
                qkvT = []
                for i in range(3):
                    pq = ps_f([GP, NSEG])
                    nc.tensor.matmul(pq[:], gain_s[:, GP * i:GP * i + GP],
                                     gpT[:])
                    tq = wp.tile([GP, NSEG], dtb, tag=f"qkvT{i}",
                                 name=f"qkvT{i}")
                    nc.vector.tensor_scalar(
                        tq[:], pq[:], gainb_s[:, i:i + 1], None, op0=ALU.add)
                    qkvT.append(tq)
                yield
                vc2 = wp.tile([GP, GH, 33], dtb, tag="vc2", name="vc2")
                for h in range(GH):
                    pv = ps_f([GP, 32], dtb)
                    nc.tensor.transpose(
                        pv[:], qkvT[2][32 * h:32 * h + 32, :],
                        ident[32 * h:32 * h + 32, 32 * h:32 * h + 32])
                    nc.vector.tensor_copy(vc2[:, h, 0:32], pv[:])
                nc.vector.memset(vc2[:, :, 32:33], 1.0)
                yield
                oT2 = wp.tile([GP, NSEG], dtb, tag="oT2", name="oT2")
                for h in range(GH):
                    ps1 = ps_f([64, 64])
                    nc.tensor.matmul(
                        ps1[:], qkvT[1][32 * h:32 * h + 32, :],
                        qkvT[0][32 * h:32 * h + 32, :])
                    a2 = wp.tile([64, 64], dtb, tag="a2", name="a2")
                    nc.scalar.activation(a2[:], ps1[:], AF.Exp)
                    po2 = ps_f([33, 64])
                    nc.tensor.matmul(po2[:], vc2[:, h, :], a2[:])
                    r2c = wp.tile([1, 64], dtf, tag="r2c", name="r2c")
                    nc.vector.tensor_copy(r2c[:], po2[32:33, :])
                    r2 = wp.tile([1, 64], dtf, tag="r2", name="r2")
                    nc.vector.reciprocal_approx_fast(r2[:], r2c[:])
                    r2b = wp.tile([1, 64], dtb, tag="r2b", name="r2b")
                    nc.vector.tensor_copy(r2b[:], r2[:])
                    pb2 = ps_f([32, 64])
                    nc.tensor.matmul(pb2[:], ones_r[0:1, 0:32], r2b[:])
                    orw2 = wp.tile([32, 64], dtb, tag="orw2", name="orw2")
                    nc.vector.tensor_copy(orw2[:], po2[0:32, :])
                    nc.vector.tensor_mul(oT2[32 * h:32 * h + 32, :], orw2[:],
                                         pb2[:])
                    yield
                pga = ps_f([GP, NSEG])
                nc.tensor.matmul(pga[:], gaout_s[:], oT2[:])
                t1 = wp.tile([GP, NSEG], dtb, tag="t1", name="t1")
                nc.vector.scalar_tensor_tensor(
                    t1[:], pga[:], gaoutb_s[:, 0:1], gpT[:],
                    op0=ALU.add, op1=ALU.add)
                px1 = ps_f([NSEG, GP], dtb)
                nc.tensor.transpose(px1[:], t1[:], ident[0:GP, 0:GP])
                x1 = wp.tile([NSEG, GP], dtf, tag="x1", name="x1")
                nc.vector.tensor_copy(x1[:], px1[:])
                gn_f = wp.tile([NSEG, GP], dtf, tag="gn_f", name="gn_f")
                gn_b = wp.tile([NSEG, GP], dtb, tag="gn_b", name="gn_b")
                layernorm(x1, ln_s["lnw1"][0:NSEG], ln_s["lnb1"][0:NSEG],
                          gn_f, gn_b)
                yield
                pgt = ps_f([GP, NSEG], dtb)
                nc.tensor.transpose(pgt[:], gn_b[:], ident[0:NSEG, 0:NSEG])
                gnT = wp.tile([GP, NSEG], dtb, tag="gnT", name="gnT")
                nc.vector.tensor_copy(gnT[:], pgt[:])
                ph1 = ps_f([NSEG, GP])
                nc.tensor.matmul(ph1[:], ones_r[0:1, 0:NSEG], fb1_s[:],
                                 start=True, stop=False)
                nc.tensor.matmul(ph1[:], gnT[:], fw1_s[:], start=False,
                                 stop=True)
                h1b = wp.tile([NSEG, GP], dtb, tag="h1b", name="h1b")
                nc.vector.tensor_scalar_max(h1b[:], ph1[:], 0.0)
                ph1t = ps_f([GP, NSEG], dtb)
                nc.tensor.transpose(ph1t[:], h1b[:], ident[0:NSEG, 0:NSEG])
                h1T = wp.tile([GP, NSEG], dtb, tag="h1T", name="h1T")
                nc.vector.tensor_copy(h1T[:], ph1t[:])
                yield
                ph2 = ps_f([NSEG, GP])
                nc.tensor.matmul(ph2[:], ones_r[0:1, 0:NSEG], fb2_s[:],
                                 start=True, stop=False)
                nc.tensor.matmul(ph2[:], h1T[:], fw2_s[:], start=False,
                                 stop=True)
                x2 = wp.tile([NSEG, GP], dtf, tag="x2", name="x2")
                nc.vector.tensor_add(x2[:], ph2[:], gn_f[:])
                go_f = wp.tile([NSEG, GP], dtf, tag="go_f", name="go_f")
                go_b = wp.tile([NSEG, GP], dtb, tag="go_b", name="go_b")
                layernorm(x2, ln_s["lnw2"][0:NSEG], ln_s["lnb2"][0:NSEG],
                          go_f, go_b)
                yield
                # ---- gather^T: gathc8[p, 0, t] = gout[gid(t), p]*fm(t) ----
                pgh = ps_f([GP, T])
                nc.tensor.matmul(pgh[:], go_b[:], mgath_s[:, b, :])
                nc.vector.tensor_copy(gathc8[b][0:GP, 0, :], pgh[:])
                yield

            def int_gen(b):
                # -------- intensity head (fp8 DoubleRow, K=512+65) --------
                for P in range(NPAIR):
                    for m in range(NT):
                        ot = op.tile([128, 2048], dtb, tag="ot", name="ot")
                        width = 0
                        for s2 in range(2):
                            sp = 2 * P + s2
                            ncs = min(1024, N_ENTITY - 1024 * sp)
                            pi = ps_pi()
                            for h2 in range(2):
                                col0 = 1024 * sp + 512 * h2
                                ncol = min(512, N_ENTITY - col0)
                                if ncol <= 0:
                                    continue
                                pv = pi[:, 512 * h2:512 * h2 + ncol]
                                for c in range(2):
                                    nc.tensor.matmul(
                                        pv,
                                        enc8[b][:, 2 * c:2 * c + 2,
                                                128 * m:128 * m + 128],
                                        w1_s[:, 2 * c:2 * c + 2,
                                             col0:col0 + ncol],
                                        perf_mode=DR, start=(c == 0),
                                        stop=False, skip_group_check=True)
                                nc.tensor.matmul(
                                    pv,
                                    gathc8[b][:, :, 128 * m:128 * m + 128],
                                    w2c_s[b][:, :, col0:col0 + ncol],
                                    perf_mode=DR, start=False, stop=True,
                                    skip_group_check=True)
                            nc.scalar.activation(
                                ot[:, 1024 * s2:1024 * s2 + ncs],
                                pi[:, :ncs], AF.Exp, scale=1.0 / INTW_SCALE)
                            width += ncs
                        # softplus second half: one Ln per strip-pair
                        nc.scalar.activation(ot[:, :width], ot[:, :width],
                                             AF.Ln, bias=1.0)
                        nc.sync.dma_start(
                            out_d[T * b + 128 * m:T * b + 128 * m + 128,
                                  2048 * P:2048 * P + width],
                            ot[:, :width])
                        yield

            def drain(g):
                for _ in g:
                    pass

            def zip_all(*gens):
                gens = list(gens)
                while gens:
                    for g in list(gens):
                        if next(g, StopIteration) is StopIteration:
                            gens.remove(g)

            def zip_until(primary, filler):
                # pace on `primary`; advance `filler` one stage per step
                for _ in primary:
                    next(filler, None)

            # Schedule: front(b0) alone; post(b0) paced against front(b1)'s
            # beginning (b1's short-dependency q/k/score chunks execute on
            # the otherwise-idle PE while post(b0)'s serial group-block chain
            # runs); then intensity(b0) interleaved 1:2 with the REST of
            # front(b1)+post(b1) -- intensity units lead so their ACT work
            # is never queued behind unissued front(b1) instructions, and
            # the 2x g1 stepping lets post(b1) finish before intensity(b0)
            # drains so intensity(b1) starts immediately after.
            drain(enc_gen(0))
            g1 = (_chain_gens(enc_gen(1), post_gen(1))
                  if BPC > 1 else iter(()))
            if debug_stop >= 3:
                zip_until(post_gen(0), g1)
                if debug_stop >= 5:
                    # Intensity(b0) units lead the remaining front(b1)
                    # chunks 1:2 -- the units' ACT work is never queued
                    # behind unissued front(b1) instructions, and the double
                    # g1 stepping lets post(b1) finish before intensity(b0)
                    # drains.
                    gi0 = int_gen(0)
                    next(gi0, None)
                    next(gi0, None)
                    while True:
                        if next(gi0, StopIteration) is StopIteration:
                            gi0 = None
                            break
                        if (next(g1, StopIteration) is StopIteration or
                                next(g1, StopIteration) is StopIteration):
                            break
                    drain(g1)
                    if gi0 is not None:
                        drain(gi0)
                    for bb in range(1, BPC):
                        drain(int_gen(bb))
                else:
                    drain(g1)
            elif BPC > 1:
                drain(g1)
    nc.compile()
    return nc


def _get_nc():
    if "nc" not in _CACHED:
        _CACHED["nc"] = build_nc()
    return _CACHED["nc"]


def _install_ntff_hook():
    """Best-effort: register the axon NTFF profile hook so trace=True works."""
    import sys, types
    if "antenv.axon_hooks" in sys.modules:
        return
    try:
        import antenv  # noqa
        from trn_agent_boot.trn_boot import _ntff_profile_via_ctypes
        mod = types.ModuleType("antenv.axon_hooks")
        hook = [_ntff_profile_via_ctypes("/opt/axon/libaxon_pjrt.so")]
        mod.set_axon_ntff_profile_hook = lambda h: hook.__setitem__(0, h)
        mod.get_axon_ntff_profile_hook = lambda: hook[0]
        sys.modules["antenv.axon_hooks"] = mod
    except Exception:
        pass


def kernel(**inputs):
    global LAST_EXEC_NS, LAST_RESULTS
    from concourse.bass_utils import run_bass_kernel_spmd

    in_maps = prep_inputs(inputs)
    nc = _get_nc()
    trace = bool(os.environ.get("BASS_TRACE"))
    if trace:
        _install_ntff_hook()
    res = run_bass_kernel_spmd(
        nc, in_maps, core_ids=list(range(NCORES)), trace=trace)
    LAST_RESULTS = res
    LAST_EXEC_NS = res.exec_time_ns
    out = np.empty((B, Lh, N_ENTITY), np.float32)
    for core in range(NCORES):
        o = res.results[core]["out"]
        for b in range(BPC):
            out[core * BPC + b] = o[T * b:T * b + Lh, :].astype(np.float32)
    return out
